# revision 15
# baseline (speedup 1.0000x reference)
"""Trainium2 Bass kernel for nn_ExpValCircuitGraphModel (GNN message passing).

Sharding: data-parallel — one graph per NeuronCore (B=8 graphs on 8 cores).
Host does graph-format conversion only (dense 0/1 masks from edge_index,
parameter repacking); all model compute runs on-device.

Device algorithm (validated against the jax reference on host, rel err 6e-5):
- TransformerConv: scores^T on PE; softmax without max-shift; q/k/v/e bf16;
  e consumed per source block by the attention matmul, which also accumulates
  the softmax denominator via an appended ones-column on v.
- ASAP masked-max via mask-matmul log-sum-exp (per-feature shift, p=20);
  output measured insensitive to masked-max error up to +-0.3.
- exp(leaky_relu(y)) == max(exp(y), exp(0.2 y)).
- fitness sigmoid as 1/(1+exp(-x)) for exact fp32 saturation; top-k via stable
  rank (ties broken by index like jax.lax.top_k); selection as one-hot P.
- A2 = S_sel^T A S_sel needed only as boolean -> bf16 0/1 count matmuls.
- global_mean_pool of the selected half as a fitness-weighted matmul.

Conv outputs live in a [64, heads, n] transposed layout (partition rows 0:64)
so every attention/normalize op is partition-aligned.
"""
import numpy as np

import concourse.bass as bass
import concourse.tile as tile
from concourse import bacc, mybir
from concourse.bass_utils import run_bass_kernel_spmd
from concourse.masks import make_identity

F32, BF16, F32R = mybir.dt.float32, mybir.dt.bfloat16, mybir.dt.float32r
AF = mybir.ActivationFunctionType
ALU = mybir.AluOpType
AX = mybir.AxisListType

B, N1, F0 = 8, 1024, 32
H1, H2 = 5, 3
HD1, HD2 = 320, 192
DH = 64
N2 = 512
PEXP = 20.0
EPS_DEN = 1e-30
NB1, NB2 = 8, 4
KBS1, KBS2 = [F0], [128, 128, 64]     # conv input feature blocks


def _r(ap):
    # fp32r needs producer-side rounding (walrus invariant); plain fp32 for now
    return ap


def build_program(scal):
    nc = bacc.Bacc("TRN2", target_bir_lowering=False, debug=False, num_devices=8)
    din = {}

    def inp(name, shape, dtype=F32):
        din[name] = nc.dram_tensor(name, shape, dtype, kind="ExternalInput").ap()

    inp("xT0", [F0, N1])
    inp("mask1c", [N1, N1], BF16)
    inp("mask1", [N1, N1], BF16)
    inp("mask1T", [N1, N1], BF16)
    inp("deg1c", [N1, 1]); inp("evcd", [5, 1])
    inp("wq1", [128, HD1]); inp("wk1", [128, HD1])
    inp("wv1a", [128, H1 * (DH + 1)]); inp("wsk1", [128, HD1])
    inp("bq1c", [3 * 128, 1]); inp("bk1c", [3 * 128, 1])
    inp("bvsk1c", [H1 * DH, 1])
    inp("attx1", [H1 * DH, 1]); inp("w1bc", [128, HD1]); inp("lew1", [H1 * DH, 3])
    inp("wq2", [3 * 128, HD2]); inp("wk2", [3 * 128, HD2])
    inp("wv2a", [3 * 128, H2 * (DH + 1)]); inp("wsk2", [3 * 128, HD2])
    inp("bq2c", [2 * 128, 1]); inp("bk2c", [2 * 128, 1])
    inp("bvsk2c", [H2 * DH, 1])
    inp("attx2", [H2 * DH, 1]); inp("w2bc", [128, HD2]); inp("lew2", [H2 * DH, 3])
    inp("mw1", [2 * 128, HD1]); inp("mw2", [3 * 128, HD1]); inp("mw3", [3 * 128, 4])
    inp("mb1c", [3 * 128, 1]); inp("mb2c", [3 * 128, 1]); inp("mb3c", [4, 1])
    inp("iotabc", [128, N1]); inp("iotac", [N1, 1])
    out_d = nc.dram_tensor("out", [4, 1], F32, kind="ExternalOutput").ap()

    with tile.TileContext(nc) as tc:
        from contextlib import ExitStack
        with ExitStack() as ctx:
            _Prog(ctx, tc, nc, din, scal).run(out_d)
    nc.compile()
    return nc


class _Prog:
    def __init__(self, ctx, tc, nc, din, scal):
        self.ctx, self.tc, self.nc, self.din, self.scal = ctx, tc, nc, din, scal
        self.const = ctx.enter_context(tc.tile_pool(name="const", bufs=1))
        self.big = ctx.enter_context(tc.tile_pool(name="big", bufs=1))
        self.work = ctx.enter_context(tc.tile_pool(name="work", bufs=1))
        self.sm = ctx.enter_context(tc.tile_pool(name="sm", bufs=1))
        self.pipe = ctx.enter_context(tc.tile_pool(name="pipe", bufs=2))

    def load(self, name, shape, dtype=F32, pool=None, rearr=None, tag=None, p=128):
        pool = pool or self.const
        t = pool.tile(shape, dtype, tag=tag or name)
        src = self.din[name]
        if rearr is not None:
            src = src.rearrange(rearr, p=p)
        self.nc.sync.dma_start(out=t[:], in_=src)
        return t

    def bcast(self, pool_ps, row_ap, width, tag):
        nc = self.nc
        pb = pool_ps.tile([128, width], F32, tag="ps_bc")
        for t0 in range(0, width, 512):
            t1 = min(width, t0 + 512)
            nc.tensor.matmul(pb[:, t0:t1], self.onesr[:], row_ap[:, t0:t1],
                             start=True, stop=True)
        sb = self.sm.tile([128, width], F32, tag=tag)
        nc.scalar.copy(out=sb[:], in_=pb[:])
        return sb

    def cols2row(self, pool_ps, col3, blksizes, tag):
        nc = self.nc
        width = sum(blksizes)
        row = self.sm.tile([1, width], F32, tag=tag)
        o = 0
        for b, w in enumerate(blksizes):
            pt = pool_ps.tile([1, 128], F32, tag="ps_c2r")
            nc.tensor.transpose(pt[:, :w], col3[:w, b], self.identity[:w, :w])
            nc.scalar.copy(out=row[:, o: o + w], in_=pt[:, :w])
            o += w
        return row

    # ------------------------------------------------------------------
    def conv(self, xT, kbs, n, nb, heads, qkobs, wq, wk, wva, wsk,
             bqc, bkc, bvskc, mask_bf, sfx):
        """xT [<=128, KB, n] fp32 input (transposed). Returns xoT [64, heads, n]
        fp32 in work tag 'xoT': per-head feature rows at partitions 0:64."""
        nc, tc = self.nc, self.tc
        KB = len(kbs)
        nsl = [slice(t0, min(n, t0 + 512)) for t0 in range(0, n, 512)]
        qT = self.work.tile([128, len(qkobs), n], BF16, tag="qT")
        kT = self.work.tile([128, len(qkobs), n], BF16, tag="kT")
        xoT = self.work.tile([DH, heads, n], F32, tag="xoT")
        vaug = self.work.tile([128, nb, heads * (DH + 1)], BF16, tag="vaug")

        with tc.tile_pool(name="cvA" + sfx, bufs=2, space="PSUM") as pA, \
             tc.tile_pool(name="cvB" + sfx, bufs=1, space="PSUM") as pB, \
             tc.tile_pool(name="cvC" + sfx, bufs=1, space="PSUM") as pC:
            # q/k projections -> bf16 [128, OB, n]
            for w, dst, bias in ((wq, qT, bqc), (wk, kT, bkc)):
                for m, ob in enumerate(qkobs):
                    pm = pA.tile([128, n], F32, tag="A")
                    for sl in nsl:
                        for kb in range(KB):
                            nc.tensor.matmul(pm[:ob, sl],
                                             _r(w[: kbs[kb], kb, m * 128: m * 128 + ob]),
                                             _r(xT[: kbs[kb], kb, sl]),
                                             start=(kb == 0), stop=(kb == KB - 1))
                    nc.vector.tensor_scalar_add(out=dst[:ob, m], in0=pm[:ob],
                                                scalar1=bias[:ob, m])
            # skip projection -> xoT per head block [64, h, n]
            for h in range(heads):
                pm = pA.tile([128, n], F32, tag="A")
                for sl in nsl:
                    for kb in range(KB):
                        nc.tensor.matmul(pm[:DH, sl],
                                         _r(wsk[: kbs[kb], kb, h * DH:(h + 1) * DH]),
                                         _r(xT[: kbs[kb], kb, sl]),
                                         start=(kb == 0), stop=(kb == KB - 1))
                nc.scalar.copy(out=xoT[:, h], in_=pm[:DH])
            # v augmented
            for sb in range(nb):
                pv = pA.tile([128, n], F32, tag="A")
                w_ = heads * (DH + 1)
                for kb in range(KB):
                    nc.tensor.matmul(pv[:, :w_],
                                     _r(xT[: kbs[kb], kb, sb * 128:(sb + 1) * 128]),
                                     _r(wva[: kbs[kb], kb]),
                                     start=(kb == 0), stop=(kb == KB - 1))
                nc.scalar.copy(out=vaug[:, sb], in_=pv[:, :w_])
            v4 = vaug[:].rearrange("p b (h x) -> p b h x", h=heads)
            nc.gpsimd.memset(v4[:, :, :, DH: DH + 1], 1.0)

            isq = float(1.0 / np.sqrt(DH))
            for h in range(heads):
                mt, mo = divmod(h * DH, 128)
                pa = pB.tile([DH + 1, n], F32, tag="B")
                for sb in range(nb):
                    psc = pA.tile([128, n], F32, tag="A")
                    for sl in nsl:
                        nc.tensor.matmul(psc[:, sl],
                                         kT[mo: mo + DH, mt, sb * 128:(sb + 1) * 128],
                                         qT[mo: mo + DH, mt, sl],
                                         start=True, stop=True)
                    eb = self.pipe.tile([128, n], BF16, tag="eblk")
                    nc.scalar.activation(out=eb[:], in_=psc[:], func=AF.Exp, scale=isq)
                    nc.vector.tensor_tensor(out=eb[:], in0=eb[:], in1=mask_bf[:, sb],
                                            op=ALU.mult)
                    for sl in nsl:
                        nc.tensor.matmul(pa[:, sl], v4[:, sb, h, :], eb[:, sl],
                                         start=(sb == 0), stop=(sb == nb - 1))
                # den lives at partition DH(=64): recip there, broadcast via PE
                inv65 = self.sm.tile([DH + 1, n], F32, tag="inv65")
                nc.vector.tensor_scalar_add(out=inv65[DH: DH + 1], in0=pa[DH: DH + 1],
                                            scalar1=EPS_DEN)
                nc.vector.reciprocal(out=inv65[DH: DH + 1], in_=inv65[DH: DH + 1])
                pbc = pC.tile([DH, n], F32, tag="C")
                for sl in nsl:
                    nc.tensor.matmul(pbc[:, sl], self.ones65[DH: DH + 1, :DH],
                                     inv65[DH: DH + 1, sl], start=True, stop=True)
                invbc = self.sm.tile([DH, n], F32, tag="invbc")
                nc.scalar.copy(out=invbc[:], in_=pbc[:])
                att = self.sm.tile([DH, n], F32, tag="attnrm")
                nc.vector.tensor_tensor(out=att[:], in0=pa[:DH], in1=invbc[:],
                                        op=ALU.mult)
                nc.vector.tensor_tensor(out=xoT[:, h], in0=xoT[:, h], in1=att[:],
                                        op=ALU.add)
            for h in range(heads):
                nc.vector.tensor_scalar_add(out=xoT[:, h], in0=xoT[:, h],
                                            scalar1=bvskc[:, h])
        return xoT

    def transpose_to_aug(self, xT, n, nb, heads, hd, tag):
        nc, tc = self.nc, self.tc
        xaug = self.work.tile([128, nb, hd + 4], F32, tag=tag)
        with tc.tile_pool(name="trp" + tag + str(n), bufs=2, space="PSUM") as pT:
            for sb in range(nb):
                for h in range(heads):
                    pt = pT.tile([128, DH], F32, tag="T")
                    nc.tensor.transpose(pt[:, :],
                                        xT[:, h, sb * 128:(sb + 1) * 128],
                                        self.identity[:DH, :DH])
                    nc.scalar.copy(out=xaug[:, sb, h * DH:(h + 1) * DH], in_=pt[:])
        return xaug

    # ------------------------------------------------------------------
    def pool(self, n, nb, heads, hd, xT, xaug, mask_bf, degc, attxc, wbc, lewc,
             cst, leb1, leb3, negleb3c, sfx, final):
        """xT: [64, heads, n] conv output; xaug [128, nb, hd+4] normal layout."""
        import os
        nc, tc = self.nc, self.tc
        sub = os.environ.get("POOL_STOP", "") if sfx == "1" else ""
        k = n // 2
        with tc.tile_pool(name="plA" + sfx, bufs=2, space="PSUM") as pA, \
             tc.tile_pool(name="plB" + sfx, bufs=1, space="PSUM") as pB, \
             tc.tile_pool(name="plS" + sfx, bufs=2, space="PSUM") as pS:
            g = self.sm.tile([DH, heads, 1], F32, tag="gcol")
            for h in range(heads):
                nc.vector.reduce_max(out=g[:, h], in_=xT[:, h], axis=AX.X)
            if sub == "g0":
                return g, g, g
            grow = self.cols2row(pS, g, [DH] * heads, "grow")
            gbc = self.bcast(pB, grow[:], hd, "gbc")
            pgbc = self.sm.tile([128, hd], F32, tag="pgbc")
            nc.vector.tensor_scalar_mul(out=pgbc[:], in0=gbc[:], scalar1=PEXP)
            if sub == "g":
                return g, g, g
            E = self.work.tile([128, nb, hd], BF16, tag="E")
            for sb in range(nb):
                y = self.pipe.tile([128, hd], F32, tag="yE")
                nc.vector.tensor_tensor(out=y[:], in0=xaug[:, sb, :hd], in1=gbc[:],
                                        op=ALU.subtract)
                nc.scalar.activation(out=E[:, sb], in_=y[:], func=AF.Exp, scale=PEXP)
            if sub == "E":
                return g, g, g
            stcol = self.sm.tile([128, nb, 1], F32, tag="stcol")
            for tb in range(nb):
                pL = pA.tile([128, hd], F32, tag="A")
                for sb in range(nb):
                    nc.tensor.matmul(pL[:], mask_bf[:, sb, tb * 128:(tb + 1) * 128],
                                     E[:, sb], start=(sb == 0), stop=(sb == nb - 1))
                L = self.pipe.tile([128, hd], F32, tag="Llse")
                nc.scalar.activation(out=L[:], in_=pL[:], func=AF.Ln)
                nc.vector.tensor_tensor(out=L[:], in0=L[:], in1=pgbc[:], op=ALU.add)
                scr = self.pipe.tile([128, hd], F32, tag="scrL")
                nc.vector.tensor_tensor(out=scr[:], in0=L[:], in1=wbc[:, :hd],
                                        op=ALU.mult)
                nc.vector.tensor_scalar_mul(out=scr[:], in0=scr[:],
                                            scalar1=float(1.0 / PEXP))
                nc.vector.reduce_sum(out=stcol[:, tb], in_=scr[:], axis=AX.X)
            strow = self.cols2row(pS, stcol, [128] * nb, "strow")
            stbc = self.bcast(pB, strow[:], n, "stfbc")
            sscol = self.sm.tile([128, nb, 1], F32, tag="sscol")
            ss2col = self.sm.tile([128, nb, 1], F32, tag="ss2col")
            for sb in range(nb):
                pss = pS.tile([128, 4], F32, tag="ps_s4")
                for h in range(heads):
                    nc.tensor.matmul(pss[:, 0:1],
                                     _r(xT[:, h, sb * 128:(sb + 1) * 128]),
                                     _r(attxc[:, h]),
                                     start=(h == 0), stop=(h == heads - 1))
                nc.vector.tensor_scalar_add(out=sscol[:, sb], in0=pss[:, 0:1],
                                            scalar1=float(cst))
                nc.vector.tensor_scalar_mul(out=ss2col[:, sb], in0=sscol[:, sb],
                                            scalar1=0.2)
            for sb in range(nb):
                pxw = pS.tile([128, 4], F32, tag="ps_s4")
                for h in range(heads):
                    nc.tensor.matmul(pxw[:, 0:3],
                                     _r(xT[:, h, sb * 128:(sb + 1) * 128]),
                                     _r(lewc[:, h]),
                                     start=(h == 0), stop=(h == heads - 1))
                nc.scalar.copy(out=xaug[:, sb, hd: hd + 3], in_=pxw[:, 0:3])
            nc.gpsimd.memset(xaug[:, :, hd + 3: hd + 4], 1.0)
        if sub == "lse":
            return stcol[:, 0:1], stcol[:, 0:1], stcol[:, 0:1]
        xnew = self.work.tile([128, nb, hd], F32, tag="xnew")
        dots = self.sm.tile([128, nb, 3], F32, tag="dots")
        acol = self.sm.tile([128, nb, 1], F32, tag="acol")
        with tc.tile_pool(name="plN" + sfx, bufs=1, space="PSUM") as pN:
            pxn = [pN.tile([128, hd + 4], F32, tag=f"xn{tb}", name=f"pxn{tb}") for tb in range(nb)]
            for sb in range(nb):
                e1 = self.pipe.tile([128, n], F32, tag="e1", bufs=1)
                nc.scalar.activation(out=e1[:], in_=stbc[:], func=AF.Exp,
                                     bias=sscol[:, sb], scale=1.0)
                e2 = self.pipe.tile([128, n], F32, tag="e2", bufs=1)
                nc.scalar.activation(out=e2[:], in_=stbc[:], func=AF.Exp,
                                     bias=ss2col[:, sb], scale=0.2)
                nc.vector.tensor_tensor(out=e1[:], in0=e1[:], in1=e2[:], op=ALU.max)
                nc.vector.tensor_tensor(out=e1[:], in0=e1[:], in1=mask_bf[:, sb],
                                        op=ALU.mult)
                for tb in range(nb):
                    nc.tensor.matmul(pxn[tb][:], _r(e1[:, tb * 128:(tb + 1) * 128]),
                                     _r(xaug[:, sb]),
                                     start=(sb == 0), stop=(sb == nb - 1))
            for tb in range(nb):
                inv = self.sm.tile([128, 1], F32, tag="invxn")
                nc.vector.tensor_scalar_add(out=inv[:], in0=pxn[tb][:, hd + 3: hd + 4],
                                            scalar1=EPS_DEN)
                nc.vector.reciprocal(out=inv[:], in_=inv[:])
                nc.vector.tensor_scalar_mul(out=xnew[:, tb], in0=pxn[tb][:, :hd],
                                            scalar1=inv[:])
                nc.vector.tensor_scalar_mul(out=dots[:, tb],
                                            in0=pxn[tb][:, hd: hd + 3], scalar1=inv[:])
                nc.vector.tensor_scalar_add(out=acol[:, tb], in0=dots[:, tb, 0:1],
                                            scalar1=float(leb1))
        if sub == "xnew":
            return acol, acol, acol
        fit = self.sm.tile([128, nb, 1], F32, tag="fit")
        with tc.tile_pool(name="plG" + sfx, bufs=1, space="PSUM") as pG:
            pag = [pG.tile([128, 1], F32, tag=f"ag{tb}", name=f"pag{tb}") for tb in range(nb)]
            for sb in range(nb):
                mf = self.pipe.tile([128, n], F32, tag="maskf", bufs=1)
                nc.vector.tensor_copy(out=mf[:], in_=mask_bf[:, sb])
                for tb in range(nb):
                    nc.tensor.matmul(pag[tb][:], mf[:, tb * 128:(tb + 1) * 128],
                                     acol[:, sb], start=(sb == 0), stop=(sb == nb - 1))
            for tb in range(nb):
                t2 = self.sm.tile([128, 1], F32, tag="ft2")
                nc.vector.tensor_tensor(out=t2[:], in0=degc[:, tb], in1=dots[:, tb, 1:2],
                                        op=ALU.mult)
                nc.vector.tensor_tensor(out=t2[:], in0=pag[tb][:], in1=t2[:],
                                        op=ALU.subtract)
                nc.vector.tensor_tensor(out=t2[:], in0=t2[:], in1=dots[:, tb, 2:3],
                                        op=ALU.add)
                nc.vector.tensor_scalar_max(out=t2[:], in0=t2[:],
                                            scalar1=float(-85.0 - leb3))
                nc.scalar.activation(out=t2[:], in_=t2[:], func=AF.Exp, scale=-1.0,
                                     bias=negleb3c[:])
                nc.vector.tensor_scalar_add(out=t2[:], in0=t2[:], scalar1=1.0)
                nc.vector.reciprocal(out=fit[:, tb], in_=t2[:])
        if sub == "fit":
            return fit, fit, fit
        with tc.tile_pool(name="plR" + sfx, bufs=1, space="PSUM") as pR, \
             tc.tile_pool(name="plRs" + sfx, bufs=2, space="PSUM") as pRs:
            fitrow = self.cols2row(pRs, fit, [128] * nb, "fitrow")
            fitbc = self.bcast(pR, fitrow[:], n, "stfbc")
            rank = self.sm.tile([128, nb, 1], F32, tag="rank")
            for tb in range(nb):
                gts = self.pipe.tile([128, n], F32, tag="e1", bufs=1)
                gtc = self.sm.tile([128, 1], F32, tag="gtc")
                nc.vector.tensor_scalar(out=gts[:], in0=fitbc[:], scalar1=fit[:, tb],
                                        scalar2=None, op0=ALU.is_gt, op1=ALU.add,
                                        accum_out=gtc[:])
                eq = self.pipe.tile([128, n], F32, tag="e2", bufs=1)
                nc.vector.tensor_scalar(out=eq[:], in0=fitbc[:], scalar1=fit[:, tb],
                                        scalar2=None, op0=ALU.is_equal)
                lt = self.pipe.tile([128, n], F32, tag="maskf", bufs=1)
                nc.vector.tensor_scalar(out=lt[:], in0=self.iotabc[:, :n],
                                        scalar1=self.iotac[:, tb],
                                        scalar2=None, op0=ALU.is_lt)
                scr2 = self.pipe.tile([128, n], F32, tag="e1", bufs=1)
                eqlt = self.sm.tile([128, 1], F32, tag="eqlt")
                nc.vector.tensor_tensor(out=scr2[:], in0=eq[:], in1=lt[:], op=ALU.mult)
                nc.vector.reduce_sum(out=eqlt[:], in_=scr2[:], axis=AX.X)
                nc.vector.tensor_tensor(out=rank[:, tb], in0=gtc[:], in1=eqlt[:],
                                        op=ALU.add)
            if final:
                wsel = self.sm.tile([128, nb, 1], F32, tag="wsel")
                for tb in range(nb):
                    nc.vector.tensor_scalar(out=wsel[:, tb], in0=rank[:, tb],
                                            scalar1=float(k), scalar2=fit[:, tb],
                                            op0=ALU.is_lt, op1=ALU.mult)
                    nc.vector.tensor_scalar_mul(out=wsel[:, tb], in0=wsel[:, tb],
                                                scalar1=float(1.0 / k))
                pgm = pRs.tile([1, hd], F32, tag="ps_gm")
                for tb in range(nb):
                    nc.tensor.matmul(pgm[:], _r(wsel[:, tb]), _r(xnew[:, tb]),
                                     start=(tb == 0), stop=(tb == nb - 1))
                gmrow = self.sm.tile([1, hd], F32, tag="gmrow")
                nc.scalar.copy(out=gmrow[:], in_=pgm[:])
                return gmrow
            Pt = self.work.tile([128, nb, k], F32, tag="xoT")
            Pb = self.work.tile([128, nb, k], BF16, tag="Pb")
            for tb in range(nb):
                nc.vector.tensor_scalar(out=Pt[:, tb], in0=self.iotabc[:, :k],
                                        scalar1=rank[:, tb], scalar2=fit[:, tb],
                                        op0=ALU.is_equal, op1=ALU.mult)
                nc.vector.tensor_scalar(out=Pb[:, tb], in0=self.iotabc[:, :k],
                                        scalar1=rank[:, tb], scalar2=None,
                                        op0=ALU.is_equal)
            return Pt, Pb, xnew

    # ------------------------------------------------------------------
    def run(self, out_d):
        nc, tc, scal = self.nc, self.tc, self.scal
        self.identity = self.const.tile([128, 128], F32, tag="identity")
        make_identity(nc, self.identity[:])
        self.onesr = self.const.tile([1, 128], F32, tag="onesr")
        nc.vector.memset(self.onesr[:], 1.0)
        self.ones65 = self.const.tile([DH + 1, 128], F32, tag="ones65")
        nc.vector.memset(self.ones65[:], 1.0)
        self.iotabc = self.load("iotabc", [128, N1])
        self.iotac = self.load("iotac", [128, NB1, 1], rearr="(b p) o -> p b o")

        xT0 = self.const.tile([F0, 1, N1], F32, tag="xT0")
        nc.sync.dma_start(out=xT0[:, 0], in_=self.din["xT0"])
        mask1c = self.work.tile([128, NB1, N1], BF16, tag="bigshare")
        nc.sync.dma_start(out=mask1c[:],
                          in_=self.din["mask1c"].rearrange("(b p) t -> p b t", p=128))
        mask1 = self.load("mask1", [128, NB1, N1], BF16, pool=self.big,
                          rearr="(b p) t -> p b t")
        deg1c = self.load("deg1c", [128, NB1, 1], rearr="(b p) o -> p b o")

        def wload(name, kb, cols, p=128):
            return self.load(name, [p, kb, cols], rearr="(b p) c -> p b c", p=p)

        wq1 = wload("wq1", 1, HD1); wk1 = wload("wk1", 1, HD1)
        wv1a = wload("wv1a", 1, H1 * (DH + 1)); wsk1 = wload("wsk1", 1, HD1)
        bq1c = wload("bq1c", 3, 1); bk1c = wload("bk1c", 3, 1)
        bvsk1c = wload("bvsk1c", H1, 1, p=DH)
        attx1 = wload("attx1", H1, 1, p=DH); lew1 = wload("lew1", H1, 3, p=DH)
        w1bc = self.load("w1bc", [128, HD1])
        negleb3_1 = self.const.tile([128, 1], F32, tag="ngl1")
        nc.vector.memset(negleb3_1[:], float(-scal["le_b3_1"]))
        negleb3_2 = self.const.tile([128, 1], F32, tag="ngl2")
        nc.vector.memset(negleb3_2[:], float(-scal["le_b3_2"]))

        import os
        stop_after = os.environ.get("STOP_AFTER", "")

        def bail(src_ap):
            outc = self.sm.tile([4, 1], F32, tag="outc")
            nc.vector.tensor_copy(out=outc[:], in_=src_ap)
            nc.sync.dma_start(out=out_d, in_=outc[:])

        # ---------------- stage 1 ----------------
        x1T = self.conv(xT0, KBS1, N1, NB1, H1, [128, 128, 64],
                        wq1, wk1, wv1a, wsk1, bq1c, bk1c, bvsk1c, mask1c, "1")
        if stop_after == "conv1":
            return bail(x1T[:4, 0, 0:1])
        x1aug = self.transpose_to_aug(x1T, N1, NB1, H1, HD1, "bigshare")
        if stop_after == "aug1":
            return bail(x1aug[:4, 0, 0:1])
        Pt, Pb, xnew1 = self.pool(N1, NB1, H1, HD1, x1T, x1aug, mask1, deg1c,
                                  attx1, w1bc, lew1, scal["cst1"], scal["le_b1_1"],
                                  scal["le_b3_1"], negleb3_1, "1", final=False)

        if stop_after == "pool1":
            return bail(Pt[:4, 0, 0:1])
        # ---------------- stage-2 glue ----------------
        x2 = self.work.tile([128, NB2, HD1], F32, tag="E")
        with tc.tile_pool(name="g2a", bufs=2, space="PSUM") as pX:
            for qb in range(NB2):
                px2 = pX.tile([128, HD1], F32, tag="X")
                for tb in range(NB1):
                    nc.tensor.matmul(px2[:], _r(Pt[:, tb, qb * 128:(qb + 1) * 128]),
                                     _r(xnew1[:, tb]),
                                     start=(tb == 0), stop=(tb == NB1 - 1))
                nc.scalar.copy(out=x2[:, qb], in_=px2[:])
        x2T = self.work.tile([128, 3, N2], F32, tag="x2T")
        with tc.tile_pool(name="g2t", bufs=2, space="PSUM") as pT:
            for qb in range(NB2):
                for m, ob in enumerate(KBS2):
                    pt = pT.tile([128, 128], F32, tag="T")
                    nc.tensor.transpose(pt[:ob, :],
                                        x2[:, qb, m * 128: m * 128 + ob],
                                        self.identity[:])
                    nc.scalar.copy(out=x2T[:ob, m, qb * 128:(qb + 1) * 128],
                                   in_=pt[:ob, :])
        mask1T = self.work.tile([128, NB1, N1], BF16, tag="bigshare")
        nc.sync.dma_start(out=mask1T[:],
                          in_=self.din["mask1T"].rearrange("(b p) t -> p b t", p=128))
        Sb = self.work.tile([128, NB1, N2], BF16, tag="kT")
        Tb = self.work.tile([128, NB1, N2], BF16, tag="vaug")
        mask2 = self.big.tile([128, NB2, N2], BF16, tag="mask2")
        mask2b = self.big.tile([128, NB2, N2], BF16, tag="mask2b")
        deg2c = self.sm.tile([128, NB2, 1], F32, tag="deg2c")
        with tc.tile_pool(name="g2s", bufs=2, space="PSUM") as pG:
            for dst, rhs in ((Sb, Pb), (Tb, Sb)):
                for sb in range(NB1):
                    pp = pG.tile([128, N2], F32, tag="G")
                    for tb in range(NB1):
                        nc.tensor.matmul(pp[:], mask1T[:, tb, sb * 128:(sb + 1) * 128],
                                         rhs[:, tb],
                                         start=(tb == 0), stop=(tb == NB1 - 1))
                    nc.scalar.copy(out=dst[:, sb], in_=pp[:])
            for pb in range(NB2):
                pc = pG.tile([128, N2], F32, tag="G")
                for sb in range(NB1):
                    nc.tensor.matmul(pc[:], Sb[:, sb, pb * 128:(pb + 1) * 128],
                                     Tb[:, sb], start=(sb == 0), stop=(sb == NB1 - 1))
                m2f = self.sm.tile([128, N2], F32, tag="m2f")
                nc.vector.tensor_scalar(out=m2f[:], in0=pc[:], scalar1=0.5,
                                        scalar2=None, op0=ALU.is_gt)
                ne = self.sm.tile([128, N2], F32, tag="m2ne")
                nc.vector.tensor_scalar(out=ne[:], in0=self.iotabc[:, :N2],
                                        scalar1=self.iotac[:, pb], scalar2=None,
                                        op0=ALU.not_equal)
                nc.vector.tensor_tensor(out=m2f[:], in0=m2f[:], in1=ne[:], op=ALU.mult)
                nc.vector.tensor_copy(out=mask2[:, pb], in_=m2f[:])
                nc.vector.tensor_scalar(out=ne[:], in0=self.iotabc[:, :N2],
                                        scalar1=self.iotac[:, pb], scalar2=None,
                                        op0=ALU.is_equal)
                nc.vector.tensor_tensor(out=m2f[:], in0=m2f[:], in1=ne[:], op=ALU.max)
                nc.vector.tensor_copy(out=mask2b[:, pb], in_=m2f[:])
            onecb = self.sm.tile([128, 1], BF16, tag="onecb")
            nc.vector.memset(onecb[:], 1.0)
            pdg = pG.tile([1, N2], F32, tag="Gd")
            for pb in range(NB2):
                nc.tensor.matmul(pdg[:], onecb[:], mask2b[:, pb],
                                 start=(pb == 0), stop=(pb == NB2 - 1))
            degrow = self.sm.tile([1, N2], F32, tag="degrow")
            nc.scalar.copy(out=degrow[:], in_=pdg[:])
            for qb in range(NB2):
                ptd = pG.tile([128, 1], F32, tag="Gt")
                nc.tensor.transpose(ptd[:, :], degrow[:, qb * 128:(qb + 1) * 128],
                                    self.identity[:1, :1])
                nc.scalar.copy(out=deg2c[:, qb], in_=ptd[:])

        if stop_after == "glue":
            return bail(deg2c[:4, 0])
        # ---------------- stage 2 ----------------
        wq2 = wload("wq2", 3, HD2); wk2 = wload("wk2", 3, HD2)
        wv2a = wload("wv2a", 3, H2 * (DH + 1)); wsk2 = wload("wsk2", 3, HD2)
        bq2c = wload("bq2c", 2, 1); bk2c = wload("bk2c", 2, 1)
        bvsk2c = wload("bvsk2c", H2, 1, p=DH)
        attx2 = wload("attx2", H2, 1, p=DH); lew2 = wload("lew2", H2, 3, p=DH)
        w2bc = self.load("w2bc", [128, HD2])

        x3T = self.conv(x2T, KBS2, N2, NB2, H2, [128, 64],
                        wq2, wk2, wv2a, wsk2, bq2c, bk2c, bvsk2c, mask2, "2")
        if stop_after == "conv2":
            return bail(x3T[:4, 0, 0:1])
        x3aug = self.transpose_to_aug(x3T, N2, NB2, H2, HD2, "bigshare")
        gmrow = self.pool(N2, NB2, H2, HD2, x3T, x3aug, mask2b, deg2c,
                          attx2, w2bc, lew2, scal["cst2"], scal["le_b1_2"],
                          scal["le_b3_2"], negleb3_2, "2", final=True)

        if stop_after == "pool2":
            outc = self.sm.tile([4, 1], F32, tag="outc")
            nc.vector.memset(outc[:], 0.0)
            nc.vector.tensor_copy(out=outc[0:1, :], in_=gmrow[0:1, 0:1])
            nc.sync.dma_start(out=out_d, in_=outc[:])
            return
        # ---------------- MLP ----------------
        mw1 = wload("mw1", 2, HD1); mw2 = wload("mw2", 3, HD1); mw3 = wload("mw3", 3, 4)
        mb1c = wload("mb1c", 3, 1); mb2c = wload("mb2c", 3, 1)
        mb3c = self.load("mb3c", [4, 1])
        obs3 = [128, 128, 64]
        with tc.tile_pool(name="mlpp", bufs=2, space="PSUM") as pM:
            merge = self.sm.tile([128, 2, 1], F32, tag="merge")
            pm0 = pM.tile([128, 1], F32, tag="Mt")
            nc.tensor.transpose(pm0[:, :], gmrow[:, :128], self.identity[:1, :1])
            nc.scalar.copy(out=merge[:, 0], in_=pm0[:])
            pm1 = pM.tile([128, 1], F32, tag="Mt")
            nc.tensor.transpose(pm1[:64, :], gmrow[:, 128:192], self.identity[:1, :1])
            nc.scalar.copy(out=merge[:64, 1], in_=pm1[:64, :])
            nc.sync.dma_start(out=merge[64:69, 1], in_=self.din["evcd"])
            kbs1 = [128, 69]
            h1 = self.sm.tile([128, 3, 1], F32, tag="h1col")
            for m in range(3):
                ph = pM.tile([128, 1], F32, tag="Mm")
                for kb in range(2):
                    nc.tensor.matmul(ph[: obs3[m], :],
                                     _r(mw1[: kbs1[kb], kb, m * 128: m * 128 + obs3[m]]),
                                     _r(merge[: kbs1[kb], kb]),
                                     start=(kb == 0), stop=(kb == 1))
                nc.scalar.activation(out=h1[: obs3[m], m], in_=ph[: obs3[m], :],
                                     func=AF.Relu, bias=mb1c[: obs3[m], m])
            h2 = self.sm.tile([128, 3, 1], F32, tag="h2col")
            for m in range(3):
                ph = pM.tile([128, 1], F32, tag="Mm")
                for kb in range(3):
                    nc.tensor.matmul(ph[: obs3[m], :],
                                     _r(mw2[: obs3[kb], kb, m * 128: m * 128 + obs3[m]]),
                                     _r(h1[: obs3[kb], kb]),
                                     start=(kb == 0), stop=(kb == 2))
                nc.scalar.activation(out=h2[: obs3[m], m], in_=ph[: obs3[m], :],
                                     func=AF.Relu, bias=mb2c[: obs3[m], m])
            po = pM.tile([128, 1], F32, tag="Mo")
            for kb in range(3):
                nc.tensor.matmul(po[:4, 0:1], _r(mw3[: obs3[kb], kb, :]),
                                 _r(h2[: obs3[kb], kb]),
                                 start=(kb == 0), stop=(kb == 2))
            outc = self.sm.tile([4, 1], F32, tag="outc")
            nc.vector.tensor_tensor(out=outc[:], in0=po[:4, 0:1], in1=mb3c[:],
                                    op=ALU.add)
            nc.sync.dma_start(out=out_d, in_=outc[:])


# ======================================================================
# host side
# ======================================================================

_CACHE = {}


def _pad_rows(a, rows):
    out = np.zeros((rows, a.shape[1]), np.float32)
    out[: a.shape[0]] = a
    return out


def _prep_shared(inputs):
    tc1, tc2 = inputs["tc1"], inputs["tc2"]
    p1, p2 = inputs["pool1"], inputs["pool2"]
    mlp = inputs["mlp"]
    f = lambda a: np.asarray(a, np.float32)

    def vaug_pack(Wv, heads):
        fin = Wv.shape[0]
        out = np.zeros((fin, heads * (DH + 1)), np.float32)
        for h in range(heads):
            out[:, h * (DH + 1): h * (DH + 1) + DH] = Wv[:, h * DH:(h + 1) * DH]
        return out

    d = {}
    d["wq1"] = _pad_rows(f(tc1["Wq"]), 128)
    d["wk1"] = _pad_rows(f(tc1["Wk"]), 128)
    d["wv1a"] = _pad_rows(vaug_pack(f(tc1["Wv"]), H1), 128)
    d["wsk1"] = _pad_rows(f(tc1["Wskip"]), 128)
    d["bq1c"] = _pad_rows(f(tc1["bq"])[:, None], 3 * 128)
    d["bk1c"] = _pad_rows(f(tc1["bk"])[:, None], 3 * 128)
    d["bvsk1c"] = (f(tc1["bv"]) + f(tc1["bskip"]))[:, None].copy()
    d["attx1"] = f(p1["att_x"])[:, None].copy()
    w1 = f(p1["Wlin"]) @ f(p1["att_q"])
    d["w1bc"] = np.tile(w1[None, :], (128, 1)).astype(np.float32)
    d["lew1"] = np.stack([f(p1["le_W1"])[:, 0], f(p1["le_W2"])[:, 0],
                          f(p1["le_W3"])[:, 0]], axis=1).astype(np.float32)
    d["wq2"] = _pad_rows(f(tc2["Wq"]), 3 * 128)
    d["wk2"] = _pad_rows(f(tc2["Wk"]), 3 * 128)
    d["wv2a"] = _pad_rows(vaug_pack(f(tc2["Wv"]), H2), 3 * 128)
    d["wsk2"] = _pad_rows(f(tc2["Wskip"]), 3 * 128)
    d["bq2c"] = _pad_rows(f(tc2["bq"])[:, None], 2 * 128)
    d["bk2c"] = _pad_rows(f(tc2["bk"])[:, None], 2 * 128)
    d["bvsk2c"] = (f(tc2["bv"]) + f(tc2["bskip"]))[:, None].copy()
    d["attx2"] = f(p2["att_x"])[:, None].copy()
    w2 = f(p2["Wlin"]) @ f(p2["att_q"])
    d["w2bc"] = np.tile(w2[None, :], (128, 1)).astype(np.float32)
    d["lew2"] = np.stack([f(p2["le_W1"])[:, 0], f(p2["le_W2"])[:, 0],
                          f(p2["le_W3"])[:, 0]], axis=1).astype(np.float32)
    d["mw1"] = _pad_rows(f(mlp["W1"]), 2 * 128)
    d["mw2"] = _pad_rows(f(mlp["W2"]), 3 * 128)
    d["mw3"] = _pad_rows(f(mlp["W3"]), 3 * 128)
    d["mb1c"] = _pad_rows(f(mlp["b1"])[:, None], 3 * 128)
    d["mb2c"] = _pad_rows(f(mlp["b2"])[:, None], 3 * 128)
    d["mb3c"] = f(mlp["b3"])[:, None].copy()
    d["iotabc"] = np.tile(np.arange(N1, dtype=np.float32)[None, :], (128, 1))
    d["iotac"] = np.arange(N1, dtype=np.float32)[:, None]
    scal = {
        "cst1": float(f(p1["blin"]) @ f(p1["att_q"]) + f(p1["att_b"])),
        "cst2": float(f(p2["blin"]) @ f(p2["att_q"]) + f(p2["att_b"])),
        "le_b1_1": float(f(p1["le_b1"])[0]), "le_b3_1": float(f(p1["le_b3"])[0]),
        "le_b1_2": float(f(p2["le_b1"])[0]), "le_b3_2": float(f(p2["le_b3"])[0]),
    }
    return scal, d


def make_in_maps(inputs):
    import ml_dtypes
    nodes = np.asarray(inputs["nodes"], np.float32)
    ei = np.asarray(inputs["edge_index"])
    ev = np.asarray(inputs["exp_value"], np.float32)
    cd = np.asarray(inputs["circuit_depth"], np.float32)
    scal, shared = _prep_shared(inputs)
    src, dst = ei[0], ei[1]
    gid = src // N1
    in_maps = []
    for b in range(B):
        m = gid == b
        A = np.zeros((N1, N1), bool)
        A[src[m] % N1, dst[m] % N1] = True
        mask1 = A.copy()
        np.fill_diagonal(mask1, True)
        xg = nodes[b * N1:(b + 1) * N1]
        im = dict(shared)
        im["xT0"] = np.ascontiguousarray(xg.T)
        im["mask1c"] = A.astype(ml_dtypes.bfloat16)
        im["mask1"] = mask1.astype(ml_dtypes.bfloat16)
        im["mask1T"] = np.ascontiguousarray(mask1.T).astype(ml_dtypes.bfloat16)
        im["deg1c"] = mask1.sum(axis=0, dtype=np.float32)[:, None]
        im["evcd"] = np.concatenate([ev[b, 0], cd[b]])[:, None].astype(np.float32)
        in_maps.append(im)
    return scal, in_maps


def kernel(**inputs):
    scal, in_maps = make_in_maps(inputs)
    key = tuple(sorted(scal.items()))
    if key not in _CACHE:
        _CACHE[key] = build_program(scal)
    nc = _CACHE[key]
    res = run_bass_kernel_spmd(nc, in_maps, list(range(B)))
    out = np.stack([res.results[i]["out"][:, 0] for i in range(B)])
    return out.astype(np.float32)


# revision 17
# speedup vs baseline: 1.1771x; 1.1771x over previous
"""Trainium2 Bass kernel for nn_ExpValCircuitGraphModel (GNN message passing).

Sharding: data-parallel — one graph per NeuronCore (B=8 graphs on 8 cores).
Host does graph-format conversion only (dense 0/1 masks from edge_index,
parameter repacking); all model compute runs on-device.

Device algorithm (validated against the jax reference on host, rel err 6e-5):
- TransformerConv: scores^T on PE; softmax without max-shift; q/k/v/e bf16;
  e consumed per source block by the attention matmul, which also accumulates
  the softmax denominator via an appended ones-column on v.
- ASAP masked-max via mask-matmul log-sum-exp (per-feature shift, p=20);
  output measured insensitive to masked-max error up to +-0.3.
- exp(leaky_relu(y)) == max(exp(y), exp(0.2 y)).
- fitness sigmoid as 1/(1+exp(-x)) for exact fp32 saturation; top-k via stable
  rank (ties broken by index like jax.lax.top_k); selection as one-hot P.
- A2 = S_sel^T A S_sel needed only as boolean -> bf16 0/1 count matmuls.
- global_mean_pool of the selected half as a fitness-weighted matmul.

Conv outputs live in a [64, heads, n] transposed layout (partition rows 0:64)
so every attention/normalize op is partition-aligned.
"""
import numpy as np

import concourse.bass as bass
import concourse.tile as tile
from concourse import bacc, mybir
from concourse.bass_utils import run_bass_kernel_spmd
from concourse.masks import make_identity

F32, BF16, F32R = mybir.dt.float32, mybir.dt.bfloat16, mybir.dt.float32r
AF = mybir.ActivationFunctionType
ALU = mybir.AluOpType
AX = mybir.AxisListType

B, N1, F0 = 8, 1024, 32
H1, H2 = 5, 3
HD1, HD2 = 320, 192
DH = 64
N2 = 512
PEXP = 20.0
EPS_DEN = 1e-30
NB1, NB2 = 8, 4
KBS1, KBS2 = [F0], [128, 128, 64]     # conv input feature blocks


def _r(ap):
    # fp32r needs producer-side rounding (walrus invariant); plain fp32 for now
    return ap


def build_program(scal):
    nc = bacc.Bacc("TRN2", target_bir_lowering=False, debug=False, num_devices=8)
    din = {}

    def inp(name, shape, dtype=F32):
        din[name] = nc.dram_tensor(name, shape, dtype, kind="ExternalInput").ap()

    inp("xT0", [F0, N1], BF16)
    inp("mask1c", [N1, N1], BF16)
    inp("mask1", [N1, N1], BF16)
    inp("mask1T", [N1, N1], BF16)
    inp("deg1c", [N1, 1]); inp("evcd", [5, 1])
    inp("wq1", [128, HD1], BF16); inp("wk1", [128, HD1], BF16)
    inp("wv1a", [128, H1 * (DH + 1)], BF16); inp("wsk1", [128, HD1], BF16)
    inp("bq1c", [3 * 128, 1]); inp("bk1c", [3 * 128, 1])
    inp("bvsk1c", [H1 * DH, 1])
    inp("attx1", [H1 * DH, 1]); inp("w1bc", [128, HD1]); inp("lew1", [H1 * DH, 3])
    inp("wq2", [3 * 128, HD2], BF16); inp("wk2", [3 * 128, HD2], BF16)
    inp("wv2a", [3 * 128, H2 * (DH + 1)], BF16); inp("wsk2", [3 * 128, HD2], BF16)
    inp("bq2c", [2 * 128, 1]); inp("bk2c", [2 * 128, 1])
    inp("bvsk2c", [H2 * DH, 1])
    inp("attx2", [H2 * DH, 1]); inp("w2bc", [128, HD2]); inp("lew2", [H2 * DH, 3])
    inp("mw1", [2 * 128, HD1]); inp("mw2", [3 * 128, HD1]); inp("mw3", [3 * 128, 4])
    inp("mb1c", [3 * 128, 1]); inp("mb2c", [3 * 128, 1]); inp("mb3c", [4, 1])
    inp("iotabc", [128, N1]); inp("iotac", [N1, 1])
    out_d = nc.dram_tensor("out", [4, 1], F32, kind="ExternalOutput").ap()

    with tile.TileContext(nc) as tc:
        from contextlib import ExitStack
        with ExitStack() as ctx:
            _Prog(ctx, tc, nc, din, scal).run(out_d)
    nc.compile()
    return nc


class _Prog:
    def __init__(self, ctx, tc, nc, din, scal):
        self.ctx, self.tc, self.nc, self.din, self.scal = ctx, tc, nc, din, scal
        self.const = ctx.enter_context(tc.tile_pool(name="const", bufs=1))
        self.big = ctx.enter_context(tc.tile_pool(name="big", bufs=1))
        self.work = ctx.enter_context(tc.tile_pool(name="work", bufs=1))
        self.sm = ctx.enter_context(tc.tile_pool(name="sm", bufs=1))
        self.pipe = ctx.enter_context(tc.tile_pool(name="pipe", bufs=2))

    def load(self, name, shape, dtype=F32, pool=None, rearr=None, tag=None, p=128):
        pool = pool or self.const
        t = pool.tile(shape, dtype, tag=tag or name)
        src = self.din[name]
        if rearr is not None:
            src = src.rearrange(rearr, p=p)
        self.nc.sync.dma_start(out=t[:], in_=src)
        return t

    def bcast(self, pool_ps, row_ap, width, tag):
        nc = self.nc
        pb = pool_ps.tile([128, width], F32, tag="ps_bc")
        for t0 in range(0, width, 512):
            t1 = min(width, t0 + 512)
            nc.tensor.matmul(pb[:, t0:t1], self.onesr[:], row_ap[:, t0:t1],
                             start=True, stop=True)
        sb = self.sm.tile([128, width], F32, tag=tag)
        nc.scalar.copy(out=sb[:], in_=pb[:])
        return sb

    def cols2row(self, pool_ps, col3, blksizes, tag):
        nc = self.nc
        width = sum(blksizes)
        row = self.sm.tile([1, width], F32, tag=tag)
        o = 0
        for b, w in enumerate(blksizes):
            pt = pool_ps.tile([1, 128], F32, tag="ps_c2r")
            nc.tensor.transpose(pt[:, :w], col3[:w, b], self.identity[:w, :w])
            nc.scalar.copy(out=row[:, o: o + w], in_=pt[:, :w])
            o += w
        return row

    # ------------------------------------------------------------------
    def conv(self, xT, kbs, n, nb, heads, qkobs, wq, wk, wva, wsk,
             bqc, bkc, bvskc, mask_bf, sfx):
        """xT [<=128, KB, n] fp32 input (transposed). Returns xoT [64, heads, n]
        fp32 in work tag 'xoT': per-head feature rows at partitions 0:64."""
        nc, tc = self.nc, self.tc
        KB = len(kbs)
        nsl = [slice(t0, min(n, t0 + 512)) for t0 in range(0, n, 512)]
        qT = self.work.tile([128, len(qkobs), n], BF16, tag="qT")
        kT = self.work.tile([128, len(qkobs), n], BF16, tag="kT")
        xoT = self.work.tile([DH, heads, n], F32, tag="xoT")
        vaug = self.work.tile([128, nb, heads * (DH + 1)], BF16, tag="vaug")

        with tc.tile_pool(name="cvA" + sfx, bufs=2, space="PSUM") as pA, \
             tc.tile_pool(name="cvB" + sfx, bufs=1, space="PSUM") as pB, \
             tc.tile_pool(name="cvC" + sfx, bufs=1, space="PSUM") as pC:
            # q/k projections -> bf16 [128, OB, n]
            for w, dst, bias in ((wq, qT, bqc), (wk, kT, bkc)):
                for m, ob in enumerate(qkobs):
                    pm = pA.tile([128, n], F32, tag="A")
                    for sl in nsl:
                        for kb in range(KB):
                            nc.tensor.matmul(pm[:ob, sl],
                                             _r(w[: kbs[kb], kb, m * 128: m * 128 + ob]),
                                             _r(xT[: kbs[kb], kb, sl]),
                                             start=(kb == 0), stop=(kb == KB - 1))
                    nc.vector.tensor_scalar_add(out=dst[:ob, m], in0=pm[:ob],
                                                scalar1=bias[:ob, m])
            # skip projection -> xoT per head block [64, h, n]
            for h in range(heads):
                pm = pA.tile([128, n], F32, tag="A")
                for sl in nsl:
                    for kb in range(KB):
                        nc.tensor.matmul(pm[:DH, sl],
                                         _r(wsk[: kbs[kb], kb, h * DH:(h + 1) * DH]),
                                         _r(xT[: kbs[kb], kb, sl]),
                                         start=(kb == 0), stop=(kb == KB - 1))
                nc.scalar.copy(out=xoT[:, h], in_=pm[:DH])
            # v augmented
            for sb in range(nb):
                pv = pA.tile([128, n], F32, tag="A")
                w_ = heads * (DH + 1)
                for kb in range(KB):
                    nc.tensor.matmul(pv[:, :w_],
                                     _r(xT[: kbs[kb], kb, sb * 128:(sb + 1) * 128]),
                                     _r(wva[: kbs[kb], kb]),
                                     start=(kb == 0), stop=(kb == KB - 1))
                nc.scalar.copy(out=vaug[:, sb], in_=pv[:, :w_])
            v4 = vaug[:].rearrange("p b (h x) -> p b h x", h=heads)
            nc.gpsimd.memset(v4[:, :, :, DH: DH + 1], 1.0)

            isq = float(1.0 / np.sqrt(DH))
            for h in range(heads):
                mt, mo = divmod(h * DH, 128)
                pa = pB.tile([DH + 1, n], F32, tag="B")
                for sb in range(nb):
                    psc = pA.tile([128, n], F32, tag="A")
                    for sl in nsl:
                        nc.tensor.matmul(psc[:, sl],
                                         kT[mo: mo + DH, mt, sb * 128:(sb + 1) * 128],
                                         qT[mo: mo + DH, mt, sl],
                                         start=True, stop=True)
                    eb = self.pipe.tile([128, n], BF16, tag="eblk")
                    nc.scalar.activation(out=eb[:], in_=psc[:], func=AF.Exp, scale=isq)
                    nc.vector.tensor_tensor(out=eb[:], in0=eb[:], in1=mask_bf[:, sb],
                                            op=ALU.mult)
                    for sl in nsl:
                        nc.tensor.matmul(pa[:, sl], v4[:, sb, h, :], eb[:, sl],
                                         start=(sb == 0), stop=(sb == nb - 1))
                # den lives at partition DH(=64): recip there, broadcast via PE
                inv65 = self.sm.tile([DH + 1, n], F32, tag="inv65")
                nc.vector.tensor_scalar_add(out=inv65[DH: DH + 1], in0=pa[DH: DH + 1],
                                            scalar1=EPS_DEN)
                nc.vector.reciprocal(out=inv65[DH: DH + 1], in_=inv65[DH: DH + 1])
                pbc = pC.tile([DH, n], F32, tag="C")
                for sl in nsl:
                    nc.tensor.matmul(pbc[:, sl], self.ones65[DH: DH + 1, :DH],
                                     inv65[DH: DH + 1, sl], start=True, stop=True)
                invbc = self.sm.tile([DH, n], F32, tag="invbc")
                nc.scalar.copy(out=invbc[:], in_=pbc[:])
                att = self.sm.tile([DH, n], F32, tag="attnrm")
                nc.vector.tensor_tensor(out=att[:], in0=pa[:DH], in1=invbc[:],
                                        op=ALU.mult)
                nc.vector.tensor_tensor(out=xoT[:, h], in0=xoT[:, h], in1=att[:],
                                        op=ALU.add)
            for h in range(heads):
                nc.vector.tensor_scalar_add(out=xoT[:, h], in0=xoT[:, h],
                                            scalar1=bvskc[:, h])
        return xoT

    def transpose_to_aug(self, xT, n, nb, heads, hd, tag):
        nc, tc = self.nc, self.tc
        xaug = self.work.tile([128, nb, hd + 4], BF16, tag=tag)
        with tc.tile_pool(name="trp" + tag + str(n), bufs=2, space="PSUM") as pT:
            for sb in range(nb):
                for h in range(heads):
                    pt = pT.tile([128, DH], F32, tag="T")
                    nc.tensor.transpose(pt[:, :],
                                        xT[:, h, sb * 128:(sb + 1) * 128],
                                        self.identity[:DH, :DH])
                    nc.scalar.copy(out=xaug[:, sb, h * DH:(h + 1) * DH], in_=pt[:])
        return xaug

    # ------------------------------------------------------------------
    def pool(self, n, nb, heads, hd, xT, xaug, mask_bf, degc, attxc, wbc, lewc,
             cst, leb1, leb3, negleb3c, sfx, final):
        """xT: [64, heads, n] conv output; xaug [128, nb, hd+4] normal layout."""
        import os
        nc, tc = self.nc, self.tc
        sub = os.environ.get("POOL_STOP", "") if sfx == "1" else ""
        k = n // 2
        with tc.tile_pool(name="plA" + sfx, bufs=2, space="PSUM") as pA, \
             tc.tile_pool(name="plB" + sfx, bufs=1, space="PSUM") as pB, \
             tc.tile_pool(name="plS" + sfx, bufs=2, space="PSUM") as pS:
            g = self.sm.tile([DH, heads, 1], F32, tag="gcol")
            for h in range(heads):
                nc.vector.reduce_max(out=g[:, h], in_=xT[:, h], axis=AX.X)
            if sub == "g0":
                return g, g, g
            grow = self.cols2row(pS, g, [DH] * heads, "grow")
            gbc = self.bcast(pB, grow[:], hd, "gbc")
            pgbc = self.sm.tile([128, hd], F32, tag="pgbc")
            nc.vector.tensor_scalar_mul(out=pgbc[:], in0=gbc[:], scalar1=PEXP)
            if sub == "g":
                return g, g, g
            E = self.work.tile([128, nb, hd], BF16, tag="E")
            for sb in range(nb):
                y = self.pipe.tile([128, hd], F32, tag="yE")
                nc.vector.tensor_tensor(out=y[:], in0=xaug[:, sb, :hd], in1=gbc[:],
                                        op=ALU.subtract)
                nc.scalar.activation(out=E[:, sb], in_=y[:], func=AF.Exp, scale=PEXP)
            if sub == "E":
                return g, g, g
            stcol = self.sm.tile([128, nb, 1], F32, tag="stcol")
            for tb in range(nb):
                pL = pA.tile([128, hd], F32, tag="A")
                for sb in range(nb):
                    nc.tensor.matmul(pL[:], mask_bf[:, sb, tb * 128:(tb + 1) * 128],
                                     E[:, sb], start=(sb == 0), stop=(sb == nb - 1))
                L = self.pipe.tile([128, hd], F32, tag="Llse")
                nc.scalar.activation(out=L[:], in_=pL[:], func=AF.Ln)
                nc.vector.tensor_tensor(out=L[:], in0=L[:], in1=pgbc[:], op=ALU.add)
                scr = self.pipe.tile([128, hd], F32, tag="scrL")
                nc.vector.tensor_tensor(out=scr[:], in0=L[:], in1=wbc[:, :hd],
                                        op=ALU.mult)
                nc.vector.tensor_scalar_mul(out=scr[:], in0=scr[:],
                                            scalar1=float(1.0 / PEXP))
                nc.vector.reduce_sum(out=stcol[:, tb], in_=scr[:], axis=AX.X)
            strow = self.cols2row(pS, stcol, [128] * nb, "strow")
            stbc = self.bcast(pB, strow[:], n, "stfbc")
            sscol = self.sm.tile([128, nb, 1], F32, tag="sscol")
            ss2col = self.sm.tile([128, nb, 1], F32, tag="ss2col")
            for sb in range(nb):
                pss = pS.tile([128, 4], F32, tag="ps_s4")
                for h in range(heads):
                    nc.tensor.matmul(pss[:, 0:1],
                                     _r(xT[:, h, sb * 128:(sb + 1) * 128]),
                                     _r(attxc[:, h]),
                                     start=(h == 0), stop=(h == heads - 1))
                nc.vector.tensor_scalar_add(out=sscol[:, sb], in0=pss[:, 0:1],
                                            scalar1=float(cst))
                nc.vector.tensor_scalar_mul(out=ss2col[:, sb], in0=sscol[:, sb],
                                            scalar1=0.2)
            for sb in range(nb):
                pxw = pS.tile([128, 4], F32, tag="ps_s4")
                for h in range(heads):
                    nc.tensor.matmul(pxw[:, 0:3],
                                     _r(xT[:, h, sb * 128:(sb + 1) * 128]),
                                     _r(lewc[:, h]),
                                     start=(h == 0), stop=(h == heads - 1))
                nc.scalar.copy(out=xaug[:, sb, hd: hd + 3], in_=pxw[:, 0:3])
            nc.gpsimd.memset(xaug[:, :, hd + 3: hd + 4], 1.0)
        if sub == "lse":
            return stcol[:, 0:1], stcol[:, 0:1], stcol[:, 0:1]
        xnew = self.work.tile([128, nb, hd], BF16, tag="xnew")
        dots = self.sm.tile([128, nb, 3], F32, tag="dots")
        acol = self.sm.tile([128, nb, 1], F32, tag="acol")
        with tc.tile_pool(name="plN" + sfx, bufs=1, space="PSUM") as pN:
            pxn = [pN.tile([128, hd + 4], F32, tag=f"xn{tb}", name=f"pxn{tb}") for tb in range(nb)]
            for sb in range(nb):
                e1 = self.pipe.tile([128, n], F32, tag="e1", bufs=1)
                nc.scalar.activation(out=e1[:], in_=stbc[:], func=AF.Exp,
                                     bias=sscol[:, sb], scale=1.0)
                e2 = self.pipe.tile([128, n], F32, tag="e2", bufs=1)
                nc.scalar.activation(out=e2[:], in_=stbc[:], func=AF.Exp,
                                     bias=ss2col[:, sb], scale=0.2)
                nc.vector.tensor_tensor(out=e1[:], in0=e1[:], in1=e2[:], op=ALU.max)
                eSb = self.pipe.tile([128, n], BF16, tag="eSb")
                nc.vector.tensor_tensor(out=eSb[:], in0=e1[:], in1=mask_bf[:, sb],
                                        op=ALU.mult)
                for tb in range(nb):
                    nc.tensor.matmul(pxn[tb][:], eSb[:, tb * 128:(tb + 1) * 128],
                                     xaug[:, sb],
                                     start=(sb == 0), stop=(sb == nb - 1))
            for tb in range(nb):
                inv = self.sm.tile([128, 1], F32, tag="invxn")
                nc.vector.tensor_scalar_add(out=inv[:], in0=pxn[tb][:, hd + 3: hd + 4],
                                            scalar1=EPS_DEN)
                nc.vector.reciprocal(out=inv[:], in_=inv[:])
                nc.vector.tensor_scalar_mul(out=xnew[:, tb], in0=pxn[tb][:, :hd],
                                            scalar1=inv[:])
                nc.vector.tensor_scalar_mul(out=dots[:, tb],
                                            in0=pxn[tb][:, hd: hd + 3], scalar1=inv[:])
                nc.vector.tensor_scalar_add(out=acol[:, tb], in0=dots[:, tb, 0:1],
                                            scalar1=float(leb1))
        if sub == "xnew":
            return acol, acol, acol
        fit = self.sm.tile([128, nb, 1], F32, tag="fit")
        with tc.tile_pool(name="plG" + sfx, bufs=1, space="PSUM") as pG:
            pag = [pG.tile([128, 1], F32, tag=f"ag{tb}", name=f"pag{tb}") for tb in range(nb)]
            for sb in range(nb):
                mf = self.pipe.tile([128, n], F32, tag="maskf", bufs=1)
                nc.vector.tensor_copy(out=mf[:], in_=mask_bf[:, sb])
                for tb in range(nb):
                    nc.tensor.matmul(pag[tb][:], mf[:, tb * 128:(tb + 1) * 128],
                                     acol[:, sb], start=(sb == 0), stop=(sb == nb - 1))
            for tb in range(nb):
                t2 = self.sm.tile([128, 1], F32, tag="ft2")
                nc.vector.tensor_tensor(out=t2[:], in0=degc[:, tb], in1=dots[:, tb, 1:2],
                                        op=ALU.mult)
                nc.vector.tensor_tensor(out=t2[:], in0=pag[tb][:], in1=t2[:],
                                        op=ALU.subtract)
                nc.vector.tensor_tensor(out=t2[:], in0=t2[:], in1=dots[:, tb, 2:3],
                                        op=ALU.add)
                nc.vector.tensor_scalar_max(out=t2[:], in0=t2[:],
                                            scalar1=float(-85.0 - leb3))
                nc.scalar.activation(out=t2[:], in_=t2[:], func=AF.Exp, scale=-1.0,
                                     bias=negleb3c[:])
                nc.vector.tensor_scalar_add(out=t2[:], in0=t2[:], scalar1=1.0)
                nc.vector.reciprocal(out=fit[:, tb], in_=t2[:])
        if sub == "fit":
            return fit, fit, fit
        with tc.tile_pool(name="plR" + sfx, bufs=1, space="PSUM") as pR, \
             tc.tile_pool(name="plRs" + sfx, bufs=2, space="PSUM") as pRs:
            fitrow = self.cols2row(pRs, fit, [128] * nb, "fitrow")
            fitbc = self.bcast(pR, fitrow[:], n, "stfbc")
            rank = self.sm.tile([128, nb, 1], F32, tag="rank")
            for tb in range(nb):
                gts = self.pipe.tile([128, n], F32, tag="e1", bufs=1)
                gtc = self.sm.tile([128, 1], F32, tag="gtc")
                nc.vector.tensor_scalar(out=gts[:], in0=fitbc[:], scalar1=fit[:, tb],
                                        scalar2=None, op0=ALU.is_gt, op1=ALU.add,
                                        accum_out=gtc[:])
                eq = self.pipe.tile([128, n], F32, tag="e2", bufs=1)
                nc.vector.tensor_scalar(out=eq[:], in0=fitbc[:], scalar1=fit[:, tb],
                                        scalar2=None, op0=ALU.is_equal)
                lt = self.pipe.tile([128, n], F32, tag="maskf", bufs=1)
                nc.vector.tensor_scalar(out=lt[:], in0=self.iotabc[:, :n],
                                        scalar1=self.iotac[:, tb],
                                        scalar2=None, op0=ALU.is_lt)
                scr2 = self.pipe.tile([128, n], F32, tag="e1", bufs=1)
                eqlt = self.sm.tile([128, 1], F32, tag="eqlt")
                nc.vector.tensor_tensor(out=scr2[:], in0=eq[:], in1=lt[:], op=ALU.mult)
                nc.vector.reduce_sum(out=eqlt[:], in_=scr2[:], axis=AX.X)
                nc.vector.tensor_tensor(out=rank[:, tb], in0=gtc[:], in1=eqlt[:],
                                        op=ALU.add)
            if final:
                wsel = self.sm.tile([128, nb, 1], BF16, tag="wsel")
                for tb in range(nb):
                    nc.vector.tensor_scalar(out=wsel[:, tb], in0=rank[:, tb],
                                            scalar1=float(k), scalar2=fit[:, tb],
                                            op0=ALU.is_lt, op1=ALU.mult)
                pgm = pRs.tile([1, hd], F32, tag="ps_gm")
                for tb in range(nb):
                    nc.tensor.matmul(pgm[:], wsel[:, tb], xnew[:, tb],
                                     start=(tb == 0), stop=(tb == nb - 1))
                gmrow = self.sm.tile([1, hd], F32, tag="gmrow")
                nc.scalar.mul(out=gmrow[:], in_=pgm[:], mul=float(1.0 / k))
                return gmrow
            Pt = self.work.tile([128, nb, k], BF16, tag="Pb")
            for tb in range(nb):
                nc.vector.tensor_scalar(out=Pt[:, tb], in0=self.iotabc[:, :k],
                                        scalar1=rank[:, tb], scalar2=fit[:, tb],
                                        op0=ALU.is_equal, op1=ALU.mult)
            return Pt, Pt, xnew

    # ------------------------------------------------------------------
    def run(self, out_d):
        nc, tc, scal = self.nc, self.tc, self.scal
        self.identity = self.const.tile([128, 128], F32, tag="identity")
        make_identity(nc, self.identity[:])
        self.onesr = self.const.tile([1, 128], F32, tag="onesr")
        nc.vector.memset(self.onesr[:], 1.0)
        self.ones65 = self.const.tile([DH + 1, 128], F32, tag="ones65")
        nc.vector.memset(self.ones65[:], 1.0)
        self.iotabc = self.load("iotabc", [128, N1])
        self.iotac = self.load("iotac", [128, NB1, 1], rearr="(b p) o -> p b o")

        xT0 = self.const.tile([F0, 1, N1], BF16, tag="xT0")
        nc.sync.dma_start(out=xT0[:, 0], in_=self.din["xT0"])
        mask1c = self.work.tile([128, NB1, N1], BF16, tag="bigshare")
        nc.sync.dma_start(out=mask1c[:],
                          in_=self.din["mask1c"].rearrange("(b p) t -> p b t", p=128))
        mask1 = self.load("mask1", [128, NB1, N1], BF16, pool=self.big,
                          rearr="(b p) t -> p b t")
        deg1c = self.load("deg1c", [128, NB1, 1], rearr="(b p) o -> p b o")

        def wload(name, kb, cols, p=128, dtype=F32):
            return self.load(name, [p, kb, cols], dtype, rearr="(b p) c -> p b c", p=p)

        wq1 = wload("wq1", 1, HD1, dtype=BF16); wk1 = wload("wk1", 1, HD1, dtype=BF16)
        wv1a = wload("wv1a", 1, H1 * (DH + 1), dtype=BF16); wsk1 = wload("wsk1", 1, HD1, dtype=BF16)
        bq1c = wload("bq1c", 3, 1); bk1c = wload("bk1c", 3, 1)
        bvsk1c = wload("bvsk1c", H1, 1, p=DH)
        attx1 = wload("attx1", H1, 1, p=DH); lew1 = wload("lew1", H1, 3, p=DH)
        w1bc = self.load("w1bc", [128, HD1])
        negleb3_1 = self.const.tile([128, 1], F32, tag="ngl1")
        nc.vector.memset(negleb3_1[:], float(-scal["le_b3_1"]))
        negleb3_2 = self.const.tile([128, 1], F32, tag="ngl2")
        nc.vector.memset(negleb3_2[:], float(-scal["le_b3_2"]))

        import os
        stop_after = os.environ.get("STOP_AFTER", "")

        def bail(src_ap):
            outc = self.sm.tile([4, 1], F32, tag="outc")
            nc.vector.tensor_copy(out=outc[:], in_=src_ap)
            nc.sync.dma_start(out=out_d, in_=outc[:])

        # ---------------- stage 1 ----------------
        x1T = self.conv(xT0, KBS1, N1, NB1, H1, [128, 128, 64],
                        wq1, wk1, wv1a, wsk1, bq1c, bk1c, bvsk1c, mask1c, "1")
        if stop_after == "conv1":
            return bail(x1T[:4, 0, 0:1])
        x1aug = self.transpose_to_aug(x1T, N1, NB1, H1, HD1, "bigshare")
        if stop_after == "aug1":
            return bail(x1aug[:4, 0, 0:1])
        Pt, Pb, xnew1 = self.pool(N1, NB1, H1, HD1, x1T, x1aug, mask1, deg1c,
                                  attx1, w1bc, lew1, scal["cst1"], scal["le_b1_1"],
                                  scal["le_b3_1"], negleb3_1, "1", final=False)

        if stop_after == "pool1":
            return bail(Pt[:4, 0, 0:1])
        # ---------------- stage-2 glue ----------------
        x2 = self.work.tile([128, NB2, HD1], BF16, tag="E")
        with tc.tile_pool(name="g2a", bufs=2, space="PSUM") as pX:
            for qb in range(NB2):
                px2 = pX.tile([128, HD1], F32, tag="X")
                for tb in range(NB1):
                    nc.tensor.matmul(px2[:], Pt[:, tb, qb * 128:(qb + 1) * 128],
                                     xnew1[:, tb],
                                     start=(tb == 0), stop=(tb == NB1 - 1))
                nc.scalar.copy(out=x2[:, qb], in_=px2[:])
        x2T = self.work.tile([128, 3, N2], BF16, tag="x2T")
        with tc.tile_pool(name="g2t", bufs=2, space="PSUM") as pT:
            identb = self.sm.tile([128, 128], BF16, tag="identb")
            nc.vector.tensor_copy(out=identb[:], in_=self.identity[:])
            for qb in range(NB2):
                for m, ob in enumerate(KBS2):
                    pt = pT.tile([128, 128], BF16, tag="T")
                    nc.tensor.transpose(pt[:ob, :],
                                        x2[:, qb, m * 128: m * 128 + ob],
                                        identb[:])
                    nc.scalar.copy(out=x2T[:ob, m, qb * 128:(qb + 1) * 128],
                                   in_=pt[:ob, :])
        mask1T = self.work.tile([128, NB1, N1], BF16, tag="bigshare")
        nc.sync.dma_start(out=mask1T[:],
                          in_=self.din["mask1T"].rearrange("(b p) t -> p b t", p=128))
        Sb = self.work.tile([128, NB1, N2], BF16, tag="kT")
        Tb = self.work.tile([128, NB1, N2], BF16, tag="vaug")
        mask2 = self.big.tile([128, NB2, N2], BF16, tag="mask2")
        mask2b = self.big.tile([128, NB2, N2], BF16, tag="mask2b")
        deg2c = self.sm.tile([128, NB2, 1], F32, tag="deg2c")
        with tc.tile_pool(name="g2s", bufs=2, space="PSUM") as pG:
            for dst, rhs in ((Sb, Pb), (Tb, Sb)):
                for sb in range(NB1):
                    pp = pG.tile([128, N2], F32, tag="G")
                    for tb in range(NB1):
                        nc.tensor.matmul(pp[:], mask1T[:, tb, sb * 128:(sb + 1) * 128],
                                         rhs[:, tb],
                                         start=(tb == 0), stop=(tb == NB1 - 1))
                    nc.scalar.copy(out=dst[:, sb], in_=pp[:])
            for pb in range(NB2):
                pc = pG.tile([128, N2], F32, tag="G")
                for sb in range(NB1):
                    nc.tensor.matmul(pc[:], Sb[:, sb, pb * 128:(pb + 1) * 128],
                                     Tb[:, sb], start=(sb == 0), stop=(sb == NB1 - 1))
                m2f = self.sm.tile([128, N2], F32, tag="m2f")
                nc.vector.tensor_scalar(out=m2f[:], in0=pc[:], scalar1=0.5,
                                        scalar2=None, op0=ALU.is_gt)
                ne = self.sm.tile([128, N2], F32, tag="m2ne")
                nc.vector.tensor_scalar(out=ne[:], in0=self.iotabc[:, :N2],
                                        scalar1=self.iotac[:, pb], scalar2=None,
                                        op0=ALU.not_equal)
                nc.vector.tensor_tensor(out=m2f[:], in0=m2f[:], in1=ne[:], op=ALU.mult)
                nc.vector.tensor_copy(out=mask2[:, pb], in_=m2f[:])
                nc.vector.tensor_scalar(out=ne[:], in0=self.iotabc[:, :N2],
                                        scalar1=self.iotac[:, pb], scalar2=None,
                                        op0=ALU.is_equal)
                nc.vector.tensor_tensor(out=m2f[:], in0=m2f[:], in1=ne[:], op=ALU.max)
                nc.vector.tensor_copy(out=mask2b[:, pb], in_=m2f[:])
            onecb = self.sm.tile([128, 1], BF16, tag="onecb")
            nc.vector.memset(onecb[:], 1.0)
            pdg = pG.tile([1, N2], F32, tag="Gd")
            for pb in range(NB2):
                nc.tensor.matmul(pdg[:], onecb[:], mask2b[:, pb],
                                 start=(pb == 0), stop=(pb == NB2 - 1))
            degrow = self.sm.tile([1, N2], F32, tag="degrow")
            nc.scalar.copy(out=degrow[:], in_=pdg[:])
            for qb in range(NB2):
                ptd = pG.tile([128, 1], F32, tag="Gt")
                nc.tensor.transpose(ptd[:, :], degrow[:, qb * 128:(qb + 1) * 128],
                                    self.identity[:1, :1])
                nc.scalar.copy(out=deg2c[:, qb], in_=ptd[:])

        if stop_after == "glue":
            return bail(deg2c[:4, 0])
        # ---------------- stage 2 ----------------
        wq2 = wload("wq2", 3, HD2, dtype=BF16); wk2 = wload("wk2", 3, HD2, dtype=BF16)
        wv2a = wload("wv2a", 3, H2 * (DH + 1), dtype=BF16); wsk2 = wload("wsk2", 3, HD2, dtype=BF16)
        bq2c = wload("bq2c", 2, 1); bk2c = wload("bk2c", 2, 1)
        bvsk2c = wload("bvsk2c", H2, 1, p=DH)
        attx2 = wload("attx2", H2, 1, p=DH); lew2 = wload("lew2", H2, 3, p=DH)
        w2bc = self.load("w2bc", [128, HD2])

        x3T = self.conv(x2T, KBS2, N2, NB2, H2, [128, 64],
                        wq2, wk2, wv2a, wsk2, bq2c, bk2c, bvsk2c, mask2, "2")
        if stop_after == "conv2":
            return bail(x3T[:4, 0, 0:1])
        x3aug = self.transpose_to_aug(x3T, N2, NB2, H2, HD2, "bigshare")
        gmrow = self.pool(N2, NB2, H2, HD2, x3T, x3aug, mask2b, deg2c,
                          attx2, w2bc, lew2, scal["cst2"], scal["le_b1_2"],
                          scal["le_b3_2"], negleb3_2, "2", final=True)

        if stop_after == "pool2":
            outc = self.sm.tile([4, 1], F32, tag="outc")
            nc.vector.memset(outc[:], 0.0)
            nc.vector.tensor_copy(out=outc[0:1, :], in_=gmrow[0:1, 0:1])
            nc.sync.dma_start(out=out_d, in_=outc[:])
            return
        # ---------------- MLP ----------------
        mw1 = wload("mw1", 2, HD1); mw2 = wload("mw2", 3, HD1); mw3 = wload("mw3", 3, 4)
        mb1c = wload("mb1c", 3, 1); mb2c = wload("mb2c", 3, 1)
        mb3c = self.load("mb3c", [4, 1])
        obs3 = [128, 128, 64]
        with tc.tile_pool(name="mlpp", bufs=2, space="PSUM") as pM:
            merge = self.sm.tile([128, 2, 1], F32, tag="merge")
            pm0 = pM.tile([128, 1], F32, tag="Mt")
            nc.tensor.transpose(pm0[:, :], gmrow[:, :128], self.identity[:1, :1])
            nc.scalar.copy(out=merge[:, 0], in_=pm0[:])
            pm1 = pM.tile([128, 1], F32, tag="Mt")
            nc.tensor.transpose(pm1[:64, :], gmrow[:, 128:192], self.identity[:1, :1])
            nc.scalar.copy(out=merge[:64, 1], in_=pm1[:64, :])
            nc.sync.dma_start(out=merge[64:69, 1], in_=self.din["evcd"])
            kbs1 = [128, 69]
            h1 = self.sm.tile([128, 3, 1], F32, tag="h1col")
            for m in range(3):
                ph = pM.tile([128, 1], F32, tag="Mm")
                for kb in range(2):
                    nc.tensor.matmul(ph[: obs3[m], :],
                                     _r(mw1[: kbs1[kb], kb, m * 128: m * 128 + obs3[m]]),
                                     _r(merge[: kbs1[kb], kb]),
                                     start=(kb == 0), stop=(kb == 1))
                nc.scalar.activation(out=h1[: obs3[m], m], in_=ph[: obs3[m], :],
                                     func=AF.Relu, bias=mb1c[: obs3[m], m])
            h2 = self.sm.tile([128, 3, 1], F32, tag="h2col")
            for m in range(3):
                ph = pM.tile([128, 1], F32, tag="Mm")
                for kb in range(3):
                    nc.tensor.matmul(ph[: obs3[m], :],
                                     _r(mw2[: obs3[kb], kb, m * 128: m * 128 + obs3[m]]),
                                     _r(h1[: obs3[kb], kb]),
                                     start=(kb == 0), stop=(kb == 2))
                nc.scalar.activation(out=h2[: obs3[m], m], in_=ph[: obs3[m], :],
                                     func=AF.Relu, bias=mb2c[: obs3[m], m])
            po = pM.tile([128, 1], F32, tag="Mo")
            for kb in range(3):
                nc.tensor.matmul(po[:4, 0:1], _r(mw3[: obs3[kb], kb, :]),
                                 _r(h2[: obs3[kb], kb]),
                                 start=(kb == 0), stop=(kb == 2))
            outc = self.sm.tile([4, 1], F32, tag="outc")
            nc.vector.tensor_tensor(out=outc[:], in0=po[:4, 0:1], in1=mb3c[:],
                                    op=ALU.add)
            nc.sync.dma_start(out=out_d, in_=outc[:])


# ======================================================================
# host side
# ======================================================================

_CACHE = {}


def _pad_rows(a, rows):
    out = np.zeros((rows, a.shape[1]), np.float32)
    out[: a.shape[0]] = a
    return out


def _prep_shared(inputs):
    tc1, tc2 = inputs["tc1"], inputs["tc2"]
    p1, p2 = inputs["pool1"], inputs["pool2"]
    mlp = inputs["mlp"]
    f = lambda a: np.asarray(a, np.float32)

    def vaug_pack(Wv, heads):
        fin = Wv.shape[0]
        out = np.zeros((fin, heads * (DH + 1)), np.float32)
        for h in range(heads):
            out[:, h * (DH + 1): h * (DH + 1) + DH] = Wv[:, h * DH:(h + 1) * DH]
        return out

    d = {}
    d["wq1"] = _pad_rows(f(tc1["Wq"]), 128)
    d["wk1"] = _pad_rows(f(tc1["Wk"]), 128)
    d["wv1a"] = _pad_rows(vaug_pack(f(tc1["Wv"]), H1), 128)
    d["wsk1"] = _pad_rows(f(tc1["Wskip"]), 128)
    d["bq1c"] = _pad_rows(f(tc1["bq"])[:, None], 3 * 128)
    d["bk1c"] = _pad_rows(f(tc1["bk"])[:, None], 3 * 128)
    d["bvsk1c"] = (f(tc1["bv"]) + f(tc1["bskip"]))[:, None].copy()
    d["attx1"] = f(p1["att_x"])[:, None].copy()
    w1 = f(p1["Wlin"]) @ f(p1["att_q"])
    d["w1bc"] = np.tile(w1[None, :], (128, 1)).astype(np.float32)
    d["lew1"] = np.stack([f(p1["le_W1"])[:, 0], f(p1["le_W2"])[:, 0],
                          f(p1["le_W3"])[:, 0]], axis=1).astype(np.float32)
    d["wq2"] = _pad_rows(f(tc2["Wq"]), 3 * 128)
    d["wk2"] = _pad_rows(f(tc2["Wk"]), 3 * 128)
    d["wv2a"] = _pad_rows(vaug_pack(f(tc2["Wv"]), H2), 3 * 128)
    d["wsk2"] = _pad_rows(f(tc2["Wskip"]), 3 * 128)
    d["bq2c"] = _pad_rows(f(tc2["bq"])[:, None], 2 * 128)
    d["bk2c"] = _pad_rows(f(tc2["bk"])[:, None], 2 * 128)
    d["bvsk2c"] = (f(tc2["bv"]) + f(tc2["bskip"]))[:, None].copy()
    d["attx2"] = f(p2["att_x"])[:, None].copy()
    w2 = f(p2["Wlin"]) @ f(p2["att_q"])
    d["w2bc"] = np.tile(w2[None, :], (128, 1)).astype(np.float32)
    d["lew2"] = np.stack([f(p2["le_W1"])[:, 0], f(p2["le_W2"])[:, 0],
                          f(p2["le_W3"])[:, 0]], axis=1).astype(np.float32)
    d["mw1"] = _pad_rows(f(mlp["W1"]), 2 * 128)
    d["mw2"] = _pad_rows(f(mlp["W2"]), 3 * 128)
    d["mw3"] = _pad_rows(f(mlp["W3"]), 3 * 128)
    d["mb1c"] = _pad_rows(f(mlp["b1"])[:, None], 3 * 128)
    d["mb2c"] = _pad_rows(f(mlp["b2"])[:, None], 3 * 128)
    d["mb3c"] = f(mlp["b3"])[:, None].copy()
    d["iotabc"] = np.tile(np.arange(N1, dtype=np.float32)[None, :], (128, 1))
    d["iotac"] = np.arange(N1, dtype=np.float32)[:, None]
    scal = {
        "cst1": float(f(p1["blin"]) @ f(p1["att_q"]) + f(p1["att_b"])),
        "cst2": float(f(p2["blin"]) @ f(p2["att_q"]) + f(p2["att_b"])),
        "le_b1_1": float(f(p1["le_b1"])[0]), "le_b3_1": float(f(p1["le_b3"])[0]),
        "le_b1_2": float(f(p2["le_b1"])[0]), "le_b3_2": float(f(p2["le_b3"])[0]),
    }
    return scal, d


def make_in_maps(inputs):
    import ml_dtypes
    BFH = ml_dtypes.bfloat16
    nodes = np.asarray(inputs["nodes"], np.float32)
    ei = np.asarray(inputs["edge_index"])
    ev = np.asarray(inputs["exp_value"], np.float32)
    cd = np.asarray(inputs["circuit_depth"], np.float32)
    scal, shared = _prep_shared(inputs)
    for w in ["wq1", "wk1", "wv1a", "wsk1", "wq2", "wk2", "wv2a", "wsk2"]:
        shared[w] = shared[w].astype(BFH)
    src, dst = ei[0], ei[1]
    gid = src // N1
    in_maps = []
    for b in range(B):
        m = gid == b
        A = np.zeros((N1, N1), bool)
        A[src[m] % N1, dst[m] % N1] = True
        mask1 = A.copy()
        np.fill_diagonal(mask1, True)
        xg = nodes[b * N1:(b + 1) * N1]
        im = dict(shared)
        im["xT0"] = np.ascontiguousarray(xg.T).astype(ml_dtypes.bfloat16)
        im["mask1c"] = A.astype(ml_dtypes.bfloat16)
        im["mask1"] = mask1.astype(ml_dtypes.bfloat16)
        im["mask1T"] = np.ascontiguousarray(mask1.T).astype(ml_dtypes.bfloat16)
        im["deg1c"] = mask1.sum(axis=0, dtype=np.float32)[:, None]
        im["evcd"] = np.concatenate([ev[b, 0], cd[b]])[:, None].astype(np.float32)
        in_maps.append(im)
    return scal, in_maps


def kernel(**inputs):
    scal, in_maps = make_in_maps(inputs)
    key = tuple(sorted(scal.items()))
    if key not in _CACHE:
        _CACHE[key] = build_program(scal)
    nc = _CACHE[key]
    res = run_bass_kernel_spmd(nc, in_maps, list(range(B)))
    out = np.stack([res.results[i]["out"][:, 0] for i in range(B)])
    return out.astype(np.float32)


# revision 18
# speedup vs baseline: 1.2238x; 1.0397x over previous
"""Trainium2 Bass kernel for nn_ExpValCircuitGraphModel (GNN message passing).

Sharding: data-parallel — one graph per NeuronCore (B=8 graphs on 8 cores).
Host does graph-format conversion only (dense 0/1 masks from edge_index,
parameter repacking); all model compute runs on-device.

Device algorithm (validated against the jax reference on host, rel err 6e-5):
- TransformerConv: scores^T on PE; softmax without max-shift; q/k/v/e bf16;
  e consumed per source block by the attention matmul, which also accumulates
  the softmax denominator via an appended ones-column on v.
- ASAP masked-max via mask-matmul log-sum-exp (per-feature shift, p=20);
  output measured insensitive to masked-max error up to +-0.3.
- exp(leaky_relu(y)) == max(exp(y), exp(0.2 y)).
- fitness sigmoid as 1/(1+exp(-x)) for exact fp32 saturation; top-k via stable
  rank (ties broken by index like jax.lax.top_k); selection as one-hot P.
- A2 = S_sel^T A S_sel needed only as boolean -> bf16 0/1 count matmuls.
- global_mean_pool of the selected half as a fitness-weighted matmul.

Conv outputs live in a [64, heads, n] transposed layout (partition rows 0:64)
so every attention/normalize op is partition-aligned.
"""
import numpy as np

import concourse.bass as bass
import concourse.tile as tile
from concourse import bacc, mybir
from concourse.bass_utils import run_bass_kernel_spmd
from concourse.masks import make_identity

F32, BF16, F32R = mybir.dt.float32, mybir.dt.bfloat16, mybir.dt.float32r
AF = mybir.ActivationFunctionType
ALU = mybir.AluOpType
AX = mybir.AxisListType

B, N1, F0 = 8, 1024, 32
H1, H2 = 5, 3
HD1, HD2 = 320, 192
DH = 64
N2 = 512
PEXP = 20.0
EPS_DEN = 1e-30
NB1, NB2 = 8, 4
KBS1, KBS2 = [F0], [128, 128, 64]     # conv input feature blocks


def _r(ap):
    # fp32r needs producer-side rounding (walrus invariant); plain fp32 for now
    return ap


def build_program(scal):
    nc = bacc.Bacc("TRN2", target_bir_lowering=False, debug=False, num_devices=8)
    din = {}

    def inp(name, shape, dtype=F32):
        din[name] = nc.dram_tensor(name, shape, dtype, kind="ExternalInput").ap()

    inp("xT0", [F0, N1], BF16)
    inp("mask1c", [N1, N1], BF16)
    inp("mask1", [N1, N1], BF16)
    inp("mask1T", [N1, N1], BF16)
    inp("deg1c", [N1, 1]); inp("evcd", [5, 1])
    inp("wq1", [128, HD1], BF16); inp("wk1", [128, HD1], BF16)
    inp("wv1a", [128, H1 * (DH + 1)], BF16); inp("wsk1", [128, HD1], BF16)
    inp("bq1c", [3 * 128, 1]); inp("bk1c", [3 * 128, 1])
    inp("bvsk1c", [H1 * DH, 1])
    inp("attx1", [H1 * DH, 1]); inp("w1bc", [128, HD1]); inp("lew1", [H1 * DH, 3])
    inp("wq2", [3 * 128, HD2], BF16); inp("wk2", [3 * 128, HD2], BF16)
    inp("wv2a", [3 * 128, H2 * (DH + 1)], BF16); inp("wsk2", [3 * 128, HD2], BF16)
    inp("bq2c", [2 * 128, 1]); inp("bk2c", [2 * 128, 1])
    inp("bvsk2c", [H2 * DH, 1])
    inp("attx2", [H2 * DH, 1]); inp("w2bc", [128, HD2]); inp("lew2", [H2 * DH, 3])
    inp("mw1", [2 * 128, HD1]); inp("mw2", [3 * 128, HD1]); inp("mw3", [3 * 128, 4])
    inp("mb1c", [3 * 128, 1]); inp("mb2c", [3 * 128, 1]); inp("mb3c", [4, 1])
    inp("iotabc", [128, N1]); inp("iotac", [N1, 1])
    out_d = nc.dram_tensor("out", [4, 1], F32, kind="ExternalOutput").ap()

    with tile.TileContext(nc) as tc:
        from contextlib import ExitStack
        with ExitStack() as ctx:
            _Prog(ctx, tc, nc, din, scal).run(out_d)
    nc.compile()
    return nc


class _Prog:
    def __init__(self, ctx, tc, nc, din, scal):
        self.ctx, self.tc, self.nc, self.din, self.scal = ctx, tc, nc, din, scal
        self.const = ctx.enter_context(tc.tile_pool(name="const", bufs=1))
        self.big = ctx.enter_context(tc.tile_pool(name="big", bufs=1))
        self.work = ctx.enter_context(tc.tile_pool(name="work", bufs=1))
        self.sm = ctx.enter_context(tc.tile_pool(name="sm", bufs=1))
        self.pipe = ctx.enter_context(tc.tile_pool(name="pipe", bufs=2))

    def load(self, name, shape, dtype=F32, pool=None, rearr=None, tag=None, p=128):
        pool = pool or self.const
        t = pool.tile(shape, dtype, tag=tag or name)
        src = self.din[name]
        if rearr is not None:
            src = src.rearrange(rearr, p=p)
        self.nc.sync.dma_start(out=t[:], in_=src)
        return t

    def bcast(self, pool_ps, row_ap, width, tag):
        nc = self.nc
        pb = pool_ps.tile([128, width], F32, tag="ps_bc")
        for t0 in range(0, width, 512):
            t1 = min(width, t0 + 512)
            nc.tensor.matmul(pb[:, t0:t1], self.onesr[:], row_ap[:, t0:t1],
                             start=True, stop=True)
        sb = self.sm.tile([128, width], F32, tag=tag)
        nc.scalar.copy(out=sb[:], in_=pb[:])
        return sb

    def cols2row(self, pool_ps, col3, blksizes, tag):
        nc = self.nc
        width = sum(blksizes)
        row = self.sm.tile([1, width], F32, tag=tag)
        o = 0
        for b, w in enumerate(blksizes):
            pt = pool_ps.tile([1, 128], F32, tag="ps_c2r")
            nc.tensor.transpose(pt[:, :w], col3[:w, b], self.identity[:w, :w])
            nc.scalar.copy(out=row[:, o: o + w], in_=pt[:, :w])
            o += w
        return row

    # ------------------------------------------------------------------
    def conv(self, xT, kbs, n, nb, heads, qkobs, wq, wk, wva, wsk,
             bqc, bkc, bvskc, mask_bf, sfx):
        """xT [<=128, KB, n] fp32 input (transposed). Returns xoT [64, heads, n]
        fp32 in work tag 'xoT': per-head feature rows at partitions 0:64."""
        nc, tc = self.nc, self.tc
        KB = len(kbs)
        nsl = [slice(t0, min(n, t0 + 512)) for t0 in range(0, n, 512)]
        qT = self.work.tile([128, len(qkobs), n], BF16, tag="qT")
        kT = self.work.tile([128, len(qkobs), n], BF16, tag="kT")
        xoT = self.work.tile([DH, heads, n], F32, tag="xoT")
        vaug = self.work.tile([128, nb, heads * (DH + 1)], BF16, tag="vaug")

        with tc.tile_pool(name="cvA" + sfx, bufs=2, space="PSUM") as pA, \
             tc.tile_pool(name="cvB" + sfx, bufs=1, space="PSUM") as pB, \
             tc.tile_pool(name="cvC" + sfx, bufs=1, space="PSUM") as pC:
            # q/k projections -> bf16 [128, OB, n]
            for w, dst, bias in ((wq, qT, bqc), (wk, kT, bkc)):
                for m, ob in enumerate(qkobs):
                    pm = pA.tile([128, n], F32, tag="A")
                    for sl in nsl:
                        for kb in range(KB):
                            nc.tensor.matmul(pm[:ob, sl],
                                             _r(w[: kbs[kb], kb, m * 128: m * 128 + ob]),
                                             _r(xT[: kbs[kb], kb, sl]),
                                             start=(kb == 0), stop=(kb == KB - 1))
                    nc.vector.tensor_scalar_add(out=dst[:ob, m], in0=pm[:ob],
                                                scalar1=bias[:ob, m])
            # skip projection -> xoT per head block [64, h, n]
            for h in range(heads):
                pm = pA.tile([128, n], F32, tag="A")
                for sl in nsl:
                    for kb in range(KB):
                        nc.tensor.matmul(pm[:DH, sl],
                                         _r(wsk[: kbs[kb], kb, h * DH:(h + 1) * DH]),
                                         _r(xT[: kbs[kb], kb, sl]),
                                         start=(kb == 0), stop=(kb == KB - 1))
                nc.scalar.copy(out=xoT[:, h], in_=pm[:DH])
            # v augmented
            for sb in range(nb):
                pv = pA.tile([128, n], F32, tag="A")
                w_ = heads * (DH + 1)
                for kb in range(KB):
                    nc.tensor.matmul(pv[:, :w_],
                                     _r(xT[: kbs[kb], kb, sb * 128:(sb + 1) * 128]),
                                     _r(wva[: kbs[kb], kb]),
                                     start=(kb == 0), stop=(kb == KB - 1))
                nc.scalar.copy(out=vaug[:, sb], in_=pv[:, :w_])
            v4 = vaug[:].rearrange("p b (h x) -> p b h x", h=heads)
            nc.gpsimd.memset(v4[:, :, :, DH: DH + 1], 1.0)

            isq = float(1.0 / np.sqrt(DH))
            for h in range(heads):
                mt, mo = divmod(h * DH, 128)
                pa = pB.tile([DH + 1, n], F32, tag="B")
                for sb in range(nb):
                    psc = pA.tile([128, n], F32, tag="A")
                    for sl in nsl:
                        nc.tensor.matmul(psc[:, sl],
                                         kT[mo: mo + DH, mt, sb * 128:(sb + 1) * 128],
                                         qT[mo: mo + DH, mt, sl],
                                         start=True, stop=True)
                    eb = self.pipe.tile([128, n], BF16, tag="eblk")
                    nc.scalar.activation(out=eb[:], in_=psc[:], func=AF.Exp, scale=isq)
                    nc.vector.tensor_tensor(out=eb[:], in0=eb[:], in1=mask_bf[:, sb],
                                            op=ALU.mult)
                    for sl in nsl:
                        nc.tensor.matmul(pa[:, sl], v4[:, sb, h, :], eb[:, sl],
                                         start=(sb == 0), stop=(sb == nb - 1))
                # den lives at partition DH(=64): recip there, broadcast via PE
                inv65 = self.sm.tile([DH + 1, n], F32, tag="inv65")
                nc.vector.tensor_scalar_add(out=inv65[DH: DH + 1], in0=pa[DH: DH + 1],
                                            scalar1=EPS_DEN)
                nc.vector.reciprocal(out=inv65[DH: DH + 1], in_=inv65[DH: DH + 1])
                pbc = pC.tile([DH, n], F32, tag="C")
                for sl in nsl:
                    nc.tensor.matmul(pbc[:, sl], self.ones65[DH: DH + 1, :DH],
                                     inv65[DH: DH + 1, sl], start=True, stop=True)
                invbc = self.sm.tile([DH, n], F32, tag="invbc")
                nc.scalar.copy(out=invbc[:], in_=pbc[:])
                att = self.sm.tile([DH, n], F32, tag="attnrm")
                nc.vector.tensor_tensor(out=att[:], in0=pa[:DH], in1=invbc[:],
                                        op=ALU.mult)
                nc.vector.tensor_tensor(out=xoT[:, h], in0=xoT[:, h], in1=att[:],
                                        op=ALU.add)
            for h in range(heads):
                nc.vector.tensor_scalar_add(out=xoT[:, h], in0=xoT[:, h],
                                            scalar1=bvskc[:, h])
        return xoT

    def transpose_to_aug(self, xT, n, nb, heads, hd, tag):
        nc, tc = self.nc, self.tc
        xaug = self.work.tile([128, nb, hd + 4], BF16, tag=tag)
        with tc.tile_pool(name="trp" + tag + str(n), bufs=2, space="PSUM") as pT:
            for sb in range(nb):
                for h in range(heads):
                    pt = pT.tile([128, DH], F32, tag="T")
                    nc.tensor.transpose(pt[:, :],
                                        xT[:, h, sb * 128:(sb + 1) * 128],
                                        self.identity[:DH, :DH])
                    nc.scalar.copy(out=xaug[:, sb, h * DH:(h + 1) * DH], in_=pt[:])
        return xaug

    # ------------------------------------------------------------------
    def pool(self, n, nb, heads, hd, xT, xaug, mask_bf, degc, attxc, wbc, lewc,
             cst, leb1, leb3, negleb3c, sfx, final):
        """xT: [64, heads, n] conv output; xaug [128, nb, hd+4] normal layout."""
        import os
        nc, tc = self.nc, self.tc
        sub = os.environ.get("POOL_STOP", "") if sfx == "1" else ""
        k = n // 2
        with tc.tile_pool(name="plA" + sfx, bufs=2, space="PSUM") as pA, \
             tc.tile_pool(name="plB" + sfx, bufs=1, space="PSUM") as pB, \
             tc.tile_pool(name="plS" + sfx, bufs=2, space="PSUM") as pS:
            g = self.sm.tile([DH, heads, 1], F32, tag="gcol")
            for h in range(heads):
                nc.vector.reduce_max(out=g[:, h], in_=xT[:, h], axis=AX.X)
            if sub == "g0":
                return g, g, g
            grow = self.cols2row(pS, g, [DH] * heads, "grow")
            gbc = self.bcast(pB, grow[:], hd, "gbc")
            pgbc = self.sm.tile([128, hd], F32, tag="pgbc")
            nc.vector.tensor_scalar_mul(out=pgbc[:], in0=gbc[:], scalar1=PEXP)
            if sub == "g":
                return g, g, g
            E = self.work.tile([128, nb, hd], BF16, tag="E")
            for sb in range(nb):
                y = self.pipe.tile([128, hd], F32, tag="yE")
                nc.vector.tensor_tensor(out=y[:], in0=xaug[:, sb, :hd], in1=gbc[:],
                                        op=ALU.subtract)
                nc.scalar.activation(out=E[:, sb], in_=y[:], func=AF.Exp, scale=PEXP)
            if sub == "E":
                return g, g, g
            stcol = self.sm.tile([128, nb, 1], F32, tag="stcol")
            for tb in range(nb):
                pL = pA.tile([128, hd], F32, tag="A")
                for sb in range(nb):
                    nc.tensor.matmul(pL[:], mask_bf[:, sb, tb * 128:(tb + 1) * 128],
                                     E[:, sb], start=(sb == 0), stop=(sb == nb - 1))
                L = self.pipe.tile([128, hd], F32, tag="Llse")
                nc.scalar.activation(out=L[:], in_=pL[:], func=AF.Ln)
                nc.vector.tensor_tensor(out=L[:], in0=L[:], in1=pgbc[:], op=ALU.add)
                scr = self.pipe.tile([128, hd], F32, tag="scrL")
                nc.vector.tensor_tensor(out=scr[:], in0=L[:], in1=wbc[:, :hd],
                                        op=ALU.mult)
                nc.vector.tensor_scalar_mul(out=scr[:], in0=scr[:],
                                            scalar1=float(1.0 / PEXP))
                nc.vector.reduce_sum(out=stcol[:, tb], in_=scr[:], axis=AX.X)
            strow = self.cols2row(pS, stcol, [128] * nb, "strow")
            stbc = self.bcast(pB, strow[:], n, "stfbc")
            sscol = self.sm.tile([128, nb, 1], F32, tag="sscol")
            ss2col = self.sm.tile([128, nb, 1], F32, tag="ss2col")
            for sb in range(nb):
                pss = pS.tile([128, 4], F32, tag="ps_s4")
                for h in range(heads):
                    nc.tensor.matmul(pss[:, 0:1],
                                     _r(xT[:, h, sb * 128:(sb + 1) * 128]),
                                     _r(attxc[:, h]),
                                     start=(h == 0), stop=(h == heads - 1))
                nc.vector.tensor_scalar_add(out=sscol[:, sb], in0=pss[:, 0:1],
                                            scalar1=float(cst))
                nc.vector.tensor_scalar_mul(out=ss2col[:, sb], in0=sscol[:, sb],
                                            scalar1=0.2)
            for sb in range(nb):
                pxw = pS.tile([128, 4], F32, tag="ps_s4")
                for h in range(heads):
                    nc.tensor.matmul(pxw[:, 0:3],
                                     _r(xT[:, h, sb * 128:(sb + 1) * 128]),
                                     _r(lewc[:, h]),
                                     start=(h == 0), stop=(h == heads - 1))
                nc.scalar.copy(out=xaug[:, sb, hd: hd + 3], in_=pxw[:, 0:3])
            nc.gpsimd.memset(xaug[:, :, hd + 3: hd + 4], 1.0)
        if sub == "lse":
            return stcol[:, 0:1], stcol[:, 0:1], stcol[:, 0:1]
        xnew = self.work.tile([128, nb, hd], BF16, tag="xnew")
        dots = self.sm.tile([128, nb, 3], F32, tag="dots")
        acol = self.sm.tile([128, nb, 1], F32, tag="acol")
        with tc.tile_pool(name="plN" + sfx, bufs=1, space="PSUM") as pN:
            pxn = [pN.tile([128, hd + 4], F32, tag=f"xn{tb}", name=f"pxn{tb}") for tb in range(nb)]
            for sb in range(nb):
                e1 = self.pipe.tile([128, n], F32, tag="e1")
                nc.scalar.activation(out=e1[:], in_=stbc[:], func=AF.Exp,
                                     bias=sscol[:, sb], scale=1.0)
                e2 = self.pipe.tile([128, n], F32, tag="e2")
                nc.scalar.activation(out=e2[:], in_=stbc[:], func=AF.Exp,
                                     bias=ss2col[:, sb], scale=0.2)
                nc.vector.tensor_tensor(out=e1[:], in0=e1[:], in1=e2[:], op=ALU.max)
                eSb = self.pipe.tile([128, n], BF16, tag="eSb")
                nc.vector.tensor_tensor(out=eSb[:], in0=e1[:], in1=mask_bf[:, sb],
                                        op=ALU.mult)
                for tb in range(nb):
                    nc.tensor.matmul(pxn[tb][:], eSb[:, tb * 128:(tb + 1) * 128],
                                     xaug[:, sb],
                                     start=(sb == 0), stop=(sb == nb - 1))
            for tb in range(nb):
                inv = self.sm.tile([128, 1], F32, tag="invxn")
                nc.vector.tensor_scalar_add(out=inv[:], in0=pxn[tb][:, hd + 3: hd + 4],
                                            scalar1=EPS_DEN)
                nc.vector.reciprocal(out=inv[:], in_=inv[:])
                nc.vector.tensor_scalar_mul(out=xnew[:, tb], in0=pxn[tb][:, :hd],
                                            scalar1=inv[:])
                nc.vector.tensor_scalar_mul(out=dots[:, tb],
                                            in0=pxn[tb][:, hd: hd + 3], scalar1=inv[:])
                nc.vector.tensor_scalar_add(out=acol[:, tb], in0=dots[:, tb, 0:1],
                                            scalar1=float(leb1))
        if sub == "xnew":
            return acol, acol, acol
        fit = self.sm.tile([128, nb, 1], F32, tag="fit")
        with tc.tile_pool(name="plG" + sfx, bufs=1, space="PSUM") as pG:
            pag = [pG.tile([128, 1], F32, tag=f"ag{tb}", name=f"pag{tb}") for tb in range(nb)]
            for sb in range(nb):
                mf = self.pipe.tile([128, n], F32, tag="maskf")
                nc.vector.tensor_copy(out=mf[:], in_=mask_bf[:, sb])
                for tb in range(nb):
                    nc.tensor.matmul(pag[tb][:], mf[:, tb * 128:(tb + 1) * 128],
                                     acol[:, sb], start=(sb == 0), stop=(sb == nb - 1))
            for tb in range(nb):
                t2 = self.sm.tile([128, 1], F32, tag="ft2")
                nc.vector.tensor_tensor(out=t2[:], in0=degc[:, tb], in1=dots[:, tb, 1:2],
                                        op=ALU.mult)
                nc.vector.tensor_tensor(out=t2[:], in0=pag[tb][:], in1=t2[:],
                                        op=ALU.subtract)
                nc.vector.tensor_tensor(out=t2[:], in0=t2[:], in1=dots[:, tb, 2:3],
                                        op=ALU.add)
                nc.vector.tensor_scalar_max(out=t2[:], in0=t2[:],
                                            scalar1=float(-85.0 - leb3))
                nc.scalar.activation(out=t2[:], in_=t2[:], func=AF.Exp, scale=-1.0,
                                     bias=negleb3c[:])
                nc.vector.tensor_scalar_add(out=t2[:], in0=t2[:], scalar1=1.0)
                nc.vector.reciprocal(out=fit[:, tb], in_=t2[:])
        if sub == "fit":
            return fit, fit, fit
        with tc.tile_pool(name="plR" + sfx, bufs=1, space="PSUM") as pR, \
             tc.tile_pool(name="plRs" + sfx, bufs=2, space="PSUM") as pRs:
            fitrow = self.cols2row(pRs, fit, [128] * nb, "fitrow")
            fitbc = self.bcast(pR, fitrow[:], n, "stfbc")
            rank = self.sm.tile([128, nb, 1], F32, tag="rank")
            for tb in range(nb):
                gts = self.pipe.tile([128, n], F32, tag="e1")
                gtc = self.sm.tile([128, 1], F32, tag="gtc")
                nc.vector.tensor_scalar(out=gts[:], in0=fitbc[:], scalar1=fit[:, tb],
                                        scalar2=None, op0=ALU.is_gt, op1=ALU.add,
                                        accum_out=gtc[:])
                eq = self.pipe.tile([128, n], F32, tag="e2")
                nc.vector.tensor_scalar(out=eq[:], in0=fitbc[:], scalar1=fit[:, tb],
                                        scalar2=None, op0=ALU.is_equal)
                lt = self.pipe.tile([128, n], F32, tag="maskf")
                nc.vector.tensor_scalar(out=lt[:], in0=self.iotabc[:, :n],
                                        scalar1=self.iotac[:, tb],
                                        scalar2=None, op0=ALU.is_lt)
                scr2 = self.pipe.tile([128, n], F32, tag="e1")
                eqlt = self.sm.tile([128, 1], F32, tag="eqlt")
                nc.vector.tensor_tensor(out=scr2[:], in0=eq[:], in1=lt[:], op=ALU.mult)
                nc.vector.reduce_sum(out=eqlt[:], in_=scr2[:], axis=AX.X)
                nc.vector.tensor_tensor(out=rank[:, tb], in0=gtc[:], in1=eqlt[:],
                                        op=ALU.add)
            if final:
                wsel = self.sm.tile([128, nb, 1], BF16, tag="wsel")
                for tb in range(nb):
                    nc.vector.tensor_scalar(out=wsel[:, tb], in0=rank[:, tb],
                                            scalar1=float(k), scalar2=fit[:, tb],
                                            op0=ALU.is_lt, op1=ALU.mult)
                pgm = pRs.tile([1, hd], F32, tag="ps_gm")
                for tb in range(nb):
                    nc.tensor.matmul(pgm[:], wsel[:, tb], xnew[:, tb],
                                     start=(tb == 0), stop=(tb == nb - 1))
                gmrow = self.sm.tile([1, hd], F32, tag="gmrow")
                nc.scalar.mul(out=gmrow[:], in_=pgm[:], mul=float(1.0 / k))
                return gmrow
            Pt = self.work.tile([128, nb, k], BF16, tag="Pb")
            for tb in range(nb):
                nc.vector.tensor_scalar(out=Pt[:, tb], in0=self.iotabc[:, :k],
                                        scalar1=rank[:, tb], scalar2=fit[:, tb],
                                        op0=ALU.is_equal, op1=ALU.mult)
            return Pt, Pt, xnew

    # ------------------------------------------------------------------
    def run(self, out_d):
        nc, tc, scal = self.nc, self.tc, self.scal
        self.identity = self.const.tile([128, 128], F32, tag="identity")
        make_identity(nc, self.identity[:])
        self.onesr = self.const.tile([1, 128], F32, tag="onesr")
        nc.vector.memset(self.onesr[:], 1.0)
        self.ones65 = self.const.tile([DH + 1, 128], F32, tag="ones65")
        nc.vector.memset(self.ones65[:], 1.0)
        self.iotabc = self.load("iotabc", [128, N1])
        self.iotac = self.load("iotac", [128, NB1, 1], rearr="(b p) o -> p b o")

        xT0 = self.const.tile([F0, 1, N1], BF16, tag="xT0")
        nc.sync.dma_start(out=xT0[:, 0], in_=self.din["xT0"])
        mask1c = self.work.tile([128, NB1, N1], BF16, tag="bigshare")
        nc.sync.dma_start(out=mask1c[:],
                          in_=self.din["mask1c"].rearrange("(b p) t -> p b t", p=128))
        mask1 = self.load("mask1", [128, NB1, N1], BF16, pool=self.big,
                          rearr="(b p) t -> p b t")
        deg1c = self.load("deg1c", [128, NB1, 1], rearr="(b p) o -> p b o")

        def wload(name, kb, cols, p=128, dtype=F32):
            return self.load(name, [p, kb, cols], dtype, rearr="(b p) c -> p b c", p=p)

        wq1 = wload("wq1", 1, HD1, dtype=BF16); wk1 = wload("wk1", 1, HD1, dtype=BF16)
        wv1a = wload("wv1a", 1, H1 * (DH + 1), dtype=BF16); wsk1 = wload("wsk1", 1, HD1, dtype=BF16)
        bq1c = wload("bq1c", 3, 1); bk1c = wload("bk1c", 3, 1)
        bvsk1c = wload("bvsk1c", H1, 1, p=DH)
        attx1 = wload("attx1", H1, 1, p=DH); lew1 = wload("lew1", H1, 3, p=DH)
        w1bc = self.load("w1bc", [128, HD1])
        negleb3_1 = self.const.tile([128, 1], F32, tag="ngl1")
        nc.vector.memset(negleb3_1[:], float(-scal["le_b3_1"]))
        negleb3_2 = self.const.tile([128, 1], F32, tag="ngl2")
        nc.vector.memset(negleb3_2[:], float(-scal["le_b3_2"]))

        import os
        stop_after = os.environ.get("STOP_AFTER", "")

        def bail(src_ap):
            outc = self.sm.tile([4, 1], F32, tag="outc")
            nc.vector.tensor_copy(out=outc[:], in_=src_ap)
            nc.sync.dma_start(out=out_d, in_=outc[:])

        # ---------------- stage 1 ----------------
        x1T = self.conv(xT0, KBS1, N1, NB1, H1, [128, 128, 64],
                        wq1, wk1, wv1a, wsk1, bq1c, bk1c, bvsk1c, mask1c, "1")
        if stop_after == "conv1":
            return bail(x1T[:4, 0, 0:1])
        x1aug = self.transpose_to_aug(x1T, N1, NB1, H1, HD1, "bigshare")
        if stop_after == "aug1":
            return bail(x1aug[:4, 0, 0:1])
        Pt, Pb, xnew1 = self.pool(N1, NB1, H1, HD1, x1T, x1aug, mask1, deg1c,
                                  attx1, w1bc, lew1, scal["cst1"], scal["le_b1_1"],
                                  scal["le_b3_1"], negleb3_1, "1", final=False)

        if stop_after == "pool1":
            return bail(Pt[:4, 0, 0:1])
        # ---------------- stage-2 glue ----------------
        x2 = self.work.tile([128, NB2, HD1], BF16, tag="E")
        with tc.tile_pool(name="g2a", bufs=2, space="PSUM") as pX:
            for qb in range(NB2):
                px2 = pX.tile([128, HD1], F32, tag="X")
                for tb in range(NB1):
                    nc.tensor.matmul(px2[:], Pt[:, tb, qb * 128:(qb + 1) * 128],
                                     xnew1[:, tb],
                                     start=(tb == 0), stop=(tb == NB1 - 1))
                nc.scalar.copy(out=x2[:, qb], in_=px2[:])
        x2T = self.work.tile([128, 3, N2], BF16, tag="x2T")
        with tc.tile_pool(name="g2t", bufs=2, space="PSUM") as pT:
            identb = self.sm.tile([128, 128], BF16, tag="identb")
            nc.vector.tensor_copy(out=identb[:], in_=self.identity[:])
            for qb in range(NB2):
                for m, ob in enumerate(KBS2):
                    pt = pT.tile([128, 128], BF16, tag="T")
                    nc.tensor.transpose(pt[:ob, :],
                                        x2[:, qb, m * 128: m * 128 + ob],
                                        identb[:])
                    nc.scalar.copy(out=x2T[:ob, m, qb * 128:(qb + 1) * 128],
                                   in_=pt[:ob, :])
        mask1T = self.work.tile([128, NB1, N1], BF16, tag="bigshare")
        nc.sync.dma_start(out=mask1T[:],
                          in_=self.din["mask1T"].rearrange("(b p) t -> p b t", p=128))
        Sb = self.work.tile([128, NB1, N2], BF16, tag="kT")
        Tb = self.work.tile([128, NB1, N2], BF16, tag="vaug")
        mask2 = self.big.tile([128, NB2, N2], BF16, tag="mask2")
        mask2b = self.big.tile([128, NB2, N2], BF16, tag="mask2b")
        deg2c = self.sm.tile([128, NB2, 1], F32, tag="deg2c")
        with tc.tile_pool(name="g2s", bufs=2, space="PSUM") as pG:
            for dst, rhs in ((Sb, Pb), (Tb, Sb)):
                for sb in range(NB1):
                    pp = pG.tile([128, N2], F32, tag="G")
                    for tb in range(NB1):
                        nc.tensor.matmul(pp[:], mask1T[:, tb, sb * 128:(sb + 1) * 128],
                                         rhs[:, tb],
                                         start=(tb == 0), stop=(tb == NB1 - 1))
                    nc.scalar.copy(out=dst[:, sb], in_=pp[:])
            for pb in range(NB2):
                pc = pG.tile([128, N2], F32, tag="G")
                for sb in range(NB1):
                    nc.tensor.matmul(pc[:], Sb[:, sb, pb * 128:(pb + 1) * 128],
                                     Tb[:, sb], start=(sb == 0), stop=(sb == NB1 - 1))
                m2f = self.sm.tile([128, N2], F32, tag="m2f")
                nc.vector.tensor_scalar(out=m2f[:], in0=pc[:], scalar1=0.5,
                                        scalar2=None, op0=ALU.is_gt)
                ne = self.sm.tile([128, N2], F32, tag="m2ne")
                nc.vector.tensor_scalar(out=ne[:], in0=self.iotabc[:, :N2],
                                        scalar1=self.iotac[:, pb], scalar2=None,
                                        op0=ALU.not_equal)
                nc.vector.tensor_tensor(out=m2f[:], in0=m2f[:], in1=ne[:], op=ALU.mult)
                nc.vector.tensor_copy(out=mask2[:, pb], in_=m2f[:])
                nc.vector.tensor_scalar(out=ne[:], in0=self.iotabc[:, :N2],
                                        scalar1=self.iotac[:, pb], scalar2=None,
                                        op0=ALU.is_equal)
                nc.vector.tensor_tensor(out=m2f[:], in0=m2f[:], in1=ne[:], op=ALU.max)
                nc.vector.tensor_copy(out=mask2b[:, pb], in_=m2f[:])
            onecb = self.sm.tile([128, 1], BF16, tag="onecb")
            nc.vector.memset(onecb[:], 1.0)
            pdg = pG.tile([1, N2], F32, tag="Gd")
            for pb in range(NB2):
                nc.tensor.matmul(pdg[:], onecb[:], mask2b[:, pb],
                                 start=(pb == 0), stop=(pb == NB2 - 1))
            degrow = self.sm.tile([1, N2], F32, tag="degrow")
            nc.scalar.copy(out=degrow[:], in_=pdg[:])
            for qb in range(NB2):
                ptd = pG.tile([128, 1], F32, tag="Gt")
                nc.tensor.transpose(ptd[:, :], degrow[:, qb * 128:(qb + 1) * 128],
                                    self.identity[:1, :1])
                nc.scalar.copy(out=deg2c[:, qb], in_=ptd[:])

        if stop_after == "glue":
            return bail(deg2c[:4, 0])
        # ---------------- stage 2 ----------------
        wq2 = wload("wq2", 3, HD2, dtype=BF16); wk2 = wload("wk2", 3, HD2, dtype=BF16)
        wv2a = wload("wv2a", 3, H2 * (DH + 1), dtype=BF16); wsk2 = wload("wsk2", 3, HD2, dtype=BF16)
        bq2c = wload("bq2c", 2, 1); bk2c = wload("bk2c", 2, 1)
        bvsk2c = wload("bvsk2c", H2, 1, p=DH)
        attx2 = wload("attx2", H2, 1, p=DH); lew2 = wload("lew2", H2, 3, p=DH)
        w2bc = self.load("w2bc", [128, HD2])

        x3T = self.conv(x2T, KBS2, N2, NB2, H2, [128, 64],
                        wq2, wk2, wv2a, wsk2, bq2c, bk2c, bvsk2c, mask2, "2")
        if stop_after == "conv2":
            return bail(x3T[:4, 0, 0:1])
        x3aug = self.transpose_to_aug(x3T, N2, NB2, H2, HD2, "bigshare")
        gmrow = self.pool(N2, NB2, H2, HD2, x3T, x3aug, mask2b, deg2c,
                          attx2, w2bc, lew2, scal["cst2"], scal["le_b1_2"],
                          scal["le_b3_2"], negleb3_2, "2", final=True)

        if stop_after == "pool2":
            outc = self.sm.tile([4, 1], F32, tag="outc")
            nc.vector.memset(outc[:], 0.0)
            nc.vector.tensor_copy(out=outc[0:1, :], in_=gmrow[0:1, 0:1])
            nc.sync.dma_start(out=out_d, in_=outc[:])
            return
        # ---------------- MLP ----------------
        mw1 = wload("mw1", 2, HD1); mw2 = wload("mw2", 3, HD1); mw3 = wload("mw3", 3, 4)
        mb1c = wload("mb1c", 3, 1); mb2c = wload("mb2c", 3, 1)
        mb3c = self.load("mb3c", [4, 1])
        obs3 = [128, 128, 64]
        with tc.tile_pool(name="mlpp", bufs=2, space="PSUM") as pM:
            merge = self.sm.tile([128, 2, 1], F32, tag="merge")
            pm0 = pM.tile([128, 1], F32, tag="Mt")
            nc.tensor.transpose(pm0[:, :], gmrow[:, :128], self.identity[:1, :1])
            nc.scalar.copy(out=merge[:, 0], in_=pm0[:])
            pm1 = pM.tile([128, 1], F32, tag="Mt")
            nc.tensor.transpose(pm1[:64, :], gmrow[:, 128:192], self.identity[:1, :1])
            nc.scalar.copy(out=merge[:64, 1], in_=pm1[:64, :])
            nc.sync.dma_start(out=merge[64:69, 1], in_=self.din["evcd"])
            kbs1 = [128, 69]
            h1 = self.sm.tile([128, 3, 1], F32, tag="h1col")
            for m in range(3):
                ph = pM.tile([128, 1], F32, tag="Mm")
                for kb in range(2):
                    nc.tensor.matmul(ph[: obs3[m], :],
                                     _r(mw1[: kbs1[kb], kb, m * 128: m * 128 + obs3[m]]),
                                     _r(merge[: kbs1[kb], kb]),
                                     start=(kb == 0), stop=(kb == 1))
                nc.scalar.activation(out=h1[: obs3[m], m], in_=ph[: obs3[m], :],
                                     func=AF.Relu, bias=mb1c[: obs3[m], m])
            h2 = self.sm.tile([128, 3, 1], F32, tag="h2col")
            for m in range(3):
                ph = pM.tile([128, 1], F32, tag="Mm")
                for kb in range(3):
                    nc.tensor.matmul(ph[: obs3[m], :],
                                     _r(mw2[: obs3[kb], kb, m * 128: m * 128 + obs3[m]]),
                                     _r(h1[: obs3[kb], kb]),
                                     start=(kb == 0), stop=(kb == 2))
                nc.scalar.activation(out=h2[: obs3[m], m], in_=ph[: obs3[m], :],
                                     func=AF.Relu, bias=mb2c[: obs3[m], m])
            po = pM.tile([128, 1], F32, tag="Mo")
            for kb in range(3):
                nc.tensor.matmul(po[:4, 0:1], _r(mw3[: obs3[kb], kb, :]),
                                 _r(h2[: obs3[kb], kb]),
                                 start=(kb == 0), stop=(kb == 2))
            outc = self.sm.tile([4, 1], F32, tag="outc")
            nc.vector.tensor_tensor(out=outc[:], in0=po[:4, 0:1], in1=mb3c[:],
                                    op=ALU.add)
            nc.sync.dma_start(out=out_d, in_=outc[:])


# ======================================================================
# host side
# ======================================================================

_CACHE = {}


def _pad_rows(a, rows):
    out = np.zeros((rows, a.shape[1]), np.float32)
    out[: a.shape[0]] = a
    return out


def _prep_shared(inputs):
    tc1, tc2 = inputs["tc1"], inputs["tc2"]
    p1, p2 = inputs["pool1"], inputs["pool2"]
    mlp = inputs["mlp"]
    f = lambda a: np.asarray(a, np.float32)

    def vaug_pack(Wv, heads):
        fin = Wv.shape[0]
        out = np.zeros((fin, heads * (DH + 1)), np.float32)
        for h in range(heads):
            out[:, h * (DH + 1): h * (DH + 1) + DH] = Wv[:, h * DH:(h + 1) * DH]
        return out

    d = {}
    d["wq1"] = _pad_rows(f(tc1["Wq"]), 128)
    d["wk1"] = _pad_rows(f(tc1["Wk"]), 128)
    d["wv1a"] = _pad_rows(vaug_pack(f(tc1["Wv"]), H1), 128)
    d["wsk1"] = _pad_rows(f(tc1["Wskip"]), 128)
    d["bq1c"] = _pad_rows(f(tc1["bq"])[:, None], 3 * 128)
    d["bk1c"] = _pad_rows(f(tc1["bk"])[:, None], 3 * 128)
    d["bvsk1c"] = (f(tc1["bv"]) + f(tc1["bskip"]))[:, None].copy()
    d["attx1"] = f(p1["att_x"])[:, None].copy()
    w1 = f(p1["Wlin"]) @ f(p1["att_q"])
    d["w1bc"] = np.tile(w1[None, :], (128, 1)).astype(np.float32)
    d["lew1"] = np.stack([f(p1["le_W1"])[:, 0], f(p1["le_W2"])[:, 0],
                          f(p1["le_W3"])[:, 0]], axis=1).astype(np.float32)
    d["wq2"] = _pad_rows(f(tc2["Wq"]), 3 * 128)
    d["wk2"] = _pad_rows(f(tc2["Wk"]), 3 * 128)
    d["wv2a"] = _pad_rows(vaug_pack(f(tc2["Wv"]), H2), 3 * 128)
    d["wsk2"] = _pad_rows(f(tc2["Wskip"]), 3 * 128)
    d["bq2c"] = _pad_rows(f(tc2["bq"])[:, None], 2 * 128)
    d["bk2c"] = _pad_rows(f(tc2["bk"])[:, None], 2 * 128)
    d["bvsk2c"] = (f(tc2["bv"]) + f(tc2["bskip"]))[:, None].copy()
    d["attx2"] = f(p2["att_x"])[:, None].copy()
    w2 = f(p2["Wlin"]) @ f(p2["att_q"])
    d["w2bc"] = np.tile(w2[None, :], (128, 1)).astype(np.float32)
    d["lew2"] = np.stack([f(p2["le_W1"])[:, 0], f(p2["le_W2"])[:, 0],
                          f(p2["le_W3"])[:, 0]], axis=1).astype(np.float32)
    d["mw1"] = _pad_rows(f(mlp["W1"]), 2 * 128)
    d["mw2"] = _pad_rows(f(mlp["W2"]), 3 * 128)
    d["mw3"] = _pad_rows(f(mlp["W3"]), 3 * 128)
    d["mb1c"] = _pad_rows(f(mlp["b1"])[:, None], 3 * 128)
    d["mb2c"] = _pad_rows(f(mlp["b2"])[:, None], 3 * 128)
    d["mb3c"] = f(mlp["b3"])[:, None].copy()
    d["iotabc"] = np.tile(np.arange(N1, dtype=np.float32)[None, :], (128, 1))
    d["iotac"] = np.arange(N1, dtype=np.float32)[:, None]
    scal = {
        "cst1": float(f(p1["blin"]) @ f(p1["att_q"]) + f(p1["att_b"])),
        "cst2": float(f(p2["blin"]) @ f(p2["att_q"]) + f(p2["att_b"])),
        "le_b1_1": float(f(p1["le_b1"])[0]), "le_b3_1": float(f(p1["le_b3"])[0]),
        "le_b1_2": float(f(p2["le_b1"])[0]), "le_b3_2": float(f(p2["le_b3"])[0]),
    }
    return scal, d


def make_in_maps(inputs):
    import ml_dtypes
    BFH = ml_dtypes.bfloat16
    nodes = np.asarray(inputs["nodes"], np.float32)
    ei = np.asarray(inputs["edge_index"])
    ev = np.asarray(inputs["exp_value"], np.float32)
    cd = np.asarray(inputs["circuit_depth"], np.float32)
    scal, shared = _prep_shared(inputs)
    for w in ["wq1", "wk1", "wv1a", "wsk1", "wq2", "wk2", "wv2a", "wsk2"]:
        shared[w] = shared[w].astype(BFH)
    src, dst = ei[0], ei[1]
    gid = src // N1
    in_maps = []
    for b in range(B):
        m = gid == b
        A = np.zeros((N1, N1), bool)
        A[src[m] % N1, dst[m] % N1] = True
        mask1 = A.copy()
        np.fill_diagonal(mask1, True)
        xg = nodes[b * N1:(b + 1) * N1]
        im = dict(shared)
        im["xT0"] = np.ascontiguousarray(xg.T).astype(ml_dtypes.bfloat16)
        im["mask1c"] = A.astype(ml_dtypes.bfloat16)
        im["mask1"] = mask1.astype(ml_dtypes.bfloat16)
        im["mask1T"] = np.ascontiguousarray(mask1.T).astype(ml_dtypes.bfloat16)
        im["deg1c"] = mask1.sum(axis=0, dtype=np.float32)[:, None]
        im["evcd"] = np.concatenate([ev[b, 0], cd[b]])[:, None].astype(np.float32)
        in_maps.append(im)
    return scal, in_maps


def kernel(**inputs):
    scal, in_maps = make_in_maps(inputs)
    key = tuple(sorted(scal.items()))
    if key not in _CACHE:
        _CACHE[key] = build_program(scal)
    nc = _CACHE[key]
    res = run_bass_kernel_spmd(nc, in_maps, list(range(B)))
    out = np.stack([res.results[i]["out"][:, 0] for i in range(B)])
    return out.astype(np.float32)


# revision 19
# speedup vs baseline: 1.2274x; 1.0030x over previous
"""Trainium2 Bass kernel for nn_ExpValCircuitGraphModel (GNN message passing).

Sharding: data-parallel — one graph per NeuronCore (B=8 graphs on 8 cores).
Host does graph-format conversion only (dense 0/1 masks from edge_index,
parameter repacking); all model compute runs on-device.

Device algorithm (validated against the jax reference on host, rel err 6e-5):
- TransformerConv: scores^T on PE; softmax without max-shift; q/k/v/e bf16;
  e consumed per source block by the attention matmul, which also accumulates
  the softmax denominator via an appended ones-column on v.
- ASAP masked-max via mask-matmul log-sum-exp (per-feature shift, p=20);
  output measured insensitive to masked-max error up to +-0.3.
- exp(leaky_relu(y)) == max(exp(y), exp(0.2 y)).
- fitness sigmoid as 1/(1+exp(-x)) for exact fp32 saturation; top-k via stable
  rank (ties broken by index like jax.lax.top_k); selection as one-hot P.
- A2 = S_sel^T A S_sel needed only as boolean -> bf16 0/1 count matmuls.
- global_mean_pool of the selected half as a fitness-weighted matmul.

Conv outputs live in a [64, heads, n] transposed layout (partition rows 0:64)
so every attention/normalize op is partition-aligned.
"""
import numpy as np

import concourse.bass as bass
import concourse.tile as tile
from concourse import bacc, mybir
from concourse.bass_utils import run_bass_kernel_spmd
from concourse.masks import make_identity

F32, BF16, F32R = mybir.dt.float32, mybir.dt.bfloat16, mybir.dt.float32r
AF = mybir.ActivationFunctionType
ALU = mybir.AluOpType
AX = mybir.AxisListType

B, N1, F0 = 8, 1024, 32
H1, H2 = 5, 3
HD1, HD2 = 320, 192
DH = 64
N2 = 512
PEXP = 20.0
EPS_DEN = 1e-30
NB1, NB2 = 8, 4
KBS1, KBS2 = [F0], [128, 128, 64]     # conv input feature blocks


def _r(ap):
    # fp32r needs producer-side rounding (walrus invariant); plain fp32 for now
    return ap


def build_program(scal):
    nc = bacc.Bacc("TRN2", target_bir_lowering=False, debug=False, num_devices=8)
    din = {}

    def inp(name, shape, dtype=F32):
        din[name] = nc.dram_tensor(name, shape, dtype, kind="ExternalInput").ap()

    inp("xT0", [F0, N1], BF16)
    inp("mask1c", [N1, N1], BF16)
    inp("mask1", [N1, N1], BF16)
    inp("mask1T", [N1, N1], BF16)
    inp("deg1c", [N1, 1]); inp("evcd", [5, 1])
    inp("wq1", [128, HD1], BF16); inp("wk1", [128, HD1], BF16)
    inp("wv1a", [128, H1 * (DH + 1)], BF16); inp("wsk1", [128, HD1], BF16)
    inp("bq1c", [3 * 128, 1]); inp("bk1c", [3 * 128, 1])
    inp("bvsk1c", [H1 * DH, 1])
    inp("attx1", [H1 * DH, 1]); inp("w1bc", [128, HD1]); inp("lew1", [H1 * DH, 3])
    inp("wq2", [3 * 128, HD2], BF16); inp("wk2", [3 * 128, HD2], BF16)
    inp("wv2a", [3 * 128, H2 * (DH + 1)], BF16); inp("wsk2", [3 * 128, HD2], BF16)
    inp("bq2c", [2 * 128, 1]); inp("bk2c", [2 * 128, 1])
    inp("bvsk2c", [H2 * DH, 1])
    inp("attx2", [H2 * DH, 1]); inp("w2bc", [128, HD2]); inp("lew2", [H2 * DH, 3])
    inp("mw1", [2 * 128, HD1]); inp("mw2", [3 * 128, HD1]); inp("mw3", [3 * 128, 4])
    inp("mb1c", [3 * 128, 1]); inp("mb2c", [3 * 128, 1]); inp("mb3c", [4, 1])
    inp("iotabc", [128, N1]); inp("iotac", [N1, 1])
    out_d = nc.dram_tensor("out", [4, 1], F32, kind="ExternalOutput").ap()

    with tile.TileContext(nc) as tc:
        from contextlib import ExitStack
        with ExitStack() as ctx:
            _Prog(ctx, tc, nc, din, scal).run(out_d)
    nc.compile()
    return nc


class _Prog:
    def __init__(self, ctx, tc, nc, din, scal):
        self.ctx, self.tc, self.nc, self.din, self.scal = ctx, tc, nc, din, scal
        self.const = ctx.enter_context(tc.tile_pool(name="const", bufs=1))
        self.big = ctx.enter_context(tc.tile_pool(name="big", bufs=1))
        self.work = ctx.enter_context(tc.tile_pool(name="work", bufs=1))
        self.sm = ctx.enter_context(tc.tile_pool(name="sm", bufs=1))
        self.pipe = ctx.enter_context(tc.tile_pool(name="pipe", bufs=2))

    def load(self, name, shape, dtype=F32, pool=None, rearr=None, tag=None, p=128):
        pool = pool or self.const
        t = pool.tile(shape, dtype, tag=tag or name)
        src = self.din[name]
        if rearr is not None:
            src = src.rearrange(rearr, p=p)
        self.nc.sync.dma_start(out=t[:], in_=src)
        return t

    def bcast(self, pool_ps, row_ap, width, tag):
        nc = self.nc
        pb = pool_ps.tile([128, width], F32, tag="ps_bc")
        for t0 in range(0, width, 512):
            t1 = min(width, t0 + 512)
            nc.tensor.matmul(pb[:, t0:t1], self.onesr[:], row_ap[:, t0:t1],
                             start=True, stop=True)
        sb = self.sm.tile([128, width], F32, tag=tag)
        nc.scalar.copy(out=sb[:], in_=pb[:])
        return sb

    def cols2row(self, pool_ps, col3, blksizes, tag):
        nc = self.nc
        width = sum(blksizes)
        row = self.sm.tile([1, width], F32, tag=tag)
        o = 0
        for b, w in enumerate(blksizes):
            pt = pool_ps.tile([1, 128], F32, tag="ps_c2r")
            nc.tensor.transpose(pt[:, :w], col3[:w, b], self.identity[:w, :w])
            nc.scalar.copy(out=row[:, o: o + w], in_=pt[:, :w])
            o += w
        return row

    # ------------------------------------------------------------------
    def conv(self, xT, kbs, n, nb, heads, qkobs, wq, wk, wva, wsk,
             bqc, bkc, bvskc, mask_bf, sfx):
        """xT [<=128, KB, n] fp32 input (transposed). Returns xoT [64, heads, n]
        fp32 in work tag 'xoT': per-head feature rows at partitions 0:64."""
        nc, tc = self.nc, self.tc
        KB = len(kbs)
        nsl = [slice(t0, min(n, t0 + 512)) for t0 in range(0, n, 512)]
        qT = self.work.tile([128, len(qkobs), n], BF16, tag="qT")
        kT = self.work.tile([128, len(qkobs), n], BF16, tag="kT")
        xoT = self.work.tile([DH, heads, n], F32, tag="xoT")
        vaug = self.work.tile([128, nb, heads * (DH + 1)], BF16, tag="vaug")

        with tc.tile_pool(name="cvA" + sfx, bufs=2, space="PSUM") as pA, \
             tc.tile_pool(name="cvB" + sfx, bufs=2, space="PSUM") as pB:
            # q/k projections -> bf16 [128, OB, n]
            for w, dst, bias in ((wq, qT, bqc), (wk, kT, bkc)):
                for m, ob in enumerate(qkobs):
                    pm = pA.tile([128, n], F32, tag="A")
                    for sl in nsl:
                        for kb in range(KB):
                            nc.tensor.matmul(pm[:ob, sl],
                                             _r(w[: kbs[kb], kb, m * 128: m * 128 + ob]),
                                             _r(xT[: kbs[kb], kb, sl]),
                                             start=(kb == 0), stop=(kb == KB - 1))
                    nc.vector.tensor_scalar_add(out=dst[:ob, m], in0=pm[:ob],
                                                scalar1=bias[:ob, m])
            # skip projection -> xoT per head block [64, h, n]
            for h in range(heads):
                pm = pA.tile([128, n], F32, tag="A")
                for sl in nsl:
                    for kb in range(KB):
                        nc.tensor.matmul(pm[:DH, sl],
                                         _r(wsk[: kbs[kb], kb, h * DH:(h + 1) * DH]),
                                         _r(xT[: kbs[kb], kb, sl]),
                                         start=(kb == 0), stop=(kb == KB - 1))
                nc.scalar.copy(out=xoT[:, h], in_=pm[:DH])
            # v augmented
            for sb in range(nb):
                pv = pA.tile([128, n], F32, tag="A")
                w_ = heads * (DH + 1)
                for kb in range(KB):
                    nc.tensor.matmul(pv[:, :w_],
                                     _r(xT[: kbs[kb], kb, sb * 128:(sb + 1) * 128]),
                                     _r(wva[: kbs[kb], kb]),
                                     start=(kb == 0), stop=(kb == KB - 1))
                nc.scalar.copy(out=vaug[:, sb], in_=pv[:, :w_])
            v4 = vaug[:].rearrange("p b (h x) -> p b h x", h=heads)
            nc.gpsimd.memset(v4[:, :, :, DH: DH + 1], 1.0)

            isq = float(1.0 / np.sqrt(DH))
            for h in range(heads):
                mt, mo = divmod(h * DH, 128)
                pa = pB.tile([DH + 1, n], F32, tag="B")
                for sb in range(nb):
                    psc = pA.tile([128, n], F32, tag="A")
                    for sl in nsl:
                        nc.tensor.matmul(psc[:, sl],
                                         kT[mo: mo + DH, mt, sb * 128:(sb + 1) * 128],
                                         qT[mo: mo + DH, mt, sl],
                                         start=True, stop=True)
                    eb = self.pipe.tile([128, n], BF16, tag="eblk")
                    nc.scalar.activation(out=eb[:], in_=psc[:], func=AF.Exp, scale=isq)
                    nc.vector.tensor_tensor(out=eb[:], in0=eb[:], in1=mask_bf[:, sb],
                                            op=ALU.mult)
                    for sl in nsl:
                        nc.tensor.matmul(pa[:, sl], v4[:, sb, h, :], eb[:, sl],
                                         start=(sb == 0), stop=(sb == nb - 1))
                # den lives at partition DH(=64): recip there, broadcast via PE
                inv65 = self.sm.tile([DH + 1, n], F32, tag="inv65")
                nc.vector.tensor_scalar_add(out=inv65[DH: DH + 1], in0=pa[DH: DH + 1],
                                            scalar1=EPS_DEN)
                nc.vector.reciprocal(out=inv65[DH: DH + 1], in_=inv65[DH: DH + 1])
                pbc = pA.tile([DH, n], F32, tag="A", name="pbc")
                for sl in nsl:
                    nc.tensor.matmul(pbc[:, sl], self.ones65[DH: DH + 1, :DH],
                                     inv65[DH: DH + 1, sl], start=True, stop=True)
                invbc = self.sm.tile([DH, n], F32, tag="invbc")
                nc.scalar.copy(out=invbc[:], in_=pbc[:])
                att = self.sm.tile([DH, n], F32, tag="attnrm")
                nc.vector.tensor_tensor(out=att[:], in0=pa[:DH], in1=invbc[:],
                                        op=ALU.mult)
                nc.vector.tensor_tensor(out=xoT[:, h], in0=xoT[:, h], in1=att[:],
                                        op=ALU.add)
            for h in range(heads):
                nc.vector.tensor_scalar_add(out=xoT[:, h], in0=xoT[:, h],
                                            scalar1=bvskc[:, h])
        return xoT

    def transpose_to_aug(self, xT, n, nb, heads, hd, tag):
        nc, tc = self.nc, self.tc
        xaug = self.work.tile([128, nb, hd + 4], BF16, tag=tag)
        with tc.tile_pool(name="trp" + tag + str(n), bufs=2, space="PSUM") as pT:
            for sb in range(nb):
                for h in range(heads):
                    pt = pT.tile([128, DH], F32, tag="T")
                    nc.tensor.transpose(pt[:, :],
                                        xT[:, h, sb * 128:(sb + 1) * 128],
                                        self.identity[:DH, :DH])
                    nc.scalar.copy(out=xaug[:, sb, h * DH:(h + 1) * DH], in_=pt[:])
        return xaug

    # ------------------------------------------------------------------
    def pool(self, n, nb, heads, hd, xT, xaug, mask_bf, degc, attxc, wbc, lewc,
             cst, leb1, leb3, negleb3c, sfx, final):
        """xT: [64, heads, n] conv output; xaug [128, nb, hd+4] normal layout."""
        import os
        nc, tc = self.nc, self.tc
        sub = os.environ.get("POOL_STOP", "") if sfx == "1" else ""
        k = n // 2
        with tc.tile_pool(name="plA" + sfx, bufs=2, space="PSUM") as pA, \
             tc.tile_pool(name="plB" + sfx, bufs=1, space="PSUM") as pB, \
             tc.tile_pool(name="plS" + sfx, bufs=2, space="PSUM") as pS:
            g = self.sm.tile([DH, heads, 1], F32, tag="gcol")
            for h in range(heads):
                nc.vector.reduce_max(out=g[:, h], in_=xT[:, h], axis=AX.X)
            if sub == "g0":
                return g, g, g
            grow = self.cols2row(pS, g, [DH] * heads, "grow")
            gbc = self.bcast(pB, grow[:], hd, "gbc")
            pgbc = self.sm.tile([128, hd], F32, tag="pgbc")
            nc.vector.tensor_scalar_mul(out=pgbc[:], in0=gbc[:], scalar1=PEXP)
            if sub == "g":
                return g, g, g
            E = self.work.tile([128, nb, hd], BF16, tag="E")
            for sb in range(nb):
                y = self.pipe.tile([128, hd], F32, tag="yE")
                nc.vector.tensor_tensor(out=y[:], in0=xaug[:, sb, :hd], in1=gbc[:],
                                        op=ALU.subtract)
                nc.scalar.activation(out=E[:, sb], in_=y[:], func=AF.Exp, scale=PEXP)
            if sub == "E":
                return g, g, g
            stcol = self.sm.tile([128, nb, 1], F32, tag="stcol")
            for tb in range(nb):
                pL = pA.tile([128, hd], F32, tag="A")
                for sb in range(nb):
                    nc.tensor.matmul(pL[:], mask_bf[:, sb, tb * 128:(tb + 1) * 128],
                                     E[:, sb], start=(sb == 0), stop=(sb == nb - 1))
                L = self.pipe.tile([128, hd], F32, tag="Llse")
                nc.scalar.activation(out=L[:], in_=pL[:], func=AF.Ln)
                nc.vector.tensor_tensor(out=L[:], in0=L[:], in1=pgbc[:], op=ALU.add)
                scr = self.pipe.tile([128, hd], F32, tag="scrL")
                nc.vector.tensor_tensor(out=scr[:], in0=L[:], in1=wbc[:, :hd],
                                        op=ALU.mult)
                nc.vector.tensor_scalar_mul(out=scr[:], in0=scr[:],
                                            scalar1=float(1.0 / PEXP))
                nc.vector.reduce_sum(out=stcol[:, tb], in_=scr[:], axis=AX.X)
            strow = self.cols2row(pS, stcol, [128] * nb, "strow")
            stbc = self.bcast(pB, strow[:], n, "stfbc")
            sscol = self.sm.tile([128, nb, 1], F32, tag="sscol")
            ss2col = self.sm.tile([128, nb, 1], F32, tag="ss2col")
            for sb in range(nb):
                pss = pS.tile([128, 4], F32, tag="ps_s4")
                for h in range(heads):
                    nc.tensor.matmul(pss[:, 0:1],
                                     _r(xT[:, h, sb * 128:(sb + 1) * 128]),
                                     _r(attxc[:, h]),
                                     start=(h == 0), stop=(h == heads - 1))
                nc.vector.tensor_scalar_add(out=sscol[:, sb], in0=pss[:, 0:1],
                                            scalar1=float(cst))
                nc.vector.tensor_scalar_mul(out=ss2col[:, sb], in0=sscol[:, sb],
                                            scalar1=0.2)
            for sb in range(nb):
                pxw = pS.tile([128, 4], F32, tag="ps_s4")
                for h in range(heads):
                    nc.tensor.matmul(pxw[:, 0:3],
                                     _r(xT[:, h, sb * 128:(sb + 1) * 128]),
                                     _r(lewc[:, h]),
                                     start=(h == 0), stop=(h == heads - 1))
                nc.scalar.copy(out=xaug[:, sb, hd: hd + 3], in_=pxw[:, 0:3])
            nc.gpsimd.memset(xaug[:, :, hd + 3: hd + 4], 1.0)
        if sub == "lse":
            return stcol[:, 0:1], stcol[:, 0:1], stcol[:, 0:1]
        xnew = self.work.tile([128, nb, hd], BF16, tag="xnew")
        dots = self.sm.tile([128, nb, 3], F32, tag="dots")
        acol = self.sm.tile([128, nb, 1], F32, tag="acol")
        with tc.tile_pool(name="plN" + sfx, bufs=1, space="PSUM") as pN:
            pxn = [pN.tile([128, hd + 4], F32, tag=f"xn{tb}", name=f"pxn{tb}") for tb in range(nb)]
            for sb in range(nb):
                e1 = self.pipe.tile([128, n], F32, tag="e1")
                nc.scalar.activation(out=e1[:], in_=stbc[:], func=AF.Exp,
                                     bias=sscol[:, sb], scale=1.0)
                e2 = self.pipe.tile([128, n], F32, tag="e2")
                nc.scalar.activation(out=e2[:], in_=stbc[:], func=AF.Exp,
                                     bias=ss2col[:, sb], scale=0.2)
                nc.vector.tensor_tensor(out=e1[:], in0=e1[:], in1=e2[:], op=ALU.max)
                eSb = self.pipe.tile([128, n], BF16, tag="eSb")
                nc.vector.tensor_tensor(out=eSb[:], in0=e1[:], in1=mask_bf[:, sb],
                                        op=ALU.mult)
                for tb in range(nb):
                    nc.tensor.matmul(pxn[tb][:], eSb[:, tb * 128:(tb + 1) * 128],
                                     xaug[:, sb],
                                     start=(sb == 0), stop=(sb == nb - 1))
            for tb in range(nb):
                inv = self.sm.tile([128, 1], F32, tag="invxn")
                nc.vector.tensor_scalar_add(out=inv[:], in0=pxn[tb][:, hd + 3: hd + 4],
                                            scalar1=EPS_DEN)
                nc.vector.reciprocal(out=inv[:], in_=inv[:])
                nc.vector.tensor_scalar_mul(out=xnew[:, tb], in0=pxn[tb][:, :hd],
                                            scalar1=inv[:])
                nc.vector.tensor_scalar_mul(out=dots[:, tb],
                                            in0=pxn[tb][:, hd: hd + 3], scalar1=inv[:])
                nc.vector.tensor_scalar_add(out=acol[:, tb], in0=dots[:, tb, 0:1],
                                            scalar1=float(leb1))
        if sub == "xnew":
            return acol, acol, acol
        fit = self.sm.tile([128, nb, 1], F32, tag="fit")
        with tc.tile_pool(name="plG" + sfx, bufs=1, space="PSUM") as pG:
            pag = [pG.tile([128, 1], F32, tag=f"ag{tb}", name=f"pag{tb}") for tb in range(nb)]
            for sb in range(nb):
                mf = self.pipe.tile([128, n], F32, tag="maskf")
                nc.vector.tensor_copy(out=mf[:], in_=mask_bf[:, sb])
                for tb in range(nb):
                    nc.tensor.matmul(pag[tb][:], mf[:, tb * 128:(tb + 1) * 128],
                                     acol[:, sb], start=(sb == 0), stop=(sb == nb - 1))
            for tb in range(nb):
                t2 = self.sm.tile([128, 1], F32, tag="ft2")
                nc.vector.tensor_tensor(out=t2[:], in0=degc[:, tb], in1=dots[:, tb, 1:2],
                                        op=ALU.mult)
                nc.vector.tensor_tensor(out=t2[:], in0=pag[tb][:], in1=t2[:],
                                        op=ALU.subtract)
                nc.vector.tensor_tensor(out=t2[:], in0=t2[:], in1=dots[:, tb, 2:3],
                                        op=ALU.add)
                nc.vector.tensor_scalar_max(out=t2[:], in0=t2[:],
                                            scalar1=float(-85.0 - leb3))
                nc.scalar.activation(out=t2[:], in_=t2[:], func=AF.Exp, scale=-1.0,
                                     bias=negleb3c[:])
                nc.vector.tensor_scalar_add(out=t2[:], in0=t2[:], scalar1=1.0)
                nc.vector.reciprocal(out=fit[:, tb], in_=t2[:])
        if sub == "fit":
            return fit, fit, fit
        with tc.tile_pool(name="plR" + sfx, bufs=1, space="PSUM") as pR, \
             tc.tile_pool(name="plRs" + sfx, bufs=2, space="PSUM") as pRs:
            fitrow = self.cols2row(pRs, fit, [128] * nb, "fitrow")
            fitbc = self.bcast(pR, fitrow[:], n, "stfbc")
            rank = self.sm.tile([128, nb, 1], F32, tag="rank")
            for tb in range(nb):
                gts = self.pipe.tile([128, n], F32, tag="e1")
                gtc = self.sm.tile([128, 1], F32, tag="gtc")
                nc.vector.tensor_scalar(out=gts[:], in0=fitbc[:], scalar1=fit[:, tb],
                                        scalar2=None, op0=ALU.is_gt, op1=ALU.add,
                                        accum_out=gtc[:])
                eq = self.pipe.tile([128, n], F32, tag="e2")
                nc.vector.tensor_scalar(out=eq[:], in0=fitbc[:], scalar1=fit[:, tb],
                                        scalar2=None, op0=ALU.is_equal)
                lt = self.pipe.tile([128, n], F32, tag="maskf")
                nc.vector.tensor_scalar(out=lt[:], in0=self.iotabc[:, :n],
                                        scalar1=self.iotac[:, tb],
                                        scalar2=None, op0=ALU.is_lt)
                scr2 = self.pipe.tile([128, n], F32, tag="e1")
                eqlt = self.sm.tile([128, 1], F32, tag="eqlt")
                nc.vector.tensor_tensor(out=scr2[:], in0=eq[:], in1=lt[:], op=ALU.mult)
                nc.vector.reduce_sum(out=eqlt[:], in_=scr2[:], axis=AX.X)
                nc.vector.tensor_tensor(out=rank[:, tb], in0=gtc[:], in1=eqlt[:],
                                        op=ALU.add)
            if final:
                wsel = self.sm.tile([128, nb, 1], BF16, tag="wsel")
                for tb in range(nb):
                    nc.vector.tensor_scalar(out=wsel[:, tb], in0=rank[:, tb],
                                            scalar1=float(k), scalar2=fit[:, tb],
                                            op0=ALU.is_lt, op1=ALU.mult)
                pgm = pRs.tile([1, hd], F32, tag="ps_gm")
                for tb in range(nb):
                    nc.tensor.matmul(pgm[:], wsel[:, tb], xnew[:, tb],
                                     start=(tb == 0), stop=(tb == nb - 1))
                gmrow = self.sm.tile([1, hd], F32, tag="gmrow")
                nc.scalar.mul(out=gmrow[:], in_=pgm[:], mul=float(1.0 / k))
                return gmrow
            Pt = self.work.tile([128, nb, k], BF16, tag="Pb")
            for tb in range(nb):
                nc.vector.tensor_scalar(out=Pt[:, tb], in0=self.iotabc[:, :k],
                                        scalar1=rank[:, tb], scalar2=fit[:, tb],
                                        op0=ALU.is_equal, op1=ALU.mult)
            return Pt, Pt, xnew

    # ------------------------------------------------------------------
    def run(self, out_d):
        nc, tc, scal = self.nc, self.tc, self.scal
        self.identity = self.const.tile([128, 128], F32, tag="identity")
        make_identity(nc, self.identity[:])
        self.onesr = self.const.tile([1, 128], F32, tag="onesr")
        nc.vector.memset(self.onesr[:], 1.0)
        self.ones65 = self.const.tile([DH + 1, 128], F32, tag="ones65")
        nc.vector.memset(self.ones65[:], 1.0)
        self.iotabc = self.load("iotabc", [128, N1])
        self.iotac = self.load("iotac", [128, NB1, 1], rearr="(b p) o -> p b o")

        xT0 = self.const.tile([F0, 1, N1], BF16, tag="xT0")
        nc.sync.dma_start(out=xT0[:, 0], in_=self.din["xT0"])
        mask1c = self.work.tile([128, NB1, N1], BF16, tag="bigshare")
        nc.sync.dma_start(out=mask1c[:],
                          in_=self.din["mask1c"].rearrange("(b p) t -> p b t", p=128))
        mask1 = self.load("mask1", [128, NB1, N1], BF16, pool=self.big,
                          rearr="(b p) t -> p b t")
        deg1c = self.load("deg1c", [128, NB1, 1], rearr="(b p) o -> p b o")

        def wload(name, kb, cols, p=128, dtype=F32):
            return self.load(name, [p, kb, cols], dtype, rearr="(b p) c -> p b c", p=p)

        wq1 = wload("wq1", 1, HD1, dtype=BF16); wk1 = wload("wk1", 1, HD1, dtype=BF16)
        wv1a = wload("wv1a", 1, H1 * (DH + 1), dtype=BF16); wsk1 = wload("wsk1", 1, HD1, dtype=BF16)
        bq1c = wload("bq1c", 3, 1); bk1c = wload("bk1c", 3, 1)
        bvsk1c = wload("bvsk1c", H1, 1, p=DH)
        attx1 = wload("attx1", H1, 1, p=DH); lew1 = wload("lew1", H1, 3, p=DH)
        w1bc = self.load("w1bc", [128, HD1])
        negleb3_1 = self.const.tile([128, 1], F32, tag="ngl1")
        nc.vector.memset(negleb3_1[:], float(-scal["le_b3_1"]))
        negleb3_2 = self.const.tile([128, 1], F32, tag="ngl2")
        nc.vector.memset(negleb3_2[:], float(-scal["le_b3_2"]))

        import os
        stop_after = os.environ.get("STOP_AFTER", "")

        def bail(src_ap):
            outc = self.sm.tile([4, 1], F32, tag="outc")
            nc.vector.tensor_copy(out=outc[:], in_=src_ap)
            nc.sync.dma_start(out=out_d, in_=outc[:])

        # ---------------- stage 1 ----------------
        x1T = self.conv(xT0, KBS1, N1, NB1, H1, [128, 128, 64],
                        wq1, wk1, wv1a, wsk1, bq1c, bk1c, bvsk1c, mask1c, "1")
        if stop_after == "conv1":
            return bail(x1T[:4, 0, 0:1])
        x1aug = self.transpose_to_aug(x1T, N1, NB1, H1, HD1, "bigshare")
        if stop_after == "aug1":
            return bail(x1aug[:4, 0, 0:1])
        Pt, Pb, xnew1 = self.pool(N1, NB1, H1, HD1, x1T, x1aug, mask1, deg1c,
                                  attx1, w1bc, lew1, scal["cst1"], scal["le_b1_1"],
                                  scal["le_b3_1"], negleb3_1, "1", final=False)

        if stop_after == "pool1":
            return bail(Pt[:4, 0, 0:1])
        # ---------------- stage-2 glue ----------------
        x2 = self.work.tile([128, NB2, HD1], BF16, tag="E")
        with tc.tile_pool(name="g2a", bufs=2, space="PSUM") as pX:
            for qb in range(NB2):
                px2 = pX.tile([128, HD1], F32, tag="X")
                for tb in range(NB1):
                    nc.tensor.matmul(px2[:], Pt[:, tb, qb * 128:(qb + 1) * 128],
                                     xnew1[:, tb],
                                     start=(tb == 0), stop=(tb == NB1 - 1))
                nc.scalar.copy(out=x2[:, qb], in_=px2[:])
        x2T = self.work.tile([128, 3, N2], BF16, tag="x2T")
        with tc.tile_pool(name="g2t", bufs=2, space="PSUM") as pT:
            identb = self.sm.tile([128, 128], BF16, tag="identb")
            nc.vector.tensor_copy(out=identb[:], in_=self.identity[:])
            for qb in range(NB2):
                for m, ob in enumerate(KBS2):
                    pt = pT.tile([128, 128], BF16, tag="T")
                    nc.tensor.transpose(pt[:ob, :],
                                        x2[:, qb, m * 128: m * 128 + ob],
                                        identb[:])
                    nc.scalar.copy(out=x2T[:ob, m, qb * 128:(qb + 1) * 128],
                                   in_=pt[:ob, :])
        mask1T = self.work.tile([128, NB1, N1], BF16, tag="bigshare")
        nc.sync.dma_start(out=mask1T[:],
                          in_=self.din["mask1T"].rearrange("(b p) t -> p b t", p=128))
        Sb = self.work.tile([128, NB1, N2], BF16, tag="kT")
        Tb = self.work.tile([128, NB1, N2], BF16, tag="vaug")
        mask2 = self.big.tile([128, NB2, N2], BF16, tag="mask2")
        mask2b = self.big.tile([128, NB2, N2], BF16, tag="mask2b")
        deg2c = self.sm.tile([128, NB2, 1], F32, tag="deg2c")
        with tc.tile_pool(name="g2s", bufs=2, space="PSUM") as pG:
            for dst, rhs in ((Sb, Pb), (Tb, Sb)):
                for sb in range(NB1):
                    pp = pG.tile([128, N2], F32, tag="G")
                    for tb in range(NB1):
                        nc.tensor.matmul(pp[:], mask1T[:, tb, sb * 128:(sb + 1) * 128],
                                         rhs[:, tb],
                                         start=(tb == 0), stop=(tb == NB1 - 1))
                    nc.scalar.copy(out=dst[:, sb], in_=pp[:])
            for pb in range(NB2):
                pc = pG.tile([128, N2], F32, tag="G")
                for sb in range(NB1):
                    nc.tensor.matmul(pc[:], Sb[:, sb, pb * 128:(pb + 1) * 128],
                                     Tb[:, sb], start=(sb == 0), stop=(sb == NB1 - 1))
                m2f = self.sm.tile([128, N2], F32, tag="m2f")
                nc.vector.tensor_scalar(out=m2f[:], in0=pc[:], scalar1=0.5,
                                        scalar2=None, op0=ALU.is_gt)
                ne = self.sm.tile([128, N2], F32, tag="m2ne")
                nc.vector.tensor_scalar(out=ne[:], in0=self.iotabc[:, :N2],
                                        scalar1=self.iotac[:, pb], scalar2=None,
                                        op0=ALU.not_equal)
                nc.vector.tensor_tensor(out=m2f[:], in0=m2f[:], in1=ne[:], op=ALU.mult)
                nc.vector.tensor_copy(out=mask2[:, pb], in_=m2f[:])
                nc.vector.tensor_scalar(out=ne[:], in0=self.iotabc[:, :N2],
                                        scalar1=self.iotac[:, pb], scalar2=None,
                                        op0=ALU.is_equal)
                nc.vector.tensor_tensor(out=m2f[:], in0=m2f[:], in1=ne[:], op=ALU.max)
                nc.vector.tensor_copy(out=mask2b[:, pb], in_=m2f[:])
            onecb = self.sm.tile([128, 1], BF16, tag="onecb")
            nc.vector.memset(onecb[:], 1.0)
            pdg = pG.tile([1, N2], F32, tag="Gd")
            for pb in range(NB2):
                nc.tensor.matmul(pdg[:], onecb[:], mask2b[:, pb],
                                 start=(pb == 0), stop=(pb == NB2 - 1))
            degrow = self.sm.tile([1, N2], F32, tag="degrow")
            nc.scalar.copy(out=degrow[:], in_=pdg[:])
            for qb in range(NB2):
                ptd = pG.tile([128, 1], F32, tag="Gt")
                nc.tensor.transpose(ptd[:, :], degrow[:, qb * 128:(qb + 1) * 128],
                                    self.identity[:1, :1])
                nc.scalar.copy(out=deg2c[:, qb], in_=ptd[:])

        if stop_after == "glue":
            return bail(deg2c[:4, 0])
        # ---------------- stage 2 ----------------
        wq2 = wload("wq2", 3, HD2, dtype=BF16); wk2 = wload("wk2", 3, HD2, dtype=BF16)
        wv2a = wload("wv2a", 3, H2 * (DH + 1), dtype=BF16); wsk2 = wload("wsk2", 3, HD2, dtype=BF16)
        bq2c = wload("bq2c", 2, 1); bk2c = wload("bk2c", 2, 1)
        bvsk2c = wload("bvsk2c", H2, 1, p=DH)
        attx2 = wload("attx2", H2, 1, p=DH); lew2 = wload("lew2", H2, 3, p=DH)
        w2bc = self.load("w2bc", [128, HD2])

        x3T = self.conv(x2T, KBS2, N2, NB2, H2, [128, 64],
                        wq2, wk2, wv2a, wsk2, bq2c, bk2c, bvsk2c, mask2, "2")
        if stop_after == "conv2":
            return bail(x3T[:4, 0, 0:1])
        x3aug = self.transpose_to_aug(x3T, N2, NB2, H2, HD2, "bigshare")
        gmrow = self.pool(N2, NB2, H2, HD2, x3T, x3aug, mask2b, deg2c,
                          attx2, w2bc, lew2, scal["cst2"], scal["le_b1_2"],
                          scal["le_b3_2"], negleb3_2, "2", final=True)

        if stop_after == "pool2":
            outc = self.sm.tile([4, 1], F32, tag="outc")
            nc.vector.memset(outc[:], 0.0)
            nc.vector.tensor_copy(out=outc[0:1, :], in_=gmrow[0:1, 0:1])
            nc.sync.dma_start(out=out_d, in_=outc[:])
            return
        # ---------------- MLP ----------------
        mw1 = wload("mw1", 2, HD1); mw2 = wload("mw2", 3, HD1); mw3 = wload("mw3", 3, 4)
        mb1c = wload("mb1c", 3, 1); mb2c = wload("mb2c", 3, 1)
        mb3c = self.load("mb3c", [4, 1])
        obs3 = [128, 128, 64]
        with tc.tile_pool(name="mlpp", bufs=2, space="PSUM") as pM:
            merge = self.sm.tile([128, 2, 1], F32, tag="merge")
            pm0 = pM.tile([128, 1], F32, tag="Mt")
            nc.tensor.transpose(pm0[:, :], gmrow[:, :128], self.identity[:1, :1])
            nc.scalar.copy(out=merge[:, 0], in_=pm0[:])
            pm1 = pM.tile([128, 1], F32, tag="Mt")
            nc.tensor.transpose(pm1[:64, :], gmrow[:, 128:192], self.identity[:1, :1])
            nc.scalar.copy(out=merge[:64, 1], in_=pm1[:64, :])
            nc.sync.dma_start(out=merge[64:69, 1], in_=self.din["evcd"])
            kbs1 = [128, 69]
            h1 = self.sm.tile([128, 3, 1], F32, tag="h1col")
            for m in range(3):
                ph = pM.tile([128, 1], F32, tag="Mm")
                for kb in range(2):
                    nc.tensor.matmul(ph[: obs3[m], :],
                                     _r(mw1[: kbs1[kb], kb, m * 128: m * 128 + obs3[m]]),
                                     _r(merge[: kbs1[kb], kb]),
                                     start=(kb == 0), stop=(kb == 1))
                nc.scalar.activation(out=h1[: obs3[m], m], in_=ph[: obs3[m], :],
                                     func=AF.Relu, bias=mb1c[: obs3[m], m])
            h2 = self.sm.tile([128, 3, 1], F32, tag="h2col")
            for m in range(3):
                ph = pM.tile([128, 1], F32, tag="Mm")
                for kb in range(3):
                    nc.tensor.matmul(ph[: obs3[m], :],
                                     _r(mw2[: obs3[kb], kb, m * 128: m * 128 + obs3[m]]),
                                     _r(h1[: obs3[kb], kb]),
                                     start=(kb == 0), stop=(kb == 2))
                nc.scalar.activation(out=h2[: obs3[m], m], in_=ph[: obs3[m], :],
                                     func=AF.Relu, bias=mb2c[: obs3[m], m])
            po = pM.tile([128, 1], F32, tag="Mo")
            for kb in range(3):
                nc.tensor.matmul(po[:4, 0:1], _r(mw3[: obs3[kb], kb, :]),
                                 _r(h2[: obs3[kb], kb]),
                                 start=(kb == 0), stop=(kb == 2))
            outc = self.sm.tile([4, 1], F32, tag="outc")
            nc.vector.tensor_tensor(out=outc[:], in0=po[:4, 0:1], in1=mb3c[:],
                                    op=ALU.add)
            nc.sync.dma_start(out=out_d, in_=outc[:])


# ======================================================================
# host side
# ======================================================================

_CACHE = {}


def _pad_rows(a, rows):
    out = np.zeros((rows, a.shape[1]), np.float32)
    out[: a.shape[0]] = a
    return out


def _prep_shared(inputs):
    tc1, tc2 = inputs["tc1"], inputs["tc2"]
    p1, p2 = inputs["pool1"], inputs["pool2"]
    mlp = inputs["mlp"]
    f = lambda a: np.asarray(a, np.float32)

    def vaug_pack(Wv, heads):
        fin = Wv.shape[0]
        out = np.zeros((fin, heads * (DH + 1)), np.float32)
        for h in range(heads):
            out[:, h * (DH + 1): h * (DH + 1) + DH] = Wv[:, h * DH:(h + 1) * DH]
        return out

    d = {}
    d["wq1"] = _pad_rows(f(tc1["Wq"]), 128)
    d["wk1"] = _pad_rows(f(tc1["Wk"]), 128)
    d["wv1a"] = _pad_rows(vaug_pack(f(tc1["Wv"]), H1), 128)
    d["wsk1"] = _pad_rows(f(tc1["Wskip"]), 128)
    d["bq1c"] = _pad_rows(f(tc1["bq"])[:, None], 3 * 128)
    d["bk1c"] = _pad_rows(f(tc1["bk"])[:, None], 3 * 128)
    d["bvsk1c"] = (f(tc1["bv"]) + f(tc1["bskip"]))[:, None].copy()
    d["attx1"] = f(p1["att_x"])[:, None].copy()
    w1 = f(p1["Wlin"]) @ f(p1["att_q"])
    d["w1bc"] = np.tile(w1[None, :], (128, 1)).astype(np.float32)
    d["lew1"] = np.stack([f(p1["le_W1"])[:, 0], f(p1["le_W2"])[:, 0],
                          f(p1["le_W3"])[:, 0]], axis=1).astype(np.float32)
    d["wq2"] = _pad_rows(f(tc2["Wq"]), 3 * 128)
    d["wk2"] = _pad_rows(f(tc2["Wk"]), 3 * 128)
    d["wv2a"] = _pad_rows(vaug_pack(f(tc2["Wv"]), H2), 3 * 128)
    d["wsk2"] = _pad_rows(f(tc2["Wskip"]), 3 * 128)
    d["bq2c"] = _pad_rows(f(tc2["bq"])[:, None], 2 * 128)
    d["bk2c"] = _pad_rows(f(tc2["bk"])[:, None], 2 * 128)
    d["bvsk2c"] = (f(tc2["bv"]) + f(tc2["bskip"]))[:, None].copy()
    d["attx2"] = f(p2["att_x"])[:, None].copy()
    w2 = f(p2["Wlin"]) @ f(p2["att_q"])
    d["w2bc"] = np.tile(w2[None, :], (128, 1)).astype(np.float32)
    d["lew2"] = np.stack([f(p2["le_W1"])[:, 0], f(p2["le_W2"])[:, 0],
                          f(p2["le_W3"])[:, 0]], axis=1).astype(np.float32)
    d["mw1"] = _pad_rows(f(mlp["W1"]), 2 * 128)
    d["mw2"] = _pad_rows(f(mlp["W2"]), 3 * 128)
    d["mw3"] = _pad_rows(f(mlp["W3"]), 3 * 128)
    d["mb1c"] = _pad_rows(f(mlp["b1"])[:, None], 3 * 128)
    d["mb2c"] = _pad_rows(f(mlp["b2"])[:, None], 3 * 128)
    d["mb3c"] = f(mlp["b3"])[:, None].copy()
    d["iotabc"] = np.tile(np.arange(N1, dtype=np.float32)[None, :], (128, 1))
    d["iotac"] = np.arange(N1, dtype=np.float32)[:, None]
    scal = {
        "cst1": float(f(p1["blin"]) @ f(p1["att_q"]) + f(p1["att_b"])),
        "cst2": float(f(p2["blin"]) @ f(p2["att_q"]) + f(p2["att_b"])),
        "le_b1_1": float(f(p1["le_b1"])[0]), "le_b3_1": float(f(p1["le_b3"])[0]),
        "le_b1_2": float(f(p2["le_b1"])[0]), "le_b3_2": float(f(p2["le_b3"])[0]),
    }
    return scal, d


def make_in_maps(inputs):
    import ml_dtypes
    BFH = ml_dtypes.bfloat16
    nodes = np.asarray(inputs["nodes"], np.float32)
    ei = np.asarray(inputs["edge_index"])
    ev = np.asarray(inputs["exp_value"], np.float32)
    cd = np.asarray(inputs["circuit_depth"], np.float32)
    scal, shared = _prep_shared(inputs)
    for w in ["wq1", "wk1", "wv1a", "wsk1", "wq2", "wk2", "wv2a", "wsk2"]:
        shared[w] = shared[w].astype(BFH)
    src, dst = ei[0], ei[1]
    gid = src // N1
    in_maps = []
    for b in range(B):
        m = gid == b
        A = np.zeros((N1, N1), bool)
        A[src[m] % N1, dst[m] % N1] = True
        mask1 = A.copy()
        np.fill_diagonal(mask1, True)
        xg = nodes[b * N1:(b + 1) * N1]
        im = dict(shared)
        im["xT0"] = np.ascontiguousarray(xg.T).astype(ml_dtypes.bfloat16)
        im["mask1c"] = A.astype(ml_dtypes.bfloat16)
        im["mask1"] = mask1.astype(ml_dtypes.bfloat16)
        im["mask1T"] = np.ascontiguousarray(mask1.T).astype(ml_dtypes.bfloat16)
        im["deg1c"] = mask1.sum(axis=0, dtype=np.float32)[:, None]
        im["evcd"] = np.concatenate([ev[b, 0], cd[b]])[:, None].astype(np.float32)
        in_maps.append(im)
    return scal, in_maps


def kernel(**inputs):
    scal, in_maps = make_in_maps(inputs)
    key = tuple(sorted(scal.items()))
    if key not in _CACHE:
        _CACHE[key] = build_program(scal)
    nc = _CACHE[key]
    res = run_bass_kernel_spmd(nc, in_maps, list(range(B)))
    out = np.stack([res.results[i]["out"][:, 0] for i in range(B)])
    return out.astype(np.float32)


# revision 21
# speedup vs baseline: 1.2398x; 1.0101x over previous
"""Trainium2 Bass kernel for nn_ExpValCircuitGraphModel (GNN message passing).

Sharding: data-parallel — one graph per NeuronCore (B=8 graphs on 8 cores).
Host does graph-format conversion only (dense 0/1 masks from edge_index,
parameter repacking); all model compute runs on-device.

Device algorithm (validated against the jax reference on host, rel err 6e-5):
- TransformerConv: scores^T on PE; softmax without max-shift; q/k/v/e bf16;
  e consumed per source block by the attention matmul, which also accumulates
  the softmax denominator via an appended ones-column on v.
- ASAP masked-max via mask-matmul log-sum-exp (per-feature shift, p=20);
  output measured insensitive to masked-max error up to +-0.3.
- exp(leaky_relu(y)) == max(exp(y), exp(0.2 y)).
- fitness sigmoid as 1/(1+exp(-x)) for exact fp32 saturation; top-k via stable
  rank (ties broken by index like jax.lax.top_k); selection as one-hot P.
- A2 = S_sel^T A S_sel needed only as boolean -> bf16 0/1 count matmuls.
- global_mean_pool of the selected half as a fitness-weighted matmul.

Conv outputs live in a [64, heads, n] transposed layout (partition rows 0:64)
so every attention/normalize op is partition-aligned.
"""
import numpy as np

import concourse.bass as bass
import concourse.tile as tile
from concourse import bacc, mybir
from concourse.bass_utils import run_bass_kernel_spmd
from concourse.masks import make_identity

F32, BF16, F32R = mybir.dt.float32, mybir.dt.bfloat16, mybir.dt.float32r
AF = mybir.ActivationFunctionType
ALU = mybir.AluOpType
AX = mybir.AxisListType

B, N1, F0 = 8, 1024, 32
H1, H2 = 5, 3
HD1, HD2 = 320, 192
DH = 64
N2 = 512
PEXP = 20.0
EPS_DEN = 1e-30
NB1, NB2 = 8, 4
KBS1, KBS2 = [F0], [128, 128, 64]     # conv input feature blocks


def _r(ap):
    # fp32r needs producer-side rounding (walrus invariant); plain fp32 for now
    return ap


def build_program(scal):
    nc = bacc.Bacc("TRN2", target_bir_lowering=False, debug=False, num_devices=8)
    din = {}

    def inp(name, shape, dtype=F32):
        din[name] = nc.dram_tensor(name, shape, dtype, kind="ExternalInput").ap()

    inp("xT0", [F0, N1], BF16)
    inp("mask1c", [N1, N1], BF16)
    inp("mask1", [N1, N1], BF16)
    inp("mask1T", [N1, N1], BF16)
    inp("deg1c", [N1, 1]); inp("evcd", [5, 1])
    inp("wq1", [128, HD1], BF16); inp("wk1", [128, HD1], BF16)
    inp("wv1a", [128, H1 * (DH + 1)], BF16); inp("wsk1", [128, HD1], BF16)
    inp("bq1c", [3 * 128, 1]); inp("bk1c", [3 * 128, 1])
    inp("bvsk1c", [H1 * DH, 1])
    inp("attx1", [H1 * DH, 1]); inp("w1bc", [128, HD1]); inp("lew1", [H1 * DH, 3])
    inp("wq2", [3 * 128, HD2], BF16); inp("wk2", [3 * 128, HD2], BF16)
    inp("wv2a", [3 * 128, H2 * (DH + 1)], BF16); inp("wsk2", [3 * 128, HD2], BF16)
    inp("bq2c", [2 * 128, 1]); inp("bk2c", [2 * 128, 1])
    inp("bvsk2c", [H2 * DH, 1])
    inp("attx2", [H2 * DH, 1]); inp("w2bc", [128, HD2]); inp("lew2", [H2 * DH, 3])
    inp("mw1", [2 * 128, HD1]); inp("mw2", [3 * 128, HD1]); inp("mw3", [3 * 128, 4])
    inp("mb1c", [3 * 128, 1]); inp("mb2c", [3 * 128, 1]); inp("mb3c", [4, 1])
    inp("iotabc", [128, N1]); inp("iotac", [N1, 1])
    out_d = nc.dram_tensor("out", [4, 1], F32, kind="ExternalOutput").ap()

    with tile.TileContext(nc) as tc:
        from contextlib import ExitStack
        with ExitStack() as ctx:
            _Prog(ctx, tc, nc, din, scal).run(out_d)
    nc.compile()
    return nc


class _Prog:
    def __init__(self, ctx, tc, nc, din, scal):
        self.ctx, self.tc, self.nc, self.din, self.scal = ctx, tc, nc, din, scal
        self.const = ctx.enter_context(tc.tile_pool(name="const", bufs=1))
        self.big = ctx.enter_context(tc.tile_pool(name="big", bufs=1))
        self.work = ctx.enter_context(tc.tile_pool(name="work", bufs=1))
        self.sm = ctx.enter_context(tc.tile_pool(name="sm", bufs=1))
        self.pipe = ctx.enter_context(tc.tile_pool(name="pipe", bufs=2))

    def load(self, name, shape, dtype=F32, pool=None, rearr=None, tag=None, p=128):
        pool = pool or self.const
        t = pool.tile(shape, dtype, tag=tag or name)
        src = self.din[name]
        if rearr is not None:
            src = src.rearrange(rearr, p=p)
        self.nc.sync.dma_start(out=t[:], in_=src)
        return t

    def bcast(self, pool_ps, row_ap, width, tag):
        nc = self.nc
        pb = pool_ps.tile([128, width], F32, tag="ps_bc")
        for t0 in range(0, width, 512):
            t1 = min(width, t0 + 512)
            nc.tensor.matmul(pb[:, t0:t1], self.onesr[:], row_ap[:, t0:t1],
                             start=True, stop=True)
        sb = self.sm.tile([128, width], F32, tag=tag)
        nc.scalar.copy(out=sb[:], in_=pb[:])
        return sb

    def cols2row(self, pool_ps, col3, blksizes, tag):
        nc = self.nc
        width = sum(blksizes)
        row = self.sm.tile([1, width], F32, tag=tag)
        o = 0
        for b, w in enumerate(blksizes):
            pt = pool_ps.tile([1, 128], F32, tag="ps_c2r")
            nc.tensor.transpose(pt[:, :w], col3[:w, b], self.identity[:w, :w])
            nc.scalar.copy(out=row[:, o: o + w], in_=pt[:, :w])
            o += w
        return row

    # ------------------------------------------------------------------
    def conv(self, xT, kbs, n, nb, heads, qkobs, wq, wk, wva, wsk,
             bqc, bkc, bvskc, mask_bf, sfx):
        """xT [<=128, KB, n] fp32 input (transposed). Returns xoT [64, heads, n]
        fp32 in work tag 'xoT': per-head feature rows at partitions 0:64."""
        nc, tc = self.nc, self.tc
        KB = len(kbs)
        nsl = [slice(t0, min(n, t0 + 512)) for t0 in range(0, n, 512)]
        qT = self.work.tile([128, len(qkobs), n], BF16, tag="qT")
        kT = self.work.tile([128, len(qkobs), n], BF16, tag="kT")
        xoT = self.work.tile([DH, heads, n], F32, tag="xoT")
        vaug = self.work.tile([128, nb, heads * (DH + 1)], BF16, tag="vaug")

        with tc.tile_pool(name="cvA" + sfx, bufs=2, space="PSUM") as pA, \
             tc.tile_pool(name="cvB" + sfx, bufs=2, space="PSUM") as pB:
            # q/k projections -> bf16 [128, OB, n]
            for w, dst, bias in ((wq, qT, bqc), (wk, kT, bkc)):
                for m, ob in enumerate(qkobs):
                    pm = pA.tile([128, n], F32, tag="A")
                    for sl in nsl:
                        for kb in range(KB):
                            nc.tensor.matmul(pm[:ob, sl],
                                             _r(w[: kbs[kb], kb, m * 128: m * 128 + ob]),
                                             _r(xT[: kbs[kb], kb, sl]),
                                             start=(kb == 0), stop=(kb == KB - 1))
                    nc.vector.tensor_scalar_add(out=dst[:ob, m], in0=pm[:ob],
                                                scalar1=bias[:ob, m])
            # skip projection -> xoT per head block [64, h, n]
            for h in range(heads):
                pm = pA.tile([128, n], F32, tag="A")
                for sl in nsl:
                    for kb in range(KB):
                        nc.tensor.matmul(pm[:DH, sl],
                                         _r(wsk[: kbs[kb], kb, h * DH:(h + 1) * DH]),
                                         _r(xT[: kbs[kb], kb, sl]),
                                         start=(kb == 0), stop=(kb == KB - 1))
                nc.scalar.copy(out=xoT[:, h], in_=pm[:DH])
            # v augmented
            for sb in range(nb):
                pv = pA.tile([128, n], F32, tag="A")
                w_ = heads * (DH + 1)
                for kb in range(KB):
                    nc.tensor.matmul(pv[:, :w_],
                                     _r(xT[: kbs[kb], kb, sb * 128:(sb + 1) * 128]),
                                     _r(wva[: kbs[kb], kb]),
                                     start=(kb == 0), stop=(kb == KB - 1))
                nc.scalar.copy(out=vaug[:, sb], in_=pv[:, :w_])
            v4 = vaug[:].rearrange("p b (h x) -> p b h x", h=heads)
            nc.gpsimd.memset(v4[:, :, :, DH: DH + 1], 1.0)

            isq = float(1.0 / np.sqrt(DH))
            # all heads' attention first (PE stays dense); raw num/den to SBUF
            paS = self.work.tile([DH + 1, heads, n], BF16, tag="paS")
            for h in range(heads):
                mt, mo = divmod(h * DH, 128)
                pa = pB.tile([DH + 1, n], F32, tag="B")
                for sb in range(nb):
                    psc = pA.tile([128, n], F32, tag="A")
                    for sl in nsl:
                        nc.tensor.matmul(psc[:, sl],
                                         kT[mo: mo + DH, mt, sb * 128:(sb + 1) * 128],
                                         qT[mo: mo + DH, mt, sl],
                                         start=True, stop=True)
                    eb = self.pipe.tile([128, n], BF16, tag="eblk")
                    nc.scalar.activation(out=eb[:], in_=psc[:], func=AF.Exp, scale=isq)
                    nc.vector.tensor_tensor(out=eb[:], in0=eb[:], in1=mask_bf[:, sb],
                                            op=ALU.mult)
                    for sl in nsl:
                        nc.tensor.matmul(pa[:, sl], v4[:, sb, h, :], eb[:, sl],
                                         start=(sb == 0), stop=(sb == nb - 1))
                nc.scalar.copy(out=paS[:, h], in_=pa[:])
            # deferred normalizes: DVE/ACT chains pipeline; PE only tiny bcasts
            for h in range(heads):
                inv65 = self.sm.tile([DH + 1, n], F32, tag="inv65")
                nc.vector.tensor_scalar_add(out=inv65[DH: DH + 1],
                                            in0=paS[DH: DH + 1, h],
                                            scalar1=EPS_DEN)
                nc.vector.reciprocal(out=inv65[DH: DH + 1], in_=inv65[DH: DH + 1])
                pbc = pA.tile([DH, n], F32, tag="A", name="pbc")
                for sl in nsl:
                    nc.tensor.matmul(pbc[:, sl], self.ones65[DH: DH + 1, :DH],
                                     inv65[DH: DH + 1, sl], start=True, stop=True)
                invbc = self.sm.tile([DH, n], F32, tag="invbc")
                nc.scalar.copy(out=invbc[:], in_=pbc[:])
                att = self.sm.tile([DH, n], F32, tag="attnrm")
                nc.vector.tensor_tensor(out=att[:], in0=paS[:DH, h], in1=invbc[:],
                                        op=ALU.mult)
                nc.vector.tensor_tensor(out=xoT[:, h], in0=xoT[:, h], in1=att[:],
                                        op=ALU.add)
                nc.vector.tensor_scalar_add(out=xoT[:, h], in0=xoT[:, h],
                                            scalar1=bvskc[:, h])
        return xoT

    def transpose_to_aug(self, xT, n, nb, heads, hd, tag):
        nc, tc = self.nc, self.tc
        xaug = self.work.tile([128, nb, hd + 4], BF16, tag=tag)
        with tc.tile_pool(name="trp" + tag + str(n), bufs=2, space="PSUM") as pT:
            for sb in range(nb):
                for h in range(heads):
                    pt = pT.tile([128, DH], F32, tag="T")
                    nc.tensor.transpose(pt[:, :],
                                        xT[:, h, sb * 128:(sb + 1) * 128],
                                        self.identity[:DH, :DH])
                    nc.scalar.copy(out=xaug[:, sb, h * DH:(h + 1) * DH], in_=pt[:])
        return xaug

    # ------------------------------------------------------------------
    def pool(self, n, nb, heads, hd, xT, xaug, mask_bf, degc, attxc, wbc, lewc,
             cst, leb1, leb3, negleb3c, sfx, final):
        """xT: [64, heads, n] conv output; xaug [128, nb, hd+4] normal layout."""
        import os
        nc, tc = self.nc, self.tc
        sub = os.environ.get("POOL_STOP", "") if sfx == "1" else ""
        k = n // 2
        with tc.tile_pool(name="plA" + sfx, bufs=2, space="PSUM") as pA, \
             tc.tile_pool(name="plB" + sfx, bufs=1, space="PSUM") as pB, \
             tc.tile_pool(name="plS" + sfx, bufs=2, space="PSUM") as pS:
            g = self.sm.tile([DH, heads, 1], F32, tag="gcol")
            for h in range(heads):
                nc.vector.reduce_max(out=g[:, h], in_=xT[:, h], axis=AX.X)
            if sub == "g0":
                return g, g, g
            grow = self.cols2row(pS, g, [DH] * heads, "grow")
            gbc = self.bcast(pB, grow[:], hd, "gbc")
            pgbc = self.sm.tile([128, hd], F32, tag="pgbc")
            nc.vector.tensor_scalar_mul(out=pgbc[:], in0=gbc[:], scalar1=PEXP)
            if sub == "g":
                return g, g, g
            E = self.work.tile([128, nb, hd], BF16, tag="E")
            for sb in range(nb):
                y = self.pipe.tile([128, hd], F32, tag="yE")
                nc.vector.tensor_tensor(out=y[:], in0=xaug[:, sb, :hd], in1=gbc[:],
                                        op=ALU.subtract)
                nc.scalar.activation(out=E[:, sb], in_=y[:], func=AF.Exp, scale=PEXP)
            if sub == "E":
                return g, g, g
            stcol = self.sm.tile([128, nb, 1], F32, tag="stcol")
            for tb in range(nb):
                pL = pA.tile([128, hd], F32, tag="A")
                for sb in range(nb):
                    nc.tensor.matmul(pL[:], mask_bf[:, sb, tb * 128:(tb + 1) * 128],
                                     E[:, sb], start=(sb == 0), stop=(sb == nb - 1))
                L = self.pipe.tile([128, hd], F32, tag="Llse")
                nc.scalar.activation(out=L[:], in_=pL[:], func=AF.Ln)
                nc.vector.tensor_tensor(out=L[:], in0=L[:], in1=pgbc[:], op=ALU.add)
                scr = self.pipe.tile([128, hd], F32, tag="scrL")
                nc.vector.tensor_tensor(out=scr[:], in0=L[:], in1=wbc[:, :hd],
                                        op=ALU.mult)
                nc.vector.tensor_scalar_mul(out=scr[:], in0=scr[:],
                                            scalar1=float(1.0 / PEXP))
                nc.vector.reduce_sum(out=stcol[:, tb], in_=scr[:], axis=AX.X)
            strow = self.cols2row(pS, stcol, [128] * nb, "strow")
            stbc = self.bcast(pB, strow[:], n, "stfbc")
            sscol = self.sm.tile([128, nb, 1], F32, tag="sscol")
            ss2col = self.sm.tile([128, nb, 1], F32, tag="ss2col")
            for sb in range(nb):
                pss = pS.tile([128, 4], F32, tag="ps_s4")
                for h in range(heads):
                    nc.tensor.matmul(pss[:, 0:1],
                                     _r(xT[:, h, sb * 128:(sb + 1) * 128]),
                                     _r(attxc[:, h]),
                                     start=(h == 0), stop=(h == heads - 1))
                nc.vector.tensor_scalar_add(out=sscol[:, sb], in0=pss[:, 0:1],
                                            scalar1=float(cst))
                nc.vector.tensor_scalar_mul(out=ss2col[:, sb], in0=sscol[:, sb],
                                            scalar1=0.2)
            for sb in range(nb):
                pxw = pS.tile([128, 4], F32, tag="ps_s4")
                for h in range(heads):
                    nc.tensor.matmul(pxw[:, 0:3],
                                     _r(xT[:, h, sb * 128:(sb + 1) * 128]),
                                     _r(lewc[:, h]),
                                     start=(h == 0), stop=(h == heads - 1))
                nc.scalar.copy(out=xaug[:, sb, hd: hd + 3], in_=pxw[:, 0:3])
            nc.gpsimd.memset(xaug[:, :, hd + 3: hd + 4], 1.0)
        if sub == "lse":
            return stcol[:, 0:1], stcol[:, 0:1], stcol[:, 0:1]
        xnew = self.work.tile([128, nb, hd], BF16, tag="xnew")
        dots = self.sm.tile([128, nb, 3], F32, tag="dots")
        acol = self.sm.tile([128, nb, 1], F32, tag="acol")
        with tc.tile_pool(name="plN" + sfx, bufs=1, space="PSUM") as pN:
            pxn = [pN.tile([128, hd + 4], F32, tag=f"xn{tb}", name=f"pxn{tb}") for tb in range(nb)]
            for sb in range(nb):
                e1 = self.pipe.tile([128, n], F32, tag="e1")
                nc.scalar.activation(out=e1[:], in_=stbc[:], func=AF.Exp,
                                     bias=sscol[:, sb], scale=1.0)
                e2 = self.pipe.tile([128, n], F32, tag="e2")
                nc.scalar.activation(out=e2[:], in_=stbc[:], func=AF.Exp,
                                     bias=ss2col[:, sb], scale=0.2)
                nc.vector.tensor_tensor(out=e1[:], in0=e1[:], in1=e2[:], op=ALU.max)
                eSb = self.pipe.tile([128, n], BF16, tag="eSb")
                nc.vector.tensor_tensor(out=eSb[:], in0=e1[:], in1=mask_bf[:, sb],
                                        op=ALU.mult)
                for tb in range(nb):
                    nc.tensor.matmul(pxn[tb][:], eSb[:, tb * 128:(tb + 1) * 128],
                                     xaug[:, sb],
                                     start=(sb == 0), stop=(sb == nb - 1))
            for tb in range(nb):
                inv = self.sm.tile([128, 1], F32, tag="invxn")
                nc.vector.tensor_scalar_add(out=inv[:], in0=pxn[tb][:, hd + 3: hd + 4],
                                            scalar1=EPS_DEN)
                nc.vector.reciprocal(out=inv[:], in_=inv[:])
                nc.vector.tensor_scalar_mul(out=xnew[:, tb], in0=pxn[tb][:, :hd],
                                            scalar1=inv[:])
                nc.vector.tensor_scalar_mul(out=dots[:, tb],
                                            in0=pxn[tb][:, hd: hd + 3], scalar1=inv[:])
                nc.vector.tensor_scalar_add(out=acol[:, tb], in0=dots[:, tb, 0:1],
                                            scalar1=float(leb1))
        if sub == "xnew":
            return acol, acol, acol
        fit = self.sm.tile([128, nb, 1], F32, tag="fit")
        with tc.tile_pool(name="plG" + sfx, bufs=1, space="PSUM") as pG:
            pag = [pG.tile([128, 1], F32, tag=f"ag{tb}", name=f"pag{tb}") for tb in range(nb)]
            for sb in range(nb):
                mf = self.pipe.tile([128, n], F32, tag="maskf", bufs=1)
                nc.vector.tensor_copy(out=mf[:], in_=mask_bf[:, sb])
                for tb in range(nb):
                    nc.tensor.matmul(pag[tb][:], mf[:, tb * 128:(tb + 1) * 128],
                                     acol[:, sb], start=(sb == 0), stop=(sb == nb - 1))
            for tb in range(nb):
                t2 = self.sm.tile([128, 1], F32, tag="ft2")
                nc.vector.tensor_tensor(out=t2[:], in0=degc[:, tb], in1=dots[:, tb, 1:2],
                                        op=ALU.mult)
                nc.vector.tensor_tensor(out=t2[:], in0=pag[tb][:], in1=t2[:],
                                        op=ALU.subtract)
                nc.vector.tensor_tensor(out=t2[:], in0=t2[:], in1=dots[:, tb, 2:3],
                                        op=ALU.add)
                nc.vector.tensor_scalar_max(out=t2[:], in0=t2[:],
                                            scalar1=float(-85.0 - leb3))
                nc.scalar.activation(out=t2[:], in_=t2[:], func=AF.Exp, scale=-1.0,
                                     bias=negleb3c[:])
                nc.vector.tensor_scalar_add(out=t2[:], in0=t2[:], scalar1=1.0)
                nc.vector.reciprocal(out=fit[:, tb], in_=t2[:])
        if sub == "fit":
            return fit, fit, fit
        with tc.tile_pool(name="plR" + sfx, bufs=1, space="PSUM") as pR, \
             tc.tile_pool(name="plRs" + sfx, bufs=2, space="PSUM") as pRs:
            fitrow = self.cols2row(pRs, fit, [128] * nb, "fitrow")
            fitbc = self.bcast(pR, fitrow[:], n, "stfbc")
            rank = self.sm.tile([128, nb, 1], F32, tag="rank")
            for tb in range(nb):
                gts = self.pipe.tile([128, n], F32, tag="e1")
                gtc = self.sm.tile([128, 1], F32, tag="gtc")
                nc.vector.tensor_scalar(out=gts[:], in0=fitbc[:], scalar1=fit[:, tb],
                                        scalar2=None, op0=ALU.is_gt, op1=ALU.add,
                                        accum_out=gtc[:])
                eq = self.pipe.tile([128, n], F32, tag="e2")
                nc.vector.tensor_scalar(out=eq[:], in0=fitbc[:], scalar1=fit[:, tb],
                                        scalar2=None, op0=ALU.is_equal)
                lt = self.pipe.tile([128, n], F32, tag="maskf", bufs=1)
                nc.vector.tensor_scalar(out=lt[:], in0=self.iotabc[:, :n],
                                        scalar1=self.iotac[:, tb],
                                        scalar2=None, op0=ALU.is_lt)
                scr2 = self.pipe.tile([128, n], F32, tag="e1")
                eqlt = self.sm.tile([128, 1], F32, tag="eqlt")
                nc.vector.tensor_tensor(out=scr2[:], in0=eq[:], in1=lt[:], op=ALU.mult)
                nc.vector.reduce_sum(out=eqlt[:], in_=scr2[:], axis=AX.X)
                nc.vector.tensor_tensor(out=rank[:, tb], in0=gtc[:], in1=eqlt[:],
                                        op=ALU.add)
            if final:
                wsel = self.sm.tile([128, nb, 1], BF16, tag="wsel")
                for tb in range(nb):
                    nc.vector.tensor_scalar(out=wsel[:, tb], in0=rank[:, tb],
                                            scalar1=float(k), scalar2=fit[:, tb],
                                            op0=ALU.is_lt, op1=ALU.mult)
                pgm = pRs.tile([1, hd], F32, tag="ps_gm")
                for tb in range(nb):
                    nc.tensor.matmul(pgm[:], wsel[:, tb], xnew[:, tb],
                                     start=(tb == 0), stop=(tb == nb - 1))
                gmrow = self.sm.tile([1, hd], F32, tag="gmrow")
                nc.scalar.mul(out=gmrow[:], in_=pgm[:], mul=float(1.0 / k))
                return gmrow
            Pt = self.work.tile([128, nb, k], BF16, tag="Pb")
            for tb in range(nb):
                nc.vector.tensor_scalar(out=Pt[:, tb], in0=self.iotabc[:, :k],
                                        scalar1=rank[:, tb], scalar2=fit[:, tb],
                                        op0=ALU.is_equal, op1=ALU.mult)
            return Pt, Pt, xnew

    # ------------------------------------------------------------------
    def run(self, out_d):
        nc, tc, scal = self.nc, self.tc, self.scal
        self.identity = self.const.tile([128, 128], F32, tag="identity")
        make_identity(nc, self.identity[:])
        self.onesr = self.const.tile([1, 128], F32, tag="onesr")
        nc.vector.memset(self.onesr[:], 1.0)
        self.ones65 = self.const.tile([DH + 1, 128], F32, tag="ones65")
        nc.vector.memset(self.ones65[:], 1.0)
        self.iotabc = self.load("iotabc", [128, N1])
        self.iotac = self.load("iotac", [128, NB1, 1], rearr="(b p) o -> p b o")

        xT0 = self.const.tile([F0, 1, N1], BF16, tag="xT0")
        nc.sync.dma_start(out=xT0[:, 0], in_=self.din["xT0"])
        mask1c = self.work.tile([128, NB1, N1], BF16, tag="bigshare")
        nc.sync.dma_start(out=mask1c[:],
                          in_=self.din["mask1c"].rearrange("(b p) t -> p b t", p=128))
        mask1 = self.load("mask1", [128, NB1, N1], BF16, pool=self.big,
                          rearr="(b p) t -> p b t")
        deg1c = self.load("deg1c", [128, NB1, 1], rearr="(b p) o -> p b o")

        def wload(name, kb, cols, p=128, dtype=F32):
            return self.load(name, [p, kb, cols], dtype, rearr="(b p) c -> p b c", p=p)

        wq1 = wload("wq1", 1, HD1, dtype=BF16); wk1 = wload("wk1", 1, HD1, dtype=BF16)
        wv1a = wload("wv1a", 1, H1 * (DH + 1), dtype=BF16); wsk1 = wload("wsk1", 1, HD1, dtype=BF16)
        bq1c = wload("bq1c", 3, 1); bk1c = wload("bk1c", 3, 1)
        bvsk1c = wload("bvsk1c", H1, 1, p=DH)
        attx1 = wload("attx1", H1, 1, p=DH); lew1 = wload("lew1", H1, 3, p=DH)
        w1bc = self.load("w1bc", [128, HD1])
        negleb3_1 = self.const.tile([128, 1], F32, tag="ngl1")
        nc.vector.memset(negleb3_1[:], float(-scal["le_b3_1"]))
        negleb3_2 = self.const.tile([128, 1], F32, tag="ngl2")
        nc.vector.memset(negleb3_2[:], float(-scal["le_b3_2"]))

        import os
        stop_after = os.environ.get("STOP_AFTER", "")

        def bail(src_ap):
            outc = self.sm.tile([4, 1], F32, tag="outc")
            nc.vector.tensor_copy(out=outc[:], in_=src_ap)
            nc.sync.dma_start(out=out_d, in_=outc[:])

        # ---------------- stage 1 ----------------
        x1T = self.conv(xT0, KBS1, N1, NB1, H1, [128, 128, 64],
                        wq1, wk1, wv1a, wsk1, bq1c, bk1c, bvsk1c, mask1c, "1")
        if stop_after == "conv1":
            return bail(x1T[:4, 0, 0:1])
        x1aug = self.transpose_to_aug(x1T, N1, NB1, H1, HD1, "bigshare")
        if stop_after == "aug1":
            return bail(x1aug[:4, 0, 0:1])
        Pt, Pb, xnew1 = self.pool(N1, NB1, H1, HD1, x1T, x1aug, mask1, deg1c,
                                  attx1, w1bc, lew1, scal["cst1"], scal["le_b1_1"],
                                  scal["le_b3_1"], negleb3_1, "1", final=False)

        if stop_after == "pool1":
            return bail(Pt[:4, 0, 0:1])
        # ---------------- stage-2 glue ----------------
        x2 = self.work.tile([128, NB2, HD1], BF16, tag="E")
        with tc.tile_pool(name="g2a", bufs=2, space="PSUM") as pX:
            for qb in range(NB2):
                px2 = pX.tile([128, HD1], F32, tag="X")
                for tb in range(NB1):
                    nc.tensor.matmul(px2[:], Pt[:, tb, qb * 128:(qb + 1) * 128],
                                     xnew1[:, tb],
                                     start=(tb == 0), stop=(tb == NB1 - 1))
                nc.scalar.copy(out=x2[:, qb], in_=px2[:])
        x2T = self.work.tile([128, 3, N2], BF16, tag="x2T")
        with tc.tile_pool(name="g2t", bufs=2, space="PSUM") as pT:
            identb = self.sm.tile([128, 128], BF16, tag="identb")
            nc.vector.tensor_copy(out=identb[:], in_=self.identity[:])
            for qb in range(NB2):
                for m, ob in enumerate(KBS2):
                    pt = pT.tile([128, 128], BF16, tag="T")
                    nc.tensor.transpose(pt[:ob, :],
                                        x2[:, qb, m * 128: m * 128 + ob],
                                        identb[:])
                    nc.scalar.copy(out=x2T[:ob, m, qb * 128:(qb + 1) * 128],
                                   in_=pt[:ob, :])
        mask1T = self.work.tile([128, NB1, N1], BF16, tag="bigshare")
        nc.sync.dma_start(out=mask1T[:],
                          in_=self.din["mask1T"].rearrange("(b p) t -> p b t", p=128))
        Sb = self.work.tile([128, NB1, N2], BF16, tag="kT")
        Tb = self.work.tile([128, NB1, N2], BF16, tag="vaug")
        mask2 = self.big.tile([128, NB2, N2], BF16, tag="mask2")
        mask2b = self.big.tile([128, NB2, N2], BF16, tag="mask2b")
        deg2c = self.sm.tile([128, NB2, 1], F32, tag="deg2c")
        with tc.tile_pool(name="g2s", bufs=2, space="PSUM") as pG:
            for dst, rhs in ((Sb, Pb), (Tb, Sb)):
                for sb in range(NB1):
                    pp = pG.tile([128, N2], F32, tag="G")
                    for tb in range(NB1):
                        nc.tensor.matmul(pp[:], mask1T[:, tb, sb * 128:(sb + 1) * 128],
                                         rhs[:, tb],
                                         start=(tb == 0), stop=(tb == NB1 - 1))
                    nc.scalar.copy(out=dst[:, sb], in_=pp[:])
            for pb in range(NB2):
                pc = pG.tile([128, N2], F32, tag="G")
                for sb in range(NB1):
                    nc.tensor.matmul(pc[:], Sb[:, sb, pb * 128:(pb + 1) * 128],
                                     Tb[:, sb], start=(sb == 0), stop=(sb == NB1 - 1))
                m2f = self.sm.tile([128, N2], F32, tag="m2f")
                nc.vector.tensor_scalar(out=m2f[:], in0=pc[:], scalar1=0.5,
                                        scalar2=None, op0=ALU.is_gt)
                ne = self.sm.tile([128, N2], F32, tag="m2ne")
                nc.vector.tensor_scalar(out=ne[:], in0=self.iotabc[:, :N2],
                                        scalar1=self.iotac[:, pb], scalar2=None,
                                        op0=ALU.not_equal)
                nc.vector.tensor_tensor(out=m2f[:], in0=m2f[:], in1=ne[:], op=ALU.mult)
                nc.vector.tensor_copy(out=mask2[:, pb], in_=m2f[:])
                nc.vector.tensor_scalar(out=ne[:], in0=self.iotabc[:, :N2],
                                        scalar1=self.iotac[:, pb], scalar2=None,
                                        op0=ALU.is_equal)
                nc.vector.tensor_tensor(out=m2f[:], in0=m2f[:], in1=ne[:], op=ALU.max)
                nc.vector.tensor_copy(out=mask2b[:, pb], in_=m2f[:])
            onecb = self.sm.tile([128, 1], BF16, tag="onecb")
            nc.vector.memset(onecb[:], 1.0)
            pdg = pG.tile([1, N2], F32, tag="Gd")
            for pb in range(NB2):
                nc.tensor.matmul(pdg[:], onecb[:], mask2b[:, pb],
                                 start=(pb == 0), stop=(pb == NB2 - 1))
            degrow = self.sm.tile([1, N2], F32, tag="degrow")
            nc.scalar.copy(out=degrow[:], in_=pdg[:])
            for qb in range(NB2):
                ptd = pG.tile([128, 1], F32, tag="Gt")
                nc.tensor.transpose(ptd[:, :], degrow[:, qb * 128:(qb + 1) * 128],
                                    self.identity[:1, :1])
                nc.scalar.copy(out=deg2c[:, qb], in_=ptd[:])

        if stop_after == "glue":
            return bail(deg2c[:4, 0])
        # ---------------- stage 2 ----------------
        wq2 = wload("wq2", 3, HD2, dtype=BF16); wk2 = wload("wk2", 3, HD2, dtype=BF16)
        wv2a = wload("wv2a", 3, H2 * (DH + 1), dtype=BF16); wsk2 = wload("wsk2", 3, HD2, dtype=BF16)
        bq2c = wload("bq2c", 2, 1); bk2c = wload("bk2c", 2, 1)
        bvsk2c = wload("bvsk2c", H2, 1, p=DH)
        attx2 = wload("attx2", H2, 1, p=DH); lew2 = wload("lew2", H2, 3, p=DH)
        w2bc = self.load("w2bc", [128, HD2])

        x3T = self.conv(x2T, KBS2, N2, NB2, H2, [128, 64],
                        wq2, wk2, wv2a, wsk2, bq2c, bk2c, bvsk2c, mask2, "2")
        if stop_after == "conv2":
            return bail(x3T[:4, 0, 0:1])
        x3aug = self.transpose_to_aug(x3T, N2, NB2, H2, HD2, "bigshare")
        gmrow = self.pool(N2, NB2, H2, HD2, x3T, x3aug, mask2b, deg2c,
                          attx2, w2bc, lew2, scal["cst2"], scal["le_b1_2"],
                          scal["le_b3_2"], negleb3_2, "2", final=True)

        if stop_after == "pool2":
            outc = self.sm.tile([4, 1], F32, tag="outc")
            nc.vector.memset(outc[:], 0.0)
            nc.vector.tensor_copy(out=outc[0:1, :], in_=gmrow[0:1, 0:1])
            nc.sync.dma_start(out=out_d, in_=outc[:])
            return
        # ---------------- MLP ----------------
        mw1 = wload("mw1", 2, HD1); mw2 = wload("mw2", 3, HD1); mw3 = wload("mw3", 3, 4)
        mb1c = wload("mb1c", 3, 1); mb2c = wload("mb2c", 3, 1)
        mb3c = self.load("mb3c", [4, 1])
        obs3 = [128, 128, 64]
        with tc.tile_pool(name="mlpp", bufs=2, space="PSUM") as pM:
            merge = self.sm.tile([128, 2, 1], F32, tag="merge")
            pm0 = pM.tile([128, 1], F32, tag="Mt")
            nc.tensor.transpose(pm0[:, :], gmrow[:, :128], self.identity[:1, :1])
            nc.scalar.copy(out=merge[:, 0], in_=pm0[:])
            pm1 = pM.tile([128, 1], F32, tag="Mt")
            nc.tensor.transpose(pm1[:64, :], gmrow[:, 128:192], self.identity[:1, :1])
            nc.scalar.copy(out=merge[:64, 1], in_=pm1[:64, :])
            nc.sync.dma_start(out=merge[64:69, 1], in_=self.din["evcd"])
            kbs1 = [128, 69]
            h1 = self.sm.tile([128, 3, 1], F32, tag="h1col")
            for m in range(3):
                ph = pM.tile([128, 1], F32, tag="Mm")
                for kb in range(2):
                    nc.tensor.matmul(ph[: obs3[m], :],
                                     _r(mw1[: kbs1[kb], kb, m * 128: m * 128 + obs3[m]]),
                                     _r(merge[: kbs1[kb], kb]),
                                     start=(kb == 0), stop=(kb == 1))
                nc.scalar.activation(out=h1[: obs3[m], m], in_=ph[: obs3[m], :],
                                     func=AF.Relu, bias=mb1c[: obs3[m], m])
            h2 = self.sm.tile([128, 3, 1], F32, tag="h2col")
            for m in range(3):
                ph = pM.tile([128, 1], F32, tag="Mm")
                for kb in range(3):
                    nc.tensor.matmul(ph[: obs3[m], :],
                                     _r(mw2[: obs3[kb], kb, m * 128: m * 128 + obs3[m]]),
                                     _r(h1[: obs3[kb], kb]),
                                     start=(kb == 0), stop=(kb == 2))
                nc.scalar.activation(out=h2[: obs3[m], m], in_=ph[: obs3[m], :],
                                     func=AF.Relu, bias=mb2c[: obs3[m], m])
            po = pM.tile([128, 1], F32, tag="Mo")
            for kb in range(3):
                nc.tensor.matmul(po[:4, 0:1], _r(mw3[: obs3[kb], kb, :]),
                                 _r(h2[: obs3[kb], kb]),
                                 start=(kb == 0), stop=(kb == 2))
            outc = self.sm.tile([4, 1], F32, tag="outc")
            nc.vector.tensor_tensor(out=outc[:], in0=po[:4, 0:1], in1=mb3c[:],
                                    op=ALU.add)
            nc.sync.dma_start(out=out_d, in_=outc[:])


# ======================================================================
# host side
# ======================================================================

_CACHE = {}


def _pad_rows(a, rows):
    out = np.zeros((rows, a.shape[1]), np.float32)
    out[: a.shape[0]] = a
    return out


def _prep_shared(inputs):
    tc1, tc2 = inputs["tc1"], inputs["tc2"]
    p1, p2 = inputs["pool1"], inputs["pool2"]
    mlp = inputs["mlp"]
    f = lambda a: np.asarray(a, np.float32)

    def vaug_pack(Wv, heads):
        fin = Wv.shape[0]
        out = np.zeros((fin, heads * (DH + 1)), np.float32)
        for h in range(heads):
            out[:, h * (DH + 1): h * (DH + 1) + DH] = Wv[:, h * DH:(h + 1) * DH]
        return out

    d = {}
    d["wq1"] = _pad_rows(f(tc1["Wq"]), 128)
    d["wk1"] = _pad_rows(f(tc1["Wk"]), 128)
    d["wv1a"] = _pad_rows(vaug_pack(f(tc1["Wv"]), H1), 128)
    d["wsk1"] = _pad_rows(f(tc1["Wskip"]), 128)
    d["bq1c"] = _pad_rows(f(tc1["bq"])[:, None], 3 * 128)
    d["bk1c"] = _pad_rows(f(tc1["bk"])[:, None], 3 * 128)
    d["bvsk1c"] = (f(tc1["bv"]) + f(tc1["bskip"]))[:, None].copy()
    d["attx1"] = f(p1["att_x"])[:, None].copy()
    w1 = f(p1["Wlin"]) @ f(p1["att_q"])
    d["w1bc"] = np.tile(w1[None, :], (128, 1)).astype(np.float32)
    d["lew1"] = np.stack([f(p1["le_W1"])[:, 0], f(p1["le_W2"])[:, 0],
                          f(p1["le_W3"])[:, 0]], axis=1).astype(np.float32)
    d["wq2"] = _pad_rows(f(tc2["Wq"]), 3 * 128)
    d["wk2"] = _pad_rows(f(tc2["Wk"]), 3 * 128)
    d["wv2a"] = _pad_rows(vaug_pack(f(tc2["Wv"]), H2), 3 * 128)
    d["wsk2"] = _pad_rows(f(tc2["Wskip"]), 3 * 128)
    d["bq2c"] = _pad_rows(f(tc2["bq"])[:, None], 2 * 128)
    d["bk2c"] = _pad_rows(f(tc2["bk"])[:, None], 2 * 128)
    d["bvsk2c"] = (f(tc2["bv"]) + f(tc2["bskip"]))[:, None].copy()
    d["attx2"] = f(p2["att_x"])[:, None].copy()
    w2 = f(p2["Wlin"]) @ f(p2["att_q"])
    d["w2bc"] = np.tile(w2[None, :], (128, 1)).astype(np.float32)
    d["lew2"] = np.stack([f(p2["le_W1"])[:, 0], f(p2["le_W2"])[:, 0],
                          f(p2["le_W3"])[:, 0]], axis=1).astype(np.float32)
    d["mw1"] = _pad_rows(f(mlp["W1"]), 2 * 128)
    d["mw2"] = _pad_rows(f(mlp["W2"]), 3 * 128)
    d["mw3"] = _pad_rows(f(mlp["W3"]), 3 * 128)
    d["mb1c"] = _pad_rows(f(mlp["b1"])[:, None], 3 * 128)
    d["mb2c"] = _pad_rows(f(mlp["b2"])[:, None], 3 * 128)
    d["mb3c"] = f(mlp["b3"])[:, None].copy()
    d["iotabc"] = np.tile(np.arange(N1, dtype=np.float32)[None, :], (128, 1))
    d["iotac"] = np.arange(N1, dtype=np.float32)[:, None]
    scal = {
        "cst1": float(f(p1["blin"]) @ f(p1["att_q"]) + f(p1["att_b"])),
        "cst2": float(f(p2["blin"]) @ f(p2["att_q"]) + f(p2["att_b"])),
        "le_b1_1": float(f(p1["le_b1"])[0]), "le_b3_1": float(f(p1["le_b3"])[0]),
        "le_b1_2": float(f(p2["le_b1"])[0]), "le_b3_2": float(f(p2["le_b3"])[0]),
    }
    return scal, d


def make_in_maps(inputs):
    import ml_dtypes
    BFH = ml_dtypes.bfloat16
    nodes = np.asarray(inputs["nodes"], np.float32)
    ei = np.asarray(inputs["edge_index"])
    ev = np.asarray(inputs["exp_value"], np.float32)
    cd = np.asarray(inputs["circuit_depth"], np.float32)
    scal, shared = _prep_shared(inputs)
    for w in ["wq1", "wk1", "wv1a", "wsk1", "wq2", "wk2", "wv2a", "wsk2"]:
        shared[w] = shared[w].astype(BFH)
    src, dst = ei[0], ei[1]
    gid = src // N1
    in_maps = []
    for b in range(B):
        m = gid == b
        A = np.zeros((N1, N1), bool)
        A[src[m] % N1, dst[m] % N1] = True
        mask1 = A.copy()
        np.fill_diagonal(mask1, True)
        xg = nodes[b * N1:(b + 1) * N1]
        im = dict(shared)
        im["xT0"] = np.ascontiguousarray(xg.T).astype(ml_dtypes.bfloat16)
        im["mask1c"] = A.astype(ml_dtypes.bfloat16)
        im["mask1"] = mask1.astype(ml_dtypes.bfloat16)
        im["mask1T"] = np.ascontiguousarray(mask1.T).astype(ml_dtypes.bfloat16)
        im["deg1c"] = mask1.sum(axis=0, dtype=np.float32)[:, None]
        im["evcd"] = np.concatenate([ev[b, 0], cd[b]])[:, None].astype(np.float32)
        in_maps.append(im)
    return scal, in_maps


def kernel(**inputs):
    scal, in_maps = make_in_maps(inputs)
    key = tuple(sorted(scal.items()))
    if key not in _CACHE:
        _CACHE[key] = build_program(scal)
    nc = _CACHE[key]
    res = run_bass_kernel_spmd(nc, in_maps, list(range(B)))
    out = np.stack([res.results[i]["out"][:, 0] for i in range(B)])
    return out.astype(np.float32)


# revision 23
# speedup vs baseline: 1.2511x; 1.0091x over previous
"""Trainium2 Bass kernel for nn_ExpValCircuitGraphModel (GNN message passing).

Sharding: data-parallel — one graph per NeuronCore (B=8 graphs on 8 cores).
Host does graph-format conversion only (dense 0/1 masks from edge_index,
parameter repacking); all model compute runs on-device.

Device algorithm (validated against the jax reference on host, rel err 6e-5):
- TransformerConv: scores^T on PE; softmax without max-shift; q/k/v/e bf16;
  e consumed per source block by the attention matmul, which also accumulates
  the softmax denominator via an appended ones-column on v.
- ASAP masked-max via mask-matmul log-sum-exp (per-feature shift, p=20);
  output measured insensitive to masked-max error up to +-0.3.
- exp(leaky_relu(y)) == max(exp(y), exp(0.2 y)).
- fitness sigmoid as 1/(1+exp(-x)) for exact fp32 saturation; top-k via stable
  rank (ties broken by index like jax.lax.top_k); selection as one-hot P.
- A2 = S_sel^T A S_sel needed only as boolean -> bf16 0/1 count matmuls.
- global_mean_pool of the selected half as a fitness-weighted matmul.

Conv outputs live in a [64, heads, n] transposed layout (partition rows 0:64)
so every attention/normalize op is partition-aligned.
"""
import numpy as np

import concourse.bass as bass
import concourse.tile as tile
from concourse import bacc, mybir
from concourse.bass_utils import run_bass_kernel_spmd
from concourse.masks import make_identity

F32, BF16, F32R = mybir.dt.float32, mybir.dt.bfloat16, mybir.dt.float32r
AF = mybir.ActivationFunctionType
ALU = mybir.AluOpType
AX = mybir.AxisListType

B, N1, F0 = 8, 1024, 32
H1, H2 = 5, 3
HD1, HD2 = 320, 192
DH = 64
N2 = 512
PEXP = 20.0
EPS_DEN = 1e-30
NB1, NB2 = 8, 4
KBS1, KBS2 = [F0], [128, 128, 64]     # conv input feature blocks


def _r(ap):
    # fp32r needs producer-side rounding (walrus invariant); plain fp32 for now
    return ap


def build_program(scal):
    nc = bacc.Bacc("TRN2", target_bir_lowering=False, debug=False, num_devices=8)
    din = {}

    def inp(name, shape, dtype=F32):
        din[name] = nc.dram_tensor(name, shape, dtype, kind="ExternalInput").ap()

    inp("xT0", [F0, N1], BF16)
    inp("mask1c", [N1, N1], BF16)
    inp("mask1", [N1, N1], BF16)
    inp("mask1T", [N1, N1], BF16)
    inp("deg1c", [N1, 1]); inp("evcd", [5, 1])
    inp("wq1", [128, HD1], BF16); inp("wk1", [128, HD1], BF16)
    inp("wv1a", [128, H1 * (DH + 1)], BF16); inp("wsk1", [128, HD1], BF16)
    inp("bq1c", [3 * 128, 1]); inp("bk1c", [3 * 128, 1])
    inp("bvsk1c", [H1 * DH, 1])
    inp("attx1", [H1 * DH, 1]); inp("w1bc", [128, HD1]); inp("lew1", [H1 * DH, 3])
    inp("wq2", [3 * 128, HD2], BF16); inp("wk2", [3 * 128, HD2], BF16)
    inp("wv2a", [3 * 128, H2 * (DH + 1)], BF16); inp("wsk2", [3 * 128, HD2], BF16)
    inp("bq2c", [2 * 128, 1]); inp("bk2c", [2 * 128, 1])
    inp("bvsk2c", [H2 * DH, 1])
    inp("attx2", [H2 * DH, 1]); inp("w2bc", [128, HD2]); inp("lew2", [H2 * DH, 3])
    inp("mw1", [2 * 128, HD1]); inp("mw2", [3 * 128, HD1]); inp("mw3", [3 * 128, 4])
    inp("mb1c", [3 * 128, 1]); inp("mb2c", [3 * 128, 1]); inp("mb3c", [4, 1])
    inp("iotabc", [128, N1]); inp("iotac", [N1, 1])
    out_d = nc.dram_tensor("out", [4, 1], F32, kind="ExternalOutput").ap()

    with tile.TileContext(nc) as tc:
        from contextlib import ExitStack
        with ExitStack() as ctx:
            _Prog(ctx, tc, nc, din, scal).run(out_d)
    nc.compile()
    return nc


class _Prog:
    def __init__(self, ctx, tc, nc, din, scal):
        self.ctx, self.tc, self.nc, self.din, self.scal = ctx, tc, nc, din, scal
        self.const = ctx.enter_context(tc.tile_pool(name="const", bufs=1))
        self.big = ctx.enter_context(tc.tile_pool(name="big", bufs=1))
        self.work = ctx.enter_context(tc.tile_pool(name="work", bufs=1))
        self.sm = ctx.enter_context(tc.tile_pool(name="sm", bufs=1))
        self.pipe = ctx.enter_context(tc.tile_pool(name="pipe", bufs=2))

    def load(self, name, shape, dtype=F32, pool=None, rearr=None, tag=None, p=128):
        pool = pool or self.const
        t = pool.tile(shape, dtype, tag=tag or name)
        src = self.din[name]
        if rearr is not None:
            src = src.rearrange(rearr, p=p)
        self.nc.sync.dma_start(out=t[:], in_=src)
        return t

    def bcast(self, pool_ps, row_ap, width, tag):
        nc = self.nc
        pb = pool_ps.tile([128, width], F32, tag="ps_bc")
        for t0 in range(0, width, 512):
            t1 = min(width, t0 + 512)
            nc.tensor.matmul(pb[:, t0:t1], self.onesr[:], row_ap[:, t0:t1],
                             start=True, stop=True)
        sb = self.sm.tile([128, width], F32, tag=tag)
        nc.scalar.copy(out=sb[:], in_=pb[:])
        return sb

    def cols2row(self, pool_ps, col3, blksizes, tag):
        nc = self.nc
        width = sum(blksizes)
        row = self.sm.tile([1, width], F32, tag=tag)
        o = 0
        for b, w in enumerate(blksizes):
            pt = pool_ps.tile([1, 128], F32, tag="ps_c2r")
            nc.tensor.transpose(pt[:, :w], col3[:w, b], self.identity[:w, :w])
            nc.scalar.copy(out=row[:, o: o + w], in_=pt[:, :w])
            o += w
        return row

    # ------------------------------------------------------------------
    def conv(self, xT, kbs, n, nb, heads, qkobs, wq, wk, wva, wsk,
             bqc, bkc, bvskc, mask_bf, sfx):
        """xT [<=128, KB, n] fp32 input (transposed). Returns xoT [64, heads, n]
        fp32 in work tag 'xoT': per-head feature rows at partitions 0:64."""
        nc, tc = self.nc, self.tc
        KB = len(kbs)
        nsl = [slice(t0, min(n, t0 + 512)) for t0 in range(0, n, 512)]
        qT = self.work.tile([128, len(qkobs), n], BF16, tag="qT")
        kT = self.work.tile([128, len(qkobs), n], BF16, tag="kT")
        xoT = self.work.tile([DH, heads, n], F32, tag="xoT")
        vaug = self.work.tile([128, nb, heads * (DH + 1)], BF16, tag="vaug")

        with tc.tile_pool(name="cvA" + sfx, bufs=2, space="PSUM") as pA, \
             tc.tile_pool(name="cvB" + sfx, bufs=2, space="PSUM") as pB:
            # q/k projections -> bf16 [128, OB, n]
            for w, dst, bias in ((wq, qT, bqc), (wk, kT, bkc)):
                for m, ob in enumerate(qkobs):
                    pm = pA.tile([128, n], F32, tag="A")
                    for sl in nsl:
                        for kb in range(KB):
                            nc.tensor.matmul(pm[:ob, sl],
                                             _r(w[: kbs[kb], kb, m * 128: m * 128 + ob]),
                                             _r(xT[: kbs[kb], kb, sl]),
                                             start=(kb == 0), stop=(kb == KB - 1))
                    nc.vector.tensor_scalar_add(out=dst[:ob, m], in0=pm[:ob],
                                                scalar1=bias[:ob, m])
            # skip projection -> xoT per head block [64, h, n]
            for h in range(heads):
                pm = pA.tile([128, n], F32, tag="A")
                for sl in nsl:
                    for kb in range(KB):
                        nc.tensor.matmul(pm[:DH, sl],
                                         _r(wsk[: kbs[kb], kb, h * DH:(h + 1) * DH]),
                                         _r(xT[: kbs[kb], kb, sl]),
                                         start=(kb == 0), stop=(kb == KB - 1))
                nc.scalar.copy(out=xoT[:, h], in_=pm[:DH])
            # v augmented
            for sb in range(nb):
                pv = pA.tile([128, n], F32, tag="A")
                w_ = heads * (DH + 1)
                for kb in range(KB):
                    nc.tensor.matmul(pv[:, :w_],
                                     _r(xT[: kbs[kb], kb, sb * 128:(sb + 1) * 128]),
                                     _r(wva[: kbs[kb], kb]),
                                     start=(kb == 0), stop=(kb == KB - 1))
                nc.scalar.copy(out=vaug[:, sb], in_=pv[:, :w_])
            v4 = vaug[:].rearrange("p b (h x) -> p b h x", h=heads)
            nc.gpsimd.memset(v4[:, :, :, DH: DH + 1], 1.0)

            isq = float(1.0 / np.sqrt(DH))
            # all heads' attention first (PE stays dense); raw num/den to SBUF
            paS = self.work.tile([DH + 1, heads, n], BF16, tag="paS")
            for h in range(heads):
                mt, mo = divmod(h * DH, 128)
                pa = pB.tile([DH + 1, n], F32, tag="B")
                for sb in range(nb):
                    psc = pA.tile([128, n], F32, tag="A")
                    for sl in nsl:
                        nc.tensor.matmul(psc[:, sl],
                                         kT[mo: mo + DH, mt, sb * 128:(sb + 1) * 128],
                                         qT[mo: mo + DH, mt, sl],
                                         start=True, stop=True)
                    eb = self.pipe.tile([128, n], BF16, tag="eblk")
                    nc.scalar.activation(out=eb[:], in_=psc[:], func=AF.Exp, scale=isq)
                    nc.vector.tensor_tensor(out=eb[:], in0=eb[:], in1=mask_bf[:, sb],
                                            op=ALU.mult)
                    for sl in nsl:
                        nc.tensor.matmul(pa[:, sl], v4[:, sb, h, :], eb[:, sl],
                                         start=(sb == 0), stop=(sb == nb - 1))
                nc.scalar.copy(out=paS[:, h], in_=pa[:])
            # deferred normalizes: DVE/ACT chains pipeline; PE only tiny bcasts
            for h in range(heads):
                inv65 = self.sm.tile([DH + 1, n], F32, tag="inv65")
                nc.vector.tensor_scalar_add(out=inv65[DH: DH + 1],
                                            in0=paS[DH: DH + 1, h],
                                            scalar1=EPS_DEN)
                nc.vector.reciprocal(out=inv65[DH: DH + 1], in_=inv65[DH: DH + 1])
                pbc = pA.tile([DH, n], F32, tag="A", name="pbc")
                for sl in nsl:
                    nc.tensor.matmul(pbc[:, sl], self.ones65[DH: DH + 1, :DH],
                                     inv65[DH: DH + 1, sl], start=True, stop=True)
                invbc = self.sm.tile([DH, n], F32, tag="invbc")
                nc.scalar.copy(out=invbc[:], in_=pbc[:])
                att = self.sm.tile([DH, n], F32, tag="attnrm")
                nc.vector.tensor_tensor(out=att[:], in0=paS[:DH, h], in1=invbc[:],
                                        op=ALU.mult)
                nc.vector.tensor_tensor(out=xoT[:, h], in0=xoT[:, h], in1=att[:],
                                        op=ALU.add)
                nc.vector.tensor_scalar_add(out=xoT[:, h], in0=xoT[:, h],
                                            scalar1=bvskc[:, h])
        return xoT

    def transpose_to_aug(self, xT, n, nb, heads, hd, tag):
        nc, tc = self.nc, self.tc
        xaug = self.work.tile([128, nb, hd + 4], BF16, tag=tag)
        with tc.tile_pool(name="trp" + tag + str(n), bufs=2, space="PSUM") as pT:
            for sb in range(nb):
                for h in range(heads):
                    pt = pT.tile([128, DH], F32, tag="T")
                    nc.tensor.transpose(pt[:, :],
                                        xT[:, h, sb * 128:(sb + 1) * 128],
                                        self.identity[:DH, :DH])
                    nc.scalar.copy(out=xaug[:, sb, h * DH:(h + 1) * DH], in_=pt[:])
        return xaug

    # ------------------------------------------------------------------
    def pool(self, n, nb, heads, hd, xT, xaug, mask_bf, degc, attxc, wbc, lewc,
             cst, leb1, leb3, negleb3c, sfx, final):
        """xT: [64, heads, n] conv output; xaug [128, nb, hd+4] normal layout."""
        import os
        nc, tc = self.nc, self.tc
        sub = os.environ.get("POOL_STOP", "") if sfx == "1" else ""
        k = n // 2
        with tc.tile_pool(name="plA" + sfx, bufs=2, space="PSUM") as pA, \
             tc.tile_pool(name="plB" + sfx, bufs=1, space="PSUM") as pB, \
             tc.tile_pool(name="plS" + sfx, bufs=2, space="PSUM") as pS:
            g = self.sm.tile([DH, heads, 1], F32, tag="gcol")
            for h in range(heads):
                nc.vector.reduce_max(out=g[:, h], in_=xT[:, h], axis=AX.X)
            if sub == "g0":
                return g, g, g
            grow = self.cols2row(pS, g, [DH] * heads, "grow")
            gbc = self.bcast(pB, grow[:], hd, "gbc")
            pgbc = self.sm.tile([128, hd], F32, tag="pgbc")
            nc.vector.tensor_scalar_mul(out=pgbc[:], in0=gbc[:], scalar1=PEXP)
            if sub == "g":
                return g, g, g
            E = self.work.tile([128, nb, hd], BF16, tag="E")
            for sb in range(nb):
                y = self.pipe.tile([128, hd], F32, tag="yE")
                nc.vector.tensor_tensor(out=y[:], in0=xaug[:, sb, :hd], in1=gbc[:],
                                        op=ALU.subtract)
                nc.scalar.activation(out=E[:, sb], in_=y[:], func=AF.Exp, scale=PEXP)
            if sub == "E":
                return g, g, g
            stcol = self.sm.tile([128, nb, 1], F32, tag="stcol")
            for tb in range(nb):
                pL = pA.tile([128, hd], F32, tag="A")
                for sb in range(nb):
                    nc.tensor.matmul(pL[:], mask_bf[:, sb, tb * 128:(tb + 1) * 128],
                                     E[:, sb], start=(sb == 0), stop=(sb == nb - 1))
                L = self.pipe.tile([128, hd], F32, tag="Llse")
                nc.scalar.activation(out=L[:], in_=pL[:], func=AF.Ln)
                nc.vector.tensor_tensor(out=L[:], in0=L[:], in1=pgbc[:], op=ALU.add)
                scr = self.pipe.tile([128, hd], F32, tag="scrL")
                nc.vector.tensor_tensor(out=scr[:], in0=L[:], in1=wbc[:, :hd],
                                        op=ALU.mult)
                nc.vector.tensor_scalar_mul(out=scr[:], in0=scr[:],
                                            scalar1=float(1.0 / PEXP))
                nc.vector.reduce_sum(out=stcol[:, tb], in_=scr[:], axis=AX.X)
            strow = self.cols2row(pS, stcol, [128] * nb, "strow")
            stbc = self.bcast(pB, strow[:], n, "stfbc")
            sscol = self.sm.tile([128, nb, 1], F32, tag="sscol")
            ss2col = self.sm.tile([128, nb, 1], F32, tag="ss2col")
            for sb in range(nb):
                pss = pS.tile([128, 4], F32, tag="ps_s4")
                for h in range(heads):
                    nc.tensor.matmul(pss[:, 0:1],
                                     _r(xT[:, h, sb * 128:(sb + 1) * 128]),
                                     _r(attxc[:, h]),
                                     start=(h == 0), stop=(h == heads - 1))
                nc.vector.tensor_scalar_add(out=sscol[:, sb], in0=pss[:, 0:1],
                                            scalar1=float(cst))
                nc.vector.tensor_scalar_mul(out=ss2col[:, sb], in0=sscol[:, sb],
                                            scalar1=0.2)
            for sb in range(nb):
                pxw = pS.tile([128, 4], F32, tag="ps_s4")
                for h in range(heads):
                    nc.tensor.matmul(pxw[:, 0:3],
                                     _r(xT[:, h, sb * 128:(sb + 1) * 128]),
                                     _r(lewc[:, h]),
                                     start=(h == 0), stop=(h == heads - 1))
                nc.scalar.copy(out=xaug[:, sb, hd: hd + 3], in_=pxw[:, 0:3])
            nc.gpsimd.memset(xaug[:, :, hd + 3: hd + 4], 1.0)
        if sub == "lse":
            return stcol[:, 0:1], stcol[:, 0:1], stcol[:, 0:1]
        xnew = self.work.tile([128, nb, hd], BF16, tag="xnew")
        dots = self.sm.tile([128, nb, 3], F32, tag="dots")
        acol = self.sm.tile([128, nb, 1], F32, tag="acol")
        with tc.tile_pool(name="plN" + sfx, bufs=1, space="PSUM") as pN:
            pxn = [pN.tile([128, hd + 4], F32, tag=f"xn{tb}", name=f"pxn{tb}") for tb in range(nb)]
            for sb in range(nb):
                e1 = self.pipe.tile([128, n], F32, tag="e1")
                nc.scalar.activation(out=e1[:], in_=stbc[:], func=AF.Exp,
                                     bias=sscol[:, sb], scale=1.0)
                e2 = self.pipe.tile([128, n], F32, tag="e2")
                nc.scalar.activation(out=e2[:], in_=stbc[:], func=AF.Exp,
                                     bias=ss2col[:, sb], scale=0.2)
                nc.vector.tensor_tensor(out=e1[:], in0=e1[:], in1=e2[:], op=ALU.max)
                eSb = self.pipe.tile([128, n], BF16, tag="eSb")
                nc.vector.tensor_tensor(out=eSb[:], in0=e1[:], in1=mask_bf[:, sb],
                                        op=ALU.mult)
                for tb in range(nb):
                    nc.tensor.matmul(pxn[tb][:], eSb[:, tb * 128:(tb + 1) * 128],
                                     xaug[:, sb],
                                     start=(sb == 0), stop=(sb == nb - 1))
            for tb in range(nb):
                inv = self.sm.tile([128, 1], F32, tag="invxn")
                nc.vector.tensor_scalar_add(out=inv[:], in0=pxn[tb][:, hd + 3: hd + 4],
                                            scalar1=EPS_DEN)
                nc.vector.reciprocal(out=inv[:], in_=inv[:])
                nc.vector.tensor_scalar_mul(out=xnew[:, tb], in0=pxn[tb][:, :hd],
                                            scalar1=inv[:])
                nc.vector.tensor_scalar_mul(out=dots[:, tb],
                                            in0=pxn[tb][:, hd: hd + 3], scalar1=inv[:])
                nc.vector.tensor_scalar_add(out=acol[:, tb], in0=dots[:, tb, 0:1],
                                            scalar1=float(leb1))
        if sub == "xnew":
            return acol, acol, acol
        fit = self.sm.tile([128, nb, 1], F32, tag="fit")
        with tc.tile_pool(name="plG" + sfx, bufs=1, space="PSUM") as pG:
            pag = [pG.tile([128, 1], F32, tag=f"ag{tb}", name=f"pag{tb}") for tb in range(nb)]
            for sb in range(nb):
                mf = self.pipe.tile([128, n], F32, tag="maskf", bufs=1)
                nc.scalar.copy(out=mf[:], in_=mask_bf[:, sb])
                for tb in range(nb):
                    nc.tensor.matmul(pag[tb][:], mf[:, tb * 128:(tb + 1) * 128],
                                     acol[:, sb], start=(sb == 0), stop=(sb == nb - 1))
            for tb in range(nb):
                t2 = self.sm.tile([128, 1], F32, tag="ft2")
                nc.vector.tensor_tensor(out=t2[:], in0=degc[:, tb], in1=dots[:, tb, 1:2],
                                        op=ALU.mult)
                nc.vector.tensor_tensor(out=t2[:], in0=pag[tb][:], in1=t2[:],
                                        op=ALU.subtract)
                nc.vector.tensor_tensor(out=t2[:], in0=t2[:], in1=dots[:, tb, 2:3],
                                        op=ALU.add)
                nc.vector.tensor_scalar_max(out=t2[:], in0=t2[:],
                                            scalar1=float(-85.0 - leb3))
                nc.scalar.activation(out=t2[:], in_=t2[:], func=AF.Exp, scale=-1.0,
                                     bias=negleb3c[:])
                nc.vector.tensor_scalar_add(out=t2[:], in0=t2[:], scalar1=1.0)
                nc.vector.reciprocal(out=fit[:, tb], in_=t2[:])
        if sub == "fit":
            return fit, fit, fit
        with tc.tile_pool(name="plR" + sfx, bufs=1, space="PSUM") as pR, \
             tc.tile_pool(name="plRs" + sfx, bufs=2, space="PSUM") as pRs:
            fitrow = self.cols2row(pRs, fit, [128] * nb, "fitrow")
            fitbc = self.bcast(pR, fitrow[:], n, "stfbc")
            rank = self.sm.tile([128, nb, 1], F32, tag="rank")
            for tb in range(nb):
                gts = self.pipe.tile([128, n], F32, tag="e1")
                gtc = self.sm.tile([128, 1], F32, tag="gtc")
                nc.vector.tensor_scalar(out=gts[:], in0=fitbc[:], scalar1=fit[:, tb],
                                        scalar2=None, op0=ALU.is_gt, op1=ALU.add,
                                        accum_out=gtc[:])
                eq = self.pipe.tile([128, n], F32, tag="e2")
                nc.vector.tensor_scalar(out=eq[:], in0=fitbc[:], scalar1=fit[:, tb],
                                        scalar2=None, op0=ALU.is_equal)
                lt = self.pipe.tile([128, n], F32, tag="maskf", bufs=1)
                nc.vector.tensor_scalar(out=lt[:], in0=self.iotabc[:, :n],
                                        scalar1=self.iotac[:, tb],
                                        scalar2=None, op0=ALU.is_lt)
                scr2 = self.pipe.tile([128, n], F32, tag="e1")
                eqlt = self.sm.tile([128, 1], F32, tag="eqlt")
                nc.vector.tensor_tensor(out=scr2[:], in0=eq[:], in1=lt[:], op=ALU.mult)
                nc.vector.reduce_sum(out=eqlt[:], in_=scr2[:], axis=AX.X)
                nc.vector.tensor_tensor(out=rank[:, tb], in0=gtc[:], in1=eqlt[:],
                                        op=ALU.add)
            if final:
                wsel = self.sm.tile([128, nb, 1], BF16, tag="wsel")
                for tb in range(nb):
                    nc.vector.tensor_scalar(out=wsel[:, tb], in0=rank[:, tb],
                                            scalar1=float(k), scalar2=fit[:, tb],
                                            op0=ALU.is_lt, op1=ALU.mult)
                pgm = pRs.tile([1, hd], F32, tag="ps_gm")
                for tb in range(nb):
                    nc.tensor.matmul(pgm[:], wsel[:, tb], xnew[:, tb],
                                     start=(tb == 0), stop=(tb == nb - 1))
                gmrow = self.sm.tile([1, hd], F32, tag="gmrow")
                nc.scalar.mul(out=gmrow[:], in_=pgm[:], mul=float(1.0 / k))
                return gmrow
            Pt = self.work.tile([128, nb, k], BF16, tag="Pb")
            for tb in range(nb):
                nc.vector.tensor_scalar(out=Pt[:, tb], in0=self.iotabc[:, :k],
                                        scalar1=rank[:, tb], scalar2=fit[:, tb],
                                        op0=ALU.is_equal, op1=ALU.mult)
            return Pt, Pt, xnew

    # ------------------------------------------------------------------
    def run(self, out_d):
        nc, tc, scal = self.nc, self.tc, self.scal
        self.identity = self.const.tile([128, 128], F32, tag="identity")
        make_identity(nc, self.identity[:])
        self.onesr = self.const.tile([1, 128], F32, tag="onesr")
        nc.vector.memset(self.onesr[:], 1.0)
        self.ones65 = self.const.tile([DH + 1, 128], F32, tag="ones65")
        nc.vector.memset(self.ones65[:], 1.0)
        self.iotabc = self.load("iotabc", [128, N1])
        self.iotac = self.load("iotac", [128, NB1, 1], rearr="(b p) o -> p b o")

        xT0 = self.const.tile([F0, 1, N1], BF16, tag="xT0")
        nc.sync.dma_start(out=xT0[:, 0], in_=self.din["xT0"])
        mask1c = self.work.tile([128, NB1, N1], BF16, tag="bigshare")
        nc.sync.dma_start(out=mask1c[:],
                          in_=self.din["mask1c"].rearrange("(b p) t -> p b t", p=128))
        mask1 = self.load("mask1", [128, NB1, N1], BF16, pool=self.big,
                          rearr="(b p) t -> p b t")
        deg1c = self.load("deg1c", [128, NB1, 1], rearr="(b p) o -> p b o")

        def wload(name, kb, cols, p=128, dtype=F32):
            return self.load(name, [p, kb, cols], dtype, rearr="(b p) c -> p b c", p=p)

        wq1 = wload("wq1", 1, HD1, dtype=BF16); wk1 = wload("wk1", 1, HD1, dtype=BF16)
        wv1a = wload("wv1a", 1, H1 * (DH + 1), dtype=BF16); wsk1 = wload("wsk1", 1, HD1, dtype=BF16)
        bq1c = wload("bq1c", 3, 1); bk1c = wload("bk1c", 3, 1)
        bvsk1c = wload("bvsk1c", H1, 1, p=DH)
        attx1 = wload("attx1", H1, 1, p=DH); lew1 = wload("lew1", H1, 3, p=DH)
        w1bc = self.load("w1bc", [128, HD1])
        negleb3_1 = self.const.tile([128, 1], F32, tag="ngl1")
        nc.vector.memset(negleb3_1[:], float(-scal["le_b3_1"]))
        negleb3_2 = self.const.tile([128, 1], F32, tag="ngl2")
        nc.vector.memset(negleb3_2[:], float(-scal["le_b3_2"]))

        import os
        stop_after = os.environ.get("STOP_AFTER", "")

        def bail(src_ap):
            outc = self.sm.tile([4, 1], F32, tag="outc")
            nc.vector.tensor_copy(out=outc[:], in_=src_ap)
            nc.sync.dma_start(out=out_d, in_=outc[:])

        # ---------------- stage 1 ----------------
        x1T = self.conv(xT0, KBS1, N1, NB1, H1, [128, 128, 64],
                        wq1, wk1, wv1a, wsk1, bq1c, bk1c, bvsk1c, mask1c, "1")
        if stop_after == "conv1":
            return bail(x1T[:4, 0, 0:1])
        x1aug = self.transpose_to_aug(x1T, N1, NB1, H1, HD1, "bigshare")
        if stop_after == "aug1":
            return bail(x1aug[:4, 0, 0:1])
        Pt, Pb, xnew1 = self.pool(N1, NB1, H1, HD1, x1T, x1aug, mask1, deg1c,
                                  attx1, w1bc, lew1, scal["cst1"], scal["le_b1_1"],
                                  scal["le_b3_1"], negleb3_1, "1", final=False)

        if stop_after == "pool1":
            return bail(Pt[:4, 0, 0:1])
        # ---------------- stage-2 glue ----------------
        x2 = self.work.tile([128, NB2, HD1], BF16, tag="E")
        mask1T = self.work.tile([128, NB1, N1], BF16, tag="bigshare")
        nc.sync.dma_start(out=mask1T[:],
                          in_=self.din["mask1T"].rearrange("(b p) t -> p b t", p=128))
        x2T = self.work.tile([128, 3, N2], BF16, tag="x2T")
        Sb = self.work.tile([128, NB1, N2], BF16, tag="kT")
        Tb = self.work.tile([128, NB1, N2], BF16, tag="vaug")
        mask2 = self.big.tile([128, NB2, N2], BF16, tag="mask2")
        mask2b = self.big.tile([128, NB2, N2], BF16, tag="mask2b")
        deg2c = self.sm.tile([128, NB2, 1], F32, tag="deg2c")
        with tc.tile_pool(name="g2s", bufs=2, space="PSUM") as pG:
            for qb in range(NB2):
                px2 = pG.tile([128, HD1], F32, tag="X")
                for tb in range(NB1):
                    nc.tensor.matmul(px2[:], Pt[:, tb, qb * 128:(qb + 1) * 128],
                                     xnew1[:, tb],
                                     start=(tb == 0), stop=(tb == NB1 - 1))
                nc.scalar.copy(out=x2[:, qb], in_=px2[:])
            identb = self.sm.tile([128, 128], BF16, tag="identb")
            nc.vector.tensor_copy(out=identb[:], in_=self.identity[:])
            for qb in range(NB2):
                for m, ob in enumerate(KBS2):
                    pt = pG.tile([128, 128], BF16, tag="T")
                    nc.tensor.transpose(pt[:ob, :],
                                        x2[:, qb, m * 128: m * 128 + ob],
                                        identb[:])
                    nc.scalar.copy(out=x2T[:ob, m, qb * 128:(qb + 1) * 128],
                                   in_=pt[:ob, :])
            for dst, rhs in ((Sb, Pb), (Tb, Sb)):
                for sb in range(NB1):
                    pp = pG.tile([128, N2], F32, tag="G")
                    for tb in range(NB1):
                        nc.tensor.matmul(pp[:], mask1T[:, tb, sb * 128:(sb + 1) * 128],
                                         rhs[:, tb],
                                         start=(tb == 0), stop=(tb == NB1 - 1))
                    nc.scalar.copy(out=dst[:, sb], in_=pp[:])
            for pb in range(NB2):
                pc = pG.tile([128, N2], F32, tag="G")
                for sb in range(NB1):
                    nc.tensor.matmul(pc[:], Sb[:, sb, pb * 128:(pb + 1) * 128],
                                     Tb[:, sb], start=(sb == 0), stop=(sb == NB1 - 1))
                m2f = self.sm.tile([128, N2], F32, tag="m2f")
                nc.vector.tensor_scalar(out=m2f[:], in0=pc[:], scalar1=0.5,
                                        scalar2=None, op0=ALU.is_gt)
                ne = self.sm.tile([128, N2], F32, tag="m2ne")
                nc.vector.tensor_scalar(out=ne[:], in0=self.iotabc[:, :N2],
                                        scalar1=self.iotac[:, pb], scalar2=None,
                                        op0=ALU.not_equal)
                nc.vector.tensor_tensor(out=m2f[:], in0=m2f[:], in1=ne[:], op=ALU.mult)
                nc.vector.tensor_copy(out=mask2[:, pb], in_=m2f[:])
                nc.vector.tensor_scalar(out=ne[:], in0=self.iotabc[:, :N2],
                                        scalar1=self.iotac[:, pb], scalar2=None,
                                        op0=ALU.is_equal)
                nc.vector.tensor_tensor(out=m2f[:], in0=m2f[:], in1=ne[:], op=ALU.max)
                nc.vector.tensor_copy(out=mask2b[:, pb], in_=m2f[:])
            onecb = self.sm.tile([128, 1], BF16, tag="onecb")
            nc.vector.memset(onecb[:], 1.0)
            pdg = pG.tile([1, N2], F32, tag="Gd", bufs=1)
            for pb in range(NB2):
                nc.tensor.matmul(pdg[:], onecb[:], mask2b[:, pb],
                                 start=(pb == 0), stop=(pb == NB2 - 1))
            degrow = self.sm.tile([1, N2], F32, tag="degrow")
            nc.scalar.copy(out=degrow[:], in_=pdg[:])
            for qb in range(NB2):
                ptd = pG.tile([128, 1], F32, tag="Gt", bufs=1)
                nc.tensor.transpose(ptd[:, :], degrow[:, qb * 128:(qb + 1) * 128],
                                    self.identity[:1, :1])
                nc.scalar.copy(out=deg2c[:, qb], in_=ptd[:])

        if stop_after == "glue":
            return bail(deg2c[:4, 0])
        # ---------------- stage 2 ----------------
        wq2 = wload("wq2", 3, HD2, dtype=BF16); wk2 = wload("wk2", 3, HD2, dtype=BF16)
        wv2a = wload("wv2a", 3, H2 * (DH + 1), dtype=BF16); wsk2 = wload("wsk2", 3, HD2, dtype=BF16)
        bq2c = wload("bq2c", 2, 1); bk2c = wload("bk2c", 2, 1)
        bvsk2c = wload("bvsk2c", H2, 1, p=DH)
        attx2 = wload("attx2", H2, 1, p=DH); lew2 = wload("lew2", H2, 3, p=DH)
        w2bc = self.load("w2bc", [128, HD2])

        x3T = self.conv(x2T, KBS2, N2, NB2, H2, [128, 64],
                        wq2, wk2, wv2a, wsk2, bq2c, bk2c, bvsk2c, mask2, "2")
        if stop_after == "conv2":
            return bail(x3T[:4, 0, 0:1])
        x3aug = self.transpose_to_aug(x3T, N2, NB2, H2, HD2, "bigshare")
        gmrow = self.pool(N2, NB2, H2, HD2, x3T, x3aug, mask2b, deg2c,
                          attx2, w2bc, lew2, scal["cst2"], scal["le_b1_2"],
                          scal["le_b3_2"], negleb3_2, "2", final=True)

        if stop_after == "pool2":
            outc = self.sm.tile([4, 1], F32, tag="outc")
            nc.vector.memset(outc[:], 0.0)
            nc.vector.tensor_copy(out=outc[0:1, :], in_=gmrow[0:1, 0:1])
            nc.sync.dma_start(out=out_d, in_=outc[:])
            return
        # ---------------- MLP ----------------
        mw1 = wload("mw1", 2, HD1); mw2 = wload("mw2", 3, HD1); mw3 = wload("mw3", 3, 4)
        mb1c = wload("mb1c", 3, 1); mb2c = wload("mb2c", 3, 1)
        mb3c = self.load("mb3c", [4, 1])
        obs3 = [128, 128, 64]
        with tc.tile_pool(name="mlpp", bufs=2, space="PSUM") as pM:
            merge = self.sm.tile([128, 2, 1], F32, tag="merge")
            pm0 = pM.tile([128, 1], F32, tag="Mt")
            nc.tensor.transpose(pm0[:, :], gmrow[:, :128], self.identity[:1, :1])
            nc.scalar.copy(out=merge[:, 0], in_=pm0[:])
            pm1 = pM.tile([128, 1], F32, tag="Mt")
            nc.tensor.transpose(pm1[:64, :], gmrow[:, 128:192], self.identity[:1, :1])
            nc.scalar.copy(out=merge[:64, 1], in_=pm1[:64, :])
            nc.sync.dma_start(out=merge[64:69, 1], in_=self.din["evcd"])
            kbs1 = [128, 69]
            h1 = self.sm.tile([128, 3, 1], F32, tag="h1col")
            for m in range(3):
                ph = pM.tile([128, 1], F32, tag="Mm")
                for kb in range(2):
                    nc.tensor.matmul(ph[: obs3[m], :],
                                     _r(mw1[: kbs1[kb], kb, m * 128: m * 128 + obs3[m]]),
                                     _r(merge[: kbs1[kb], kb]),
                                     start=(kb == 0), stop=(kb == 1))
                nc.scalar.activation(out=h1[: obs3[m], m], in_=ph[: obs3[m], :],
                                     func=AF.Relu, bias=mb1c[: obs3[m], m])
            h2 = self.sm.tile([128, 3, 1], F32, tag="h2col")
            for m in range(3):
                ph = pM.tile([128, 1], F32, tag="Mm")
                for kb in range(3):
                    nc.tensor.matmul(ph[: obs3[m], :],
                                     _r(mw2[: obs3[kb], kb, m * 128: m * 128 + obs3[m]]),
                                     _r(h1[: obs3[kb], kb]),
                                     start=(kb == 0), stop=(kb == 2))
                nc.scalar.activation(out=h2[: obs3[m], m], in_=ph[: obs3[m], :],
                                     func=AF.Relu, bias=mb2c[: obs3[m], m])
            po = pM.tile([128, 1], F32, tag="Mo")
            for kb in range(3):
                nc.tensor.matmul(po[:4, 0:1], _r(mw3[: obs3[kb], kb, :]),
                                 _r(h2[: obs3[kb], kb]),
                                 start=(kb == 0), stop=(kb == 2))
            outc = self.sm.tile([4, 1], F32, tag="outc")
            nc.vector.tensor_tensor(out=outc[:], in0=po[:4, 0:1], in1=mb3c[:],
                                    op=ALU.add)
            nc.sync.dma_start(out=out_d, in_=outc[:])


# ======================================================================
# host side
# ======================================================================

_CACHE = {}


def _pad_rows(a, rows):
    out = np.zeros((rows, a.shape[1]), np.float32)
    out[: a.shape[0]] = a
    return out


def _prep_shared(inputs):
    tc1, tc2 = inputs["tc1"], inputs["tc2"]
    p1, p2 = inputs["pool1"], inputs["pool2"]
    mlp = inputs["mlp"]
    f = lambda a: np.asarray(a, np.float32)

    def vaug_pack(Wv, heads):
        fin = Wv.shape[0]
        out = np.zeros((fin, heads * (DH + 1)), np.float32)
        for h in range(heads):
            out[:, h * (DH + 1): h * (DH + 1) + DH] = Wv[:, h * DH:(h + 1) * DH]
        return out

    d = {}
    d["wq1"] = _pad_rows(f(tc1["Wq"]), 128)
    d["wk1"] = _pad_rows(f(tc1["Wk"]), 128)
    d["wv1a"] = _pad_rows(vaug_pack(f(tc1["Wv"]), H1), 128)
    d["wsk1"] = _pad_rows(f(tc1["Wskip"]), 128)
    d["bq1c"] = _pad_rows(f(tc1["bq"])[:, None], 3 * 128)
    d["bk1c"] = _pad_rows(f(tc1["bk"])[:, None], 3 * 128)
    d["bvsk1c"] = (f(tc1["bv"]) + f(tc1["bskip"]))[:, None].copy()
    d["attx1"] = f(p1["att_x"])[:, None].copy()
    w1 = f(p1["Wlin"]) @ f(p1["att_q"])
    d["w1bc"] = np.tile(w1[None, :], (128, 1)).astype(np.float32)
    d["lew1"] = np.stack([f(p1["le_W1"])[:, 0], f(p1["le_W2"])[:, 0],
                          f(p1["le_W3"])[:, 0]], axis=1).astype(np.float32)
    d["wq2"] = _pad_rows(f(tc2["Wq"]), 3 * 128)
    d["wk2"] = _pad_rows(f(tc2["Wk"]), 3 * 128)
    d["wv2a"] = _pad_rows(vaug_pack(f(tc2["Wv"]), H2), 3 * 128)
    d["wsk2"] = _pad_rows(f(tc2["Wskip"]), 3 * 128)
    d["bq2c"] = _pad_rows(f(tc2["bq"])[:, None], 2 * 128)
    d["bk2c"] = _pad_rows(f(tc2["bk"])[:, None], 2 * 128)
    d["bvsk2c"] = (f(tc2["bv"]) + f(tc2["bskip"]))[:, None].copy()
    d["attx2"] = f(p2["att_x"])[:, None].copy()
    w2 = f(p2["Wlin"]) @ f(p2["att_q"])
    d["w2bc"] = np.tile(w2[None, :], (128, 1)).astype(np.float32)
    d["lew2"] = np.stack([f(p2["le_W1"])[:, 0], f(p2["le_W2"])[:, 0],
                          f(p2["le_W3"])[:, 0]], axis=1).astype(np.float32)
    d["mw1"] = _pad_rows(f(mlp["W1"]), 2 * 128)
    d["mw2"] = _pad_rows(f(mlp["W2"]), 3 * 128)
    d["mw3"] = _pad_rows(f(mlp["W3"]), 3 * 128)
    d["mb1c"] = _pad_rows(f(mlp["b1"])[:, None], 3 * 128)
    d["mb2c"] = _pad_rows(f(mlp["b2"])[:, None], 3 * 128)
    d["mb3c"] = f(mlp["b3"])[:, None].copy()
    d["iotabc"] = np.tile(np.arange(N1, dtype=np.float32)[None, :], (128, 1))
    d["iotac"] = np.arange(N1, dtype=np.float32)[:, None]
    scal = {
        "cst1": float(f(p1["blin"]) @ f(p1["att_q"]) + f(p1["att_b"])),
        "cst2": float(f(p2["blin"]) @ f(p2["att_q"]) + f(p2["att_b"])),
        "le_b1_1": float(f(p1["le_b1"])[0]), "le_b3_1": float(f(p1["le_b3"])[0]),
        "le_b1_2": float(f(p2["le_b1"])[0]), "le_b3_2": float(f(p2["le_b3"])[0]),
    }
    return scal, d


def make_in_maps(inputs):
    import ml_dtypes
    BFH = ml_dtypes.bfloat16
    nodes = np.asarray(inputs["nodes"], np.float32)
    ei = np.asarray(inputs["edge_index"])
    ev = np.asarray(inputs["exp_value"], np.float32)
    cd = np.asarray(inputs["circuit_depth"], np.float32)
    scal, shared = _prep_shared(inputs)
    for w in ["wq1", "wk1", "wv1a", "wsk1", "wq2", "wk2", "wv2a", "wsk2"]:
        shared[w] = shared[w].astype(BFH)
    src, dst = ei[0], ei[1]
    gid = src // N1
    in_maps = []
    for b in range(B):
        m = gid == b
        A = np.zeros((N1, N1), bool)
        A[src[m] % N1, dst[m] % N1] = True
        mask1 = A.copy()
        np.fill_diagonal(mask1, True)
        xg = nodes[b * N1:(b + 1) * N1]
        im = dict(shared)
        im["xT0"] = np.ascontiguousarray(xg.T).astype(ml_dtypes.bfloat16)
        im["mask1c"] = A.astype(ml_dtypes.bfloat16)
        im["mask1"] = mask1.astype(ml_dtypes.bfloat16)
        im["mask1T"] = np.ascontiguousarray(mask1.T).astype(ml_dtypes.bfloat16)
        im["deg1c"] = mask1.sum(axis=0, dtype=np.float32)[:, None]
        im["evcd"] = np.concatenate([ev[b, 0], cd[b]])[:, None].astype(np.float32)
        in_maps.append(im)
    return scal, in_maps


def kernel(**inputs):
    scal, in_maps = make_in_maps(inputs)
    key = tuple(sorted(scal.items()))
    if key not in _CACHE:
        _CACHE[key] = build_program(scal)
    nc = _CACHE[key]
    res = run_bass_kernel_spmd(nc, in_maps, list(range(B)))
    out = np.stack([res.results[i]["out"][:, 0] for i in range(B)])
    return out.astype(np.float32)
